# revision 1
# baseline (speedup 1.0000x reference)
"""DiT block kernel for Trainium2 (Bass/Tile), 8-core data parallel.

Shapes (hardcoded from the problem spec):
  x: (8, 1024, 1152), t_emb: (8, 1152)
  w_qkv (1152, 3456), w_proj (1152, 1152), w_fc1 (1152, 4608),
  w_fc2 (4608, 1152), w_ada (1152, 6912) + biases.

Strategy: batch-parallel across 8 cores (one batch element each, no
collectives). Inside a core, activations live in "transposed" layout
[D on partitions, tokens on free] so every projection is
out_T = W.T @ x_T with lhsT = W exactly as stored in DRAM.
LayerNorm statistics reduce over the partition (D) axis via ones-vector
matmuls; softmax runs in transposed orientation (keys on partitions,
no max subtraction -- scores are bounded ~+-8 here) with sums collected
through a ones-column appended to V. Matmuls run in float32r (full PE
rate at N>=256) except attention/proj/fc2 which run in bf16.
"""

import os
import threading
from contextlib import ExitStack

import numpy as np

import concourse.bass as bass
import concourse.mybir as mybir
import concourse.tile as tile
from concourse import bacc
from concourse.bass_utils import run_bass_kernel_spmd
from concourse.masks import make_identity

F32 = mybir.dt.float32
F32R = mybir.dt.float32r
BF16 = mybir.dt.bfloat16
AF = mybir.ActivationFunctionType
ALU = mybir.AluOpType

NCORES = 8
D = 1152
NT = 1024          # tokens per core (batch element)
KT = D // 128      # 9 partition-tiles of D
H = 16
HD = 72
HID = 4 * D        # 4608
MQK = (2 * D) // 128   # 18 output tiles for q,k
MH = HID // 128        # 36
EPS = 1e-6
ISC = 1.0 / float(np.sqrt(HD))

# v output column slices aligned to head boundaries (each >=256 for f32r)
V_SLICES = [(0, 432, 0, 6), (432, 864, 6, 12), (864, 1152, 12, 16)]


def _r(ap):
    return ap.bitcast(F32R)


def _head_segs(d0, n):
    """Split logical rows [d0, d0+n) of a [*,128]-tiled stacked tensor into
    (ktile, part0, length, dst_offset) segments within 128-partition tiles."""
    segs = []
    off = 0
    while n > 0:
        kt_i, p0 = divmod(d0, 128)
        ln = min(n, 128 - p0)
        segs.append((kt_i, p0, ln, off))
        d0 += ln
        off += ln
        n -= ln
    return segs


def _build_program():
    nc = bacc.Bacc(
        "TRN2", target_bir_lowering=False, debug=False, enable_asserts=False
    )
    ins = {}
    ins["x"] = nc.dram_tensor("x", [NT, D], F32, kind="ExternalInput").ap()
    ins["t_emb"] = nc.dram_tensor("t_emb", [D], F32, kind="ExternalInput").ap()
    for name, shape in [
        ("w_qkv", [D, 3 * D]), ("b_qkv", [3 * D]),
        ("w_proj", [D, D]), ("b_proj", [D]),
        ("w_fc1", [D, HID]), ("b_fc1", [HID]),
        ("w_fc2", [HID, D]), ("b_fc2", [D]),
        ("w_ada", [D, 6 * D]), ("b_ada", [6 * D]),
    ]:
        ins[name] = nc.dram_tensor(name, shape, F32, kind="ExternalInput").ap()
    out_dram = nc.dram_tensor("out", [NT, D], F32, kind="ExternalOutput").ap()

    with tile.TileContext(nc) as tc:
        _body(tc, ins, out_dram)
    nc.compile()
    return nc


def _ln_stats_and_modulate(tc, nc, src, dst, ada_pp, shift_c, scale_c,
                           ones_col, pst, pln, ps_st):
    """dst[:,k,:] = ((src-mean)*rstd) * ada_pp[:,scale_c,k] + ada_pp[:,shift_c,k]
    (mean/rstd over the partition (D) axis per token; scale_c holds 1+scale;
    dst is bf16). Stats for both 512-token halves are emitted first so the
    PE finishes them early; applies follow per half so downstream matmuls
    on half 0 can start while half 1 is still being modulated."""
    ps_x, ps_q, st = {}, {}, {}
    for n in range(2):
        nsl = slice(n * 512, (n + 1) * 512)
        ps_x[n] = ps_st.tile([1, 512], F32, tag="st", name=f"psx{n}")
        ps_q[n] = ps_st.tile([1, 512], F32, tag="st", name=f"psq{n}")
        for k in range(KT):
            xb = pln.tile([128, 512], BF16, tag="xb", bufs=3, name="xb")
            nc.scalar.copy(xb[:, :], src[:, k, nsl])
            sq_b = pln.tile([128, 512], BF16, tag="sqb", bufs=3, name="sq_b")
            nc.vector.tensor_mul(sq_b[:, :], src[:, k, nsl], src[:, k, nsl])
            nc.tensor.matmul(
                ps_x[n][:, :], ones_col[:, :], xb[:, :],
                start=(k == 0), stop=(k == KT - 1), skip_group_check=True,
            )
            nc.tensor.matmul(
                ps_q[n][:, :], ones_col[:, :], sq_b[:, :],
                start=(k == 0), stop=(k == KT - 1), skip_group_check=True,
            )
    eps_sb = pst.tile([1, 1], F32, tag="eps", bufs=1, name="eps_sb")
    nc.vector.memset(eps_sb[:, :], EPS)
    for n in range(2):
        # rows: 0 = mean, 1 = E[x^2] -> rstd
        st[n] = pst.tile([1, 2, 512], F32, tag="lnst", bufs=2, name=f"st{n}")
        nc.vector.tensor_scalar_mul(st[n][:, 0, :], ps_x[n][:, :], 1.0 / D)
        work = pst.tile([1, 512], F32, tag="lnwork", bufs=2, name="work")
        nc.vector.tensor_mul(work[:, :], st[n][:, 0, :], st[n][:, 0, :])
        nc.vector.scalar_tensor_tensor(
            st[n][:, 1, :], ps_q[n][:, :], 1.0 / D, work[:, :],
            ALU.mult, ALU.subtract,
        )
        nc.scalar.activation(st[n][:, 1, :], st[n][:, 1, :], AF.Sqrt,
                             bias=eps_sb[:, :], scale=1.0)
        nc.vector.reciprocal(st[n][:, 1, :], st[n][:, 1, :])
    for n in range(2):
        nsl = slice(n * 512, (n + 1) * 512)
        meanB = pln.tile([128, 512], F32, tag="meanB", bufs=2, name="meanB")
        rstdB = pln.tile([128, 512], F32, tag="rstdB", bufs=2, name="rstdB")
        nc.gpsimd.partition_broadcast(meanB[:, :], st[n][:, 0, :])
        nc.gpsimd.partition_broadcast(rstdB[:, :], st[n][:, 1, :])
        for k in range(KT):
            tmp = pln.tile([128, 512], F32, tag="lnt", bufs=3, name="tmp")
            nc.vector.tensor_sub(tmp[:, :], src[:, k, nsl], meanB[:, :])
            nc.vector.tensor_mul(tmp[:, :], tmp[:, :], rstdB[:, :])
            nc.scalar.activation(
                dst[:, k, nsl], tmp[:, :], AF.Identity,
                bias=ada_pp[:, shift_c, k:k + 1],
                scale=ada_pp[:, scale_c, k:k + 1],
            )


def _truncate_out(tc, nc, out_dram):
    with tc.tile_pool(name="ptrunc", bufs=1) as p:
        z = p.tile([128, D], F32, name="z")
        nc.vector.memset(z[:, :], 0.0)
        for tt in range(NT // 128):
            nc.sync.dma_start(out_dram[tt * 128:(tt + 1) * 128, :], z[:, :])


def _body(tc, ins, out_dram):
    nc = tc.nc
    phase_limit = float(os.environ.get("BASS_PHASES", "6"))
    ctx = ExitStack()
    with ctx:
        dram = ctx.enter_context(tc.tile_pool(name="dram", bufs=1, space="DRAM"))
        ada_dr = dram.tile([6 * D], F32)
        w2_dr = dram.tile([HID, D], BF16)

        pers = ctx.enter_context(tc.tile_pool(name="pers", bufs=1))
        ident = pers.tile([128, 128], F32)
        make_identity(nc, ident[:, :])
        ones_col = pers.tile([128, 1], BF16)
        nc.vector.memset(ones_col[:, :], 1.0)
        t_pp = pers.tile([128, KT], F32)
        nc.sync.dma_start(t_pp[:, :], ins["t_emb"].rearrange("(k p) -> p k", p=128))
        t_pb = pers.tile([128, KT], BF16)
        nc.scalar.activation(t_pb[:, :], t_pp[:, :], AF.Silu)

        bqk_pp = pers.tile([128, MQK], F32)
        bv_pp = pers.tile([72, H], F32)
        bproj_pp = pers.tile([128, KT], F32)
        bfc1_pp = pers.tile([128, MH], F32)
        bfc2_pp = pers.tile([128, KT], F32)
        bada_pp = pers.tile([128, 6, KT], F32)
        ada_pp = pers.tile([128, 6, KT], F32)

        def emit_bias_loads():
            nc.sync.dma_start(
                bqk_pp[:, :],
                ins["b_qkv"][0:2 * D].rearrange("(m p) -> p m", p=128))
            nc.sync.dma_start(
                bv_pp[:, :],
                ins["b_qkv"][2 * D:3 * D].rearrange("(h p) -> p h", p=72))
            nc.sync.dma_start(
                bproj_pp[:, :], ins["b_proj"].rearrange("(m p) -> p m", p=128))
            nc.sync.dma_start(
                bfc1_pp[:, :], ins["b_fc1"].rearrange("(m p) -> p m", p=128))
            nc.sync.dma_start(
                bfc2_pp[:, :], ins["b_fc2"].rearrange("(m p) -> p m", p=128))
            nc.sync.dma_start(
                bada_pp[:, :, :],
                ins["b_ada"].rearrange("(c k p) -> p c k", k=KT, p=128))
        xT = pers.tile([128, KT, NT], F32)   # becomes x2T after residual 1
        # weight-stream pool spanning all phases: lets the scheduler prefetch
        # the next phase's weights across pool boundaries
        pw_s = ctx.enter_context(tc.tile_pool(name="pw_s", bufs=1))

        # ============ phase 1: ada, x load+transpose, LN1 ====================
        es_mod1 = ExitStack()
        pmod1 = es_mod1.enter_context(tc.tile_pool(name="pmod1", bufs=1))
        mod1T = pmod1.tile([128, KT, NT], BF16, name="mod1T")

        with tc.tile_pool(name="p1w", bufs=1) as p1w, \
             tc.tile_pool(name="pst", bufs=1) as pst, \
             tc.tile_pool(name="pln", bufs=1) as pln:
            with tc.tile_pool(name="ps_pro", bufs=2, space="PSUM") as ps_pro, \
                 tc.tile_pool(name="pxin", bufs=3) as pxin, \
                 tc.tile_pool(name="ps_tr", bufs=2, space="PSUM") as ps_tr:

                def emit_transpose_block(tt):
                    xin = pxin.tile([128, D], F32, tag="xin", name="xin")
                    nc.sync.dma_start(
                        xin[:, :], ins["x"][tt * 128:(tt + 1) * 128, :])
                    for kd in range(KT):
                        pt = ps_tr.tile([128, 128], F32, tag="ptr", name="pt")
                        nc.tensor.transpose(
                            pt[:, :], xin[:, kd * 128:(kd + 1) * 128],
                            ident[:, :],
                        )
                        tsl = slice(tt * 128, (tt + 1) * 128)
                        if kd % 2 == 0:
                            nc.vector.tensor_copy(xT[:, kd, tsl], pt[:, :])
                        else:
                            nc.scalar.copy(xT[:, kd, tsl], pt[:, :])

                def emit_ada_chunk(n):
                    pa = ps_pro.tile([1, 384], F32, tag="psada", name="pa")
                    for k in range(KT):
                        wada_t = p1w.tile([128, 384], F32, tag="wsk", bufs=4,
                                          name="wada_t")
                        nc.sync.dma_start(
                            wada_t[:, :],
                            ins["w_ada"][k * 128:(k + 1) * 128,
                                         n * 384:(n + 1) * 384],
                        )
                        wada_b = p1w.tile([128, 384], BF16, tag="wskb", bufs=4,
                                          name="wada_b")
                        if k % 2 == 0:
                            nc.gpsimd.tensor_copy(wada_b[:, :], wada_t[:, :])
                        else:
                            nc.vector.tensor_copy(wada_b[:, :], wada_t[:, :])
                        nc.tensor.matmul(
                            pa[:, :], t_pb[:, k:k + 1], wada_b[:, :],
                            start=(k == 0), stop=(k == KT - 1),
                        )
                    asb = pst.tile([1, 384], F32, tag="asb", bufs=2, name="asb")
                    nc.vector.tensor_copy(asb[:, :], pa[:, :])
                    nc.sync.dma_start(
                        ada_dr[n * 384:(n + 1) * 384]
                        .rearrange("(a b) -> a b", a=1),
                        asb[0:1, :],
                    )

                # interleave: ada chunk n and transpose block(s) alternate so
                # the PE fills DMA wait time of one with the other
                for i in range(8):
                    emit_transpose_block(i)
                    if i < 6:
                        emit_ada_chunk(i)
                    if i == 0:
                        emit_bias_loads()
                for c in range(2):
                    nc.sync.dma_start(
                        ada_pp[:, c, :],
                        ada_dr[c * D:(c + 1) * D].rearrange("(k p) -> p k", p=128),
                    )
                nc.vector.tensor_add(ada_pp[:, 0:2, :], ada_pp[:, 0:2, :],
                                     bada_pp[:, 0:2, :])
                nc.vector.tensor_scalar_add(ada_pp[:, 1, :], ada_pp[:, 1, :], 1.0)

            if phase_limit > 0.6:
              with tc.tile_pool(name="ps_st", bufs=4, space="PSUM") as ps_st, \
                 tc.tile_pool(name="ps_bc", bufs=2, space="PSUM") as ps_bc:
                _ln_stats_and_modulate(
                    tc, nc, xT, mod1T, ada_pp, 0, 1, ones_col,
                    pst, pln, ps_st,
                )

        if phase_limit <= 1:
            es_mod1.close()
            return _truncate_out(tc, nc, out_dram)

        # ============ phase 2: qkv ==========================================
        es_qkv = ExitStack()
        pqks = es_qkv.enter_context(tc.tile_pool(name="pqks", bufs=1, side="right"))
        qk_st = pqks.tile([128, MQK, NT], BF16, name="qk_st")
        pvaug = es_qkv.enter_context(
            tc.tile_pool(name="pvaug", bufs=1, side="right"))
        # per head: cols 0..72 = v, col 96 = ones (sum row lands on an
        # aligned PSUM partition), cols 72..96 zero padding
        v_aug = pvaug.tile([128, NT // 128, H, 97], BF16, name="v_aug")
        nc.gpsimd.memset(v_aug[:, :, :, HD:97], 0.0)
        nc.gpsimd.memset(v_aug[:, :, :, 96:97], 1.0)

        with tc.tile_pool(name="p2w", bufs=1) as p2w, \
             tc.tile_pool(name="ps_mm", bufs=4, space="PSUM") as ps_mm:
            for mo in range(MQK):
                wqk_t = pw_s.tile([128, KT, 128], F32, tag="ws", bufs=3,
                                  name="wqk_t")
                nc.sync.dma_start(
                    wqk_t[:, :, :],
                    ins["w_qkv"][:, mo * 128:(mo + 1) * 128]
                    .rearrange("(k p) m -> p k m", p=128),
                )
                wqk_b = pw_s.tile([128, KT, 128], BF16, tag="wsb", bufs=3,
                                  name="wqk_b")
                nc.gpsimd.tensor_copy(wqk_b[:, :, :], wqk_t[:, :, :])
                for n in range(2):
                    pm = ps_mm.tile([128, 512], F32, tag="mm", name="pm")
                    for k in range(KT):
                        nc.tensor.matmul(
                            pm[:, :], wqk_b[:, k, :],
                            mod1T[:, k, n * 512:(n + 1) * 512],
                            start=(k == 0), stop=(k == KT - 1),
                        )
                    nc.scalar.activation(
                        qk_st[:, mo, n * 512:(n + 1) * 512], pm[:, :],
                        AF.Identity, bias=bqk_pp[:, mo:mo + 1], scale=1.0,
                    )
            for (c0, c1, h0, h1) in V_SLICES:
                wv_t = p2w.tile([128, KT, 432], F32, tag="wv", bufs=2,
                                name="wv_t")
                nc.sync.dma_start(
                    wv_t[:, :, 0:c1 - c0],
                    ins["w_qkv"][:, 2 * D + c0:2 * D + c1]
                    .rearrange("(k p) m -> p k m", p=128),
                )
                wv_b = p2w.tile([128, KT, 432], BF16, tag="wvb", bufs=2,
                                name="wv_b")
                nc.gpsimd.tensor_copy(wv_b[:, :, 0:c1 - c0], wv_t[:, :, 0:c1 - c0])
                for tt in range(NT // 128):
                    pmv = ps_mm.tile([128, 512], F32, tag="mm", name="pmv")
                    for k in range(KT):
                        nc.tensor.matmul(
                            pmv[:, 0:c1 - c0],
                            mod1T[:, k, tt * 128:(tt + 1) * 128],
                            wv_b[:, k, 0:c1 - c0],
                            start=(k == 0), stop=(k == KT - 1),
                        )
                    for h in range(h0, h1):
                        nc.vector.tensor_copy(
                            v_aug[:, tt, h, 0:HD],
                            pmv[:, h * HD - c0:(h + 1) * HD - c0],
                        )
        es_mod1.close()
        if phase_limit <= 2:
            es_qkv.close()
            return _truncate_out(tc, nc, out_dram)

        # ============ phase 3: attention ====================================
        es_ao = ExitStack()
        pastk = es_ao.enter_context(tc.tile_pool(name="pastk", bufs=1))
        attn_st = pastk.tile([128, KT, NT], BF16, name="attn_st")

        with tc.tile_pool(name="pheads", bufs=2) as pheads, \
             tc.tile_pool(name="pexp", bufs=3) as pexp, \
             tc.tile_pool(name="pattn", bufs=2) as pattn, \
             tc.tile_pool(name="p3w", bufs=1) as p3w, \
             tc.tile_pool(name="ps_s", bufs=3, space="PSUM") as ps_s, \
             tc.tile_pool(name="ps_av", bufs=4, space="PSUM") as ps_av:
            ps_a3 = ps_s  # [1,384] ada tiles share the pool (own tag, 1 buf)

            def emit_late_ada_chunk(n):
                pa = ps_a3.tile([1, 384], F32, tag="psada3", bufs=1,
                                name="pa3")
                for k in range(KT):
                    wada_t = p3w.tile([128, 384], F32, tag="wsk", bufs=4,
                                      name="wada_t3")
                    nc.sync.dma_start(
                        wada_t[:, :],
                        ins["w_ada"][k * 128:(k + 1) * 128,
                                     n * 384:(n + 1) * 384],
                    )
                    wada_b = p3w.tile([128, 384], BF16, tag="wskb", bufs=3,
                                      name="wada_b3")
                    nc.gpsimd.tensor_copy(wada_b[:, :], wada_t[:, :])
                    nc.tensor.matmul(
                        pa[:, :], t_pb[:, k:k + 1], wada_b[:, :],
                        start=(k == 0), stop=(k == KT - 1),
                    )
                asb = p3w.tile([1, 384], F32, tag="asb", bufs=1, name="asb3")
                nc.vector.tensor_copy(asb[:, :], pa[:, :])
                nc.sync.dma_start(
                    ada_dr[n * 384:(n + 1) * 384]
                    .rearrange("(a b) -> a b", a=1),
                    asb[0:1, :],
                )

            def emit_w2_convert(k):
                w2src = p3w.tile([128, D], F32, tag="w2src", bufs=2,
                                 name="w2src")
                nc.sync.dma_start(
                    w2src[:, :], ins["w_fc2"][k * 128:(k + 1) * 128, :]
                )
                w2b = p3w.tile([128, D], BF16, tag="w2b", bufs=2, name="w2b")
                nc.vector.tensor_copy(w2b[:, :], w2src[:, :])
                nc.sync.dma_start(w2_dr[k * 128:(k + 1) * 128, :], w2b[:, :])

            def emit_filler(h):
                # spread late-ada (12 chunks) and w2 conversion (36 blocks)
                # across the 16 head iterations
                if h < 12:
                    emit_late_ada_chunk(6 + h)
                if h == 11:
                    for c in range(2, 6):
                        nc.sync.dma_start(
                            ada_pp[:, c, :],
                            ada_dr[c * D:(c + 1) * D]
                            .rearrange("(k p) -> p k", p=128),
                        )
                    nc.vector.tensor_add(ada_pp[:, 2:6, :], ada_pp[:, 2:6, :],
                                         bada_pp[:, 2:6, :])
                    nc.vector.tensor_scalar_add(ada_pp[:, 4, :],
                                                ada_pp[:, 4, :], 1.0)
                for k2 in range((h * 36) // H, ((h + 1) * 36) // H):
                    emit_w2_convert(k2)

            for h in range(H):
                emit_filler(h)
                q_h = pheads.tile([72, NT], BF16, tag="qh", name="q_h")
                k_h = pheads.tile([72, NT], BF16, tag="kh", name="k_h")
                for (kt_i, p0, ln, off) in _head_segs(h * HD, HD):
                    nc.sync.dma_start(
                        q_h[off:off + ln, :], qk_st[p0:p0 + ln, kt_i, :]
                    )
                for (kt_i, p0, ln, off) in _head_segs(D + h * HD, HD):
                    nc.sync.dma_start(
                        k_h[off:off + ln, :], qk_st[p0:p0 + ln, kt_i, :]
                    )
                attn_f = pattn.tile([72, NT], F32, tag="attnf", bufs=1,
                                    name="attn_f")
                attn_h = pattn.tile([72, NT], BF16, tag="attnh", name="attn_h")
                for n in range(2):
                    nsl = slice(n * 512, (n + 1) * 512)
                    exp_hn = pexp.tile([128, NT // 128, 512], BF16, tag="exp",
                                       bufs=3, name="exp_hn")
                    for kt_i in range(NT // 128):
                        pss = ps_s.tile([128, 512], F32, tag="s", name="pss")
                        nc.tensor.matmul(
                            pss[:, :], k_h[:, kt_i * 128:(kt_i + 1) * 128],
                            q_h[:, nsl], start=True, stop=True,
                        )
                        nc.scalar.activation(
                            exp_hn[:, kt_i, :], pss[:, :], AF.Exp, scale=ISC
                        )
                    pav = ps_av.tile([97, 512], F32, tag="av", name="pav")
                    for kt_i in range(NT // 128):
                        nc.tensor.matmul(
                            pav[:, :], v_aug[:, kt_i, h, :], exp_hn[:, kt_i, :],
                            start=(kt_i == 0), stop=(kt_i == NT // 128 - 1),
                        )
                    recip = pattn.tile([1, 512], F32, tag="recip", bufs=2,
                                       name="recip")
                    nc.vector.reciprocal(recip[:, :], pav[96:97, :])
                    bca = pattn.tile([72, 512], F32, tag="bca", name="bca")
                    nc.gpsimd.partition_broadcast(bca[:, :], recip[:, :])
                    nc.vector.tensor_mul(attn_f[:, nsl], pav[0:72, :], bca[:, :])
                    nc.vector.tensor_scalar_add(
                        attn_h[:, nsl], attn_f[:, nsl], bv_pp[:, h:h + 1]
                    )
                for (kt_i, p0, ln, off) in _head_segs(h * HD, HD):
                    nc.sync.dma_start(
                        attn_st[p0:p0 + ln, kt_i, :], attn_h[off:off + ln, :]
                    )
        es_qkv.close()
        if phase_limit <= 3:
            es_ao.close()
            return _truncate_out(tc, nc, out_dram)

        # ============ phase 4: proj + residual1 + LN2 ========================
        es_mod2 = ExitStack()
        pmod2 = es_mod2.enter_context(
            tc.tile_pool(name="pmod2", bufs=1, side="right"))
        mod2T = pmod2.tile([128, KT, NT], BF16, name="mod2T")

        with tc.tile_pool(name="p4w", bufs=1) as p4w, \
             tc.tile_pool(name="pst4", bufs=1) as pst4, \
             tc.tile_pool(name="pln4", bufs=1) as pln4:
            with tc.tile_pool(name="ps_mm2", bufs=4, space="PSUM") as ps_mm2:
                for mo in range(KT):
                    wp_f = pw_s.tile([128, KT, 128], F32, tag="ws", bufs=3,
                                     name="wp_f")
                    nc.sync.dma_start(
                        wp_f[:, :, :],
                        ins["w_proj"][:, mo * 128:(mo + 1) * 128]
                        .rearrange("(k p) m -> p k m", p=128),
                    )
                    wp_b = pw_s.tile([128, KT, 128], BF16, tag="wsb", bufs=3,
                                     name="wp_b")
                    nc.gpsimd.tensor_copy(wp_b[:, :, :], wp_f[:, :, :])
                    for n in range(2):
                        nsl = slice(n * 512, (n + 1) * 512)
                        pm2 = ps_mm2.tile([128, 512], F32, tag="mm2", name="pm2")
                        for k in range(KT):
                            nc.tensor.matmul(
                                pm2[:, :], wp_b[:, k, :], attn_st[:, k, nsl],
                                start=(k == 0), stop=(k == KT - 1),
                            )
                        t_sb = p4w.tile([128, 512], F32, tag="tsb", bufs=2,
                                        name="t_sb")
                        nc.scalar.activation(
                            t_sb[:, :], pm2[:, :], AF.Identity,
                            bias=bproj_pp[:, mo:mo + 1], scale=1.0,
                        )
                        nc.vector.scalar_tensor_tensor(
                            xT[:, mo, nsl], t_sb[:, :], ada_pp[:, 2, mo:mo + 1],
                            xT[:, mo, nsl], ALU.mult, ALU.add,
                        )


            with tc.tile_pool(name="ps_st2", bufs=4, space="PSUM") as ps_st2, \
                 tc.tile_pool(name="ps_bc2", bufs=2, space="PSUM") as ps_bc2:
                _ln_stats_and_modulate(
                    tc, nc, xT, mod2T, ada_pp, 3, 4, ones_col,
                    pst4, pln4, ps_st2,
                )
        es_ao.close()
        if phase_limit <= 4:
            es_mod2.close()
            return _truncate_out(tc, nc, out_dram)

        # ============ phase 5: FFN ==========================================
        es_o = ExitStack()
        po = es_o.enter_context(tc.tile_pool(name="po", bufs=1))
        o_full = po.tile([128, KT, NT], F32, name="o_full")

        with tc.tile_pool(name="p5w", bufs=1) as p5w, \
             tc.tile_pool(name="ph", bufs=1) as ph, \
             tc.tile_pool(name="p5h", bufs=1) as p5h, \
             tc.tile_pool(name="ps_f1", bufs=2, space="PSUM") as ps_f1, \
             tc.tile_pool(name="ps_f2", bufs=4, space="PSUM") as ps_f2:
            hT_sb = p5h.tile([128, MH, NT], BF16, name="hT_sb")
            for mo in range(MH):
                wf1_t = pw_s.tile([128, KT, 128], F32, tag="ws", bufs=3,
                                  name="wf1_t")
                nc.sync.dma_start(
                    wf1_t[:, :, :],
                    ins["w_fc1"][:, mo * 128:(mo + 1) * 128]
                    .rearrange("(k p) m -> p k m", p=128),
                )
                wf1_b = pw_s.tile([128, KT, 128], BF16, tag="wsb", bufs=3,
                                  name="wf1_b")
                nc.gpsimd.tensor_copy(wf1_b[:, :, :], wf1_t[:, :, :])
                for n in range(2):
                    pf1 = ps_f1.tile([128, 512], F32, tag="f1", name="pf1")
                    for k in range(KT):
                        nc.tensor.matmul(
                            pf1[:, :], wf1_b[:, k, :],
                            mod2T[:, k, n * 512:(n + 1) * 512],
                            start=(k == 0), stop=(k == KT - 1),
                        )
                    nc.scalar.activation(
                        hT_sb[:, mo, n * 512:(n + 1) * 512], pf1[:, :],
                        AF.Gelu_apprx_tanh,
                        bias=bfc1_pp[:, mo:mo + 1], scale=1.0,
                    )
            # fc2 in groups of 2 m-tiles (4 psum banks) so 2 banks remain
            # for output transposes interleaved right behind each group
            for ms in ([0, 1], [2, 3], [4, 5], [6, 7], [8]):
                pms = {}
                for m in ms:
                    for n in range(2):
                        pms[(m, n)] = ps_f2.tile(
                            [128, 512], F32, tag="f2", bufs=4,
                            name=f"f2_{m}_{n}"
                        )
                w = 128 * len(ms)
                for k in range(MH):
                    w2_rd = p5w.tile([128, 384], BF16, tag="w2rd", bufs=8,
                                     name="w2_rd")
                    nc.sync.dma_start(
                        w2_rd[:, 0:w],
                        w2_dr[k * 128:(k + 1) * 128,
                              ms[0] * 128:ms[0] * 128 + w],
                    )
                    for n in range(2):
                        for i, m in enumerate(ms):
                            nc.tensor.matmul(
                                pms[(m, n)][:, :],
                                w2_rd[:, i * 128:(i + 1) * 128],
                                hT_sb[:, k, n * 512:(n + 1) * 512],
                                start=(k == 0), stop=(k == MH - 1),
                                skip_group_check=True,
                            )
                for m in ms:
                    for n in range(2):
                        nsl = slice(n * 512, (n + 1) * 512)
                        t2 = p5w.tile([128, 512], F32, tag="tsb", bufs=3,
                                      name="t2")
                        nc.scalar.activation(
                            t2[:, :], pms[(m, n)][:, :], AF.Identity,
                            bias=bfc2_pp[:, m:m + 1], scale=1.0,
                        )
                        nc.vector.scalar_tensor_tensor(
                            o_full[:, m, nsl], t2[:, :],
                            ada_pp[:, 5, m:m + 1], xT[:, m, nsl],
                            ALU.mult, ALU.add,
                        )
                    for tt in range(NT // 128):
                        pt = ps_f2.tile([128, 128], F32, tag="tro", bufs=2,
                                        name="pt6")
                        nc.tensor.transpose(
                            pt[:, :], o_full[:, m, tt * 128:(tt + 1) * 128],
                            ident[:, :],
                        )
                        ot = ph.tile([128, 128], F32, tag="ot", bufs=4,
                                     name="ot")
                        if tt % 2 == 0:
                            nc.vector.tensor_copy(ot[:, :], pt[:, :])
                        else:
                            nc.scalar.copy(ot[:, :], pt[:, :])
                        nc.sync.dma_start(
                            out_dram[tt * 128:(tt + 1) * 128,
                                     m * 128:(m + 1) * 128],
                            ot[:, :],
                        )
        es_mod2.close()
        es_o.close()


_LOCK = threading.Lock()
_PROG = None


def _get_program():
    global _PROG
    with _LOCK:
        if _PROG is None:
            _PROG = _build_program()
    return _PROG


def _make_in_maps(inputs):
    arrs = {k: np.ascontiguousarray(np.asarray(v, dtype=np.float32))
            for k, v in inputs.items()}
    in_maps = []
    for c in range(NCORES):
        m = {k: v for k, v in arrs.items() if k not in ("x", "t_emb")}
        m["x"] = np.ascontiguousarray(arrs["x"][c])
        m["t_emb"] = np.ascontiguousarray(arrs["t_emb"][c])
        in_maps.append(m)
    return in_maps


def kernel(**inputs):
    nc = _get_program()
    res = run_bass_kernel_spmd(nc, _make_in_maps(inputs), core_ids=list(range(NCORES)))
    return np.stack([r["out"] for r in res.results], axis=0)


def kernel_traced(inputs, **kw):
    """test-harness helper: returns full BassKernelResults with trace."""
    nc = _get_program()
    return run_bass_kernel_spmd(
        nc, _make_in_maps(inputs), core_ids=list(range(NCORES)), trace=True, **kw
    )



# revision 24
# speedup vs baseline: 1.3221x; 1.3221x over previous
"""DiT block kernel for Trainium2 (Bass/Tile), 8-core data parallel.

Shapes (hardcoded from the problem spec):
  x: (8, 1024, 1152), t_emb: (8, 1152)
  w_qkv (1152, 3456), w_proj (1152, 1152), w_fc1 (1152, 4608),
  w_fc2 (4608, 1152), w_ada (1152, 6912) + biases.

Strategy: batch-parallel across 8 cores (one batch element each, no
collectives). Activations live transposed [D on partitions, tokens free].
The large matmuls (qkv, attention AV, proj, fc1, fc2) run in fp8e4 with
DoubleRow perf mode (two 128-row k-tiles contracted per instruction);
scale factors for fp8 range are folded into the existing activation
bias/scale stages so no extra elementwise work is added.  LayerNorm
statistics reduce over the partition axis via ones-vector f32r matmuls;
softmax runs transposed (keys on partitions) with denominators collected
through a ones-column appended to V and a fused divide.  q/k are produced
per-head directly (M=72 matmuls cost the same per column as M=128), so
attention needs no partition-crossing gather DMAs.  Weights stream
through big staged f32 DMA loads (few, large transfers) and are
converted on-chip; ada (error-sensitive) stays f32r.
"""

import threading
from contextlib import ExitStack

import numpy as np

import concourse.bass as bass
import concourse.mybir as mybir
import concourse.tile as tile
from concourse import bacc
from concourse.bass_utils import run_bass_kernel_spmd
from concourse.masks import make_identity

F32 = mybir.dt.float32
F32R = mybir.dt.float32r
BF16 = mybir.dt.bfloat16
FP8 = mybir.dt.float8e4
AF = mybir.ActivationFunctionType
ALU = mybir.AluOpType
DR = mybir.MatmulPerfMode.DoubleRow

NCORES = 8
D = 1152
NT = 1024
KT = D // 128       # 9
KTP = KT + 1        # padded to even for DoubleRow pairs
H = 16
HD = 72
HID = 4 * D
MH = HID // 128     # 36
EPS = 1e-6
ISC = 1.0 / float(np.sqrt(HD))

# fp8 scale factors
WS = 64.0           # weights
AS = 8.0            # modulated activations (mod1/mod2)
QS = 2.0            # q/k
PS = 4.0            # attention output
ES = ISC / (QS * QS)  # exp() input scale applied to the scores psum

# v output column slices aligned to head boundaries
V_SLICES = [(0, 432, 0, 6), (432, 864, 6, 12), (864, 1152, 12, 16)]


def _r(ap):
    return ap.bitcast(F32R)


def _build_program():
    nc = bacc.Bacc(
        "TRN2", target_bir_lowering=False, debug=False, enable_asserts=False
    )
    ins = {}
    ins["x"] = nc.dram_tensor("x", [NT, D], F32, kind="ExternalInput").ap()
    ins["t_emb"] = nc.dram_tensor("t_emb", [D], F32, kind="ExternalInput").ap()
    for name, shape in [
        ("w_qkv", [D, 3 * D]), ("b_qkv", [3 * D]),
        ("w_proj", [D, D]), ("b_proj", [D]),
        ("w_fc1", [D, HID]), ("b_fc1", [HID]),
        ("w_fc2", [HID, D]), ("b_fc2", [D]),
        ("w_ada", [D, 6 * D]), ("b_ada", [6 * D]),
    ]:
        ins[name] = nc.dram_tensor(name, shape, F32, kind="ExternalInput").ap()
    out_dram = nc.dram_tensor("out", [NT, D], F32, kind="ExternalOutput").ap()

    with tile.TileContext(nc) as tc:
        _body(tc, ins, out_dram)
    nc.compile()
    return nc


def _ln_stats(tc, nc, src, ones_col, pst, pln, ps_st):
    """Return st[n] = [mean; rstd] rows [1, 2, 512] per 512-token half,
    reducing over the partition (D) axis of src [128, KT, NT] f32."""
    ps_x, ps_q, st = {}, {}, {}
    for n in range(2):
        nsl = slice(n * 512, (n + 1) * 512)
        ps_x[n] = ps_st.tile([1, 512], F32, tag="st", name=f"psx{n}")
        ps_q[n] = ps_st.tile([1, 512], F32, tag="st", name=f"psq{n}")
        for k in range(KT):
            xb = pln.tile([128, 512], BF16, tag="xb", bufs=3, name="xb")
            nc.gpsimd.tensor_copy(xb[:, :], src[:, k, nsl])
            sq = pln.tile([128, 512], BF16, tag="sq", bufs=3, name="sq")
            nc.vector.tensor_mul(sq[:, :], src[:, k, nsl], src[:, k, nsl])
            nc.tensor.matmul(
                ps_x[n][:, :], ones_col[:, :], xb[:, :],
                start=(k == 0), stop=(k == KT - 1), skip_group_check=True,
            )
            nc.tensor.matmul(
                ps_q[n][:, :], ones_col[:, :], sq[:, :],
                start=(k == 0), stop=(k == KT - 1), skip_group_check=True,
            )
    eps_sb = pst.tile([1, 1], F32, tag="eps", bufs=1, name="eps_sb")
    nc.vector.memset(eps_sb[:, :], EPS)
    for n in range(2):
        st[n] = pst.tile([1, 2, 512], F32, tag="lnst", bufs=2, name=f"st{n}")
        nc.vector.tensor_scalar_mul(st[n][:, 0, :], ps_x[n][:, :], 1.0 / D)
        work = pst.tile([1, 512], F32, tag="lnwork", bufs=2, name="work")
        nc.vector.tensor_mul(work[:, :], st[n][:, 0, :], st[n][:, 0, :])
        nc.vector.scalar_tensor_tensor(
            st[n][:, 1, :], ps_q[n][:, :], 1.0 / D, work[:, :],
            ALU.mult, ALU.subtract,
        )
        nc.scalar.activation(st[n][:, 1, :], st[n][:, 1, :], AF.Sqrt,
                             bias=eps_sb[:, :], scale=1.0)
        nc.vector.reciprocal(st[n][:, 1, :], st[n][:, 1, :])
    return st


def _ln_apply(tc, nc, src, dst, st, ada_pp, sh_c, sc_c, pln):
    """dst[:,k,nsl] (fp8) = ((src-mean)*rstd) * ada[sc_c] + ada[sh_c]
    (ada params pre-scaled by AS)."""
    for n in range(2):
        nsl = slice(n * 512, (n + 1) * 512)
        meanB = pln.tile([128, 512], F32, tag="meanB", bufs=2, name="meanB")
        rstdB = pln.tile([128, 512], F32, tag="rstdB", bufs=2, name="rstdB")
        nc.gpsimd.partition_broadcast(meanB[:, :], st[n][:, 0, :])
        nc.gpsimd.partition_broadcast(rstdB[:, :], st[n][:, 1, :])
        for k in range(KT):
            t1 = pln.tile([128, 512], F32, tag="lnt1", bufs=3, name="t1")
            nc.vector.tensor_sub(t1[:, :], src[:, k, nsl], meanB[:, :])
            t2 = pln.tile([128, 512], F32, tag="lnt2", bufs=3, name="t2")
            if k % 3 == 0:
                nc.gpsimd.tensor_mul(t2[:, :], t1[:, :], rstdB[:, :])
            else:
                nc.vector.tensor_mul(t2[:, :], t1[:, :], rstdB[:, :])
            nc.scalar.activation(
                dst[:, k, nsl], t2[:, :], AF.Identity,
                bias=ada_pp[:, sh_c, k:k + 1],
                scale=ada_pp[:, sc_c, k:k + 1],
            )


def _body(tc, ins, out_dram):
    nc = tc.nc
    ctx = ExitStack()
    with ctx:
        dram = ctx.enter_context(tc.tile_pool(name="dram", bufs=1, space="DRAM"))
        ada_dr = dram.tile([6 * D], F32)

        pers = ctx.enter_context(tc.tile_pool(name="pers", bufs=1))
        ident = pers.tile([128, 128], F32)
        make_identity(nc, ident[:, :])
        ones_col = pers.tile([128, 1], BF16)
        nc.vector.memset(ones_col[:, :], 1.0)
        ones_row = pers.tile([1, 128], BF16)
        nc.vector.memset(ones_row[:, :], 1.0)

        t_pp = pers.tile([128, KT], F32)
        nc.sync.dma_start(t_pp[:, :], ins["t_emb"].rearrange("(k p) -> p k", p=128))
        t_pr = pers.tile([128, KT], F32R)
        nc.scalar.activation(t_pr[:, :], t_pp[:, :], AF.Silu)

        bq_s = pers.tile([72, H], F32)
        bk_s = pers.tile([72, H], F32)
        bv_row = pers.tile([1, D], F32)
        bv_b = pers.tile([1, D], BF16)
        bproj_pp = pers.tile([128, KT], F32)
        bfc1_pp = pers.tile([128, MH], F32)
        bfc2_pp = pers.tile([128, KT], F32)
        bada_pp = pers.tile([128, 6, KT], F32)
        ada_pp = pers.tile([128, 6, KT], F32)

        def emit_bias_loads():
            nc.sync.dma_start(
                bq_s[:, :], ins["b_qkv"][0:D].rearrange("(h p) -> p h", p=72))
            nc.sync.dma_start(
                bk_s[:, :], ins["b_qkv"][D:2 * D].rearrange("(h p) -> p h", p=72))
            nc.sync.dma_start(
                bv_row[:, :],
                ins["b_qkv"][2 * D:3 * D].rearrange("(a b) -> a b", a=1))
            # bv enters the v accumulation in (AS*WS)-scaled psum units
            nc.vector.tensor_scalar_mul(bv_b[:, :], bv_row[:, :], AS * WS)
            nc.sync.dma_start(
                bproj_pp[:, :], ins["b_proj"].rearrange("(m p) -> p m", p=128))
            nc.sync.dma_start(
                bfc1_pp[:, :], ins["b_fc1"].rearrange("(m p) -> p m", p=128))
            nc.sync.dma_start(
                bfc2_pp[:, :], ins["b_fc2"].rearrange("(m p) -> p m", p=128))
            nc.sync.dma_start(
                bada_pp[:, :, :],
                ins["b_ada"].rearrange("(c k p) -> p c k", k=KT, p=128))
            # pre-scale q/k biases by QS (folded into the psum->fp8 copies)
            nc.vector.tensor_scalar_mul(bq_s[:, :], bq_s[:, :], QS)
            nc.vector.tensor_scalar_mul(bk_s[:, :], bk_s[:, :], QS)

        xT = pers.tile([128, KT, NT], F32)      # becomes x2T after residual 1
        mod12T = pers.tile([128, KTP, NT], FP8)  # mod1T, later reused as mod2T
        nc.gpsimd.memset(mod12T[:, KT, :], 0.0)  # DoubleRow pad k-tile

        # ================= phase A: x load/transpose, ada, LN1 ==============
        es_a = ExitStack()
        pst = es_a.enter_context(tc.tile_pool(name="pst", bufs=1))
        pln = es_a.enter_context(tc.tile_pool(name="pln", bufs=1))

        def emit_ada_chunk(c, p1w, ps_pro):
            """chunk c covers w_ada cols [c*384, (c+1)*384); param p=c//3."""
            wst = p1w.tile([128, KT, 384], F32R, tag="adast", bufs=2, name="wst")
            nc.sync.dma_start(
                wst[:, :, :],
                ins["w_ada"][:, c * 384:(c + 1) * 384]
                .rearrange("(k p) m -> p k m", p=128).bitcast(F32R),
            )
            pa = ps_pro.tile([1, 384], F32, tag="psada", bufs=2, name="pa")
            for k in range(KT):
                nc.tensor.matmul(
                    pa[:, :], t_pr[:, k:k + 1], wst[:, k, :],
                    start=(k == 0), stop=(k == KT - 1),
                )
            asb = p1w.tile([1, 384], F32, tag="asb", bufs=3, name="asb")
            nc.vector.tensor_copy(asb[:, :], pa[:, :])
            nc.sync.dma_start(
                ada_dr[c * 384:(c + 1) * 384].rearrange("(a b) -> a b", a=1),
                asb[0:1, :],
            )

        def emit_ada_pp_load(cs):
            """Load+finalize ada params cs (list) into ada_pp; params 0/1
            (shift_a/scale_a) and 3/4 are pre-scaled by AS; 1/4 get +1."""
            for c in cs:
                nc.sync.dma_start(
                    ada_pp[:, c, :],
                    ada_dr[c * D:(c + 1) * D].rearrange("(k p) -> p k", p=128),
                )
            lo, hi = min(cs), max(cs) + 1
            nc.vector.tensor_add(ada_pp[:, lo:hi, :], ada_pp[:, lo:hi, :],
                                 bada_pp[:, lo:hi, :])
            for c in cs:
                if c in (1, 4):
                    nc.vector.tensor_scalar_add(ada_pp[:, c, :],
                                                ada_pp[:, c, :], 1.0)
                if c in (0, 1, 3, 4):
                    nc.vector.tensor_scalar_mul(ada_pp[:, c, :],
                                                ada_pp[:, c, :], AS)

        with tc.tile_pool(name="p1w", bufs=1) as p1w, \
             tc.tile_pool(name="pxin", bufs=3) as pxin, \
             tc.tile_pool(name="ps_pro", bufs=2, space="PSUM") as ps_pro, \
             tc.tile_pool(name="ps_tr", bufs=2, space="PSUM") as ps_tr:

            def emit_transpose_block(tt):
                xin = pxin.tile([128, D], F32, tag="xin", name="xin")
                nc.sync.dma_start(
                    xin[:, :], ins["x"][tt * 128:(tt + 1) * 128, :])
                for kd in range(KT):
                    pt = ps_tr.tile([128, 128], F32, tag="ptr", name="pt")
                    nc.tensor.transpose(
                        pt[:, :], xin[:, kd * 128:(kd + 1) * 128], ident[:, :])
                    tsl = slice(tt * 128, (tt + 1) * 128)
                    if kd % 2 == 0:
                        nc.vector.tensor_copy(xT[:, kd, tsl], pt[:, :])
                    else:
                        nc.scalar.copy(xT[:, kd, tsl], pt[:, :])

            for i in range(8):
                emit_transpose_block(i)
                if i == 0:
                    emit_bias_loads()
                if i < 6:
                    emit_ada_chunk(i, p1w, ps_pro)
            emit_ada_pp_load([0, 1])

        with tc.tile_pool(name="ps_st", bufs=4, space="PSUM") as ps_st:
            st1 = _ln_stats(tc, nc, xT, ones_col, pst, pln, ps_st)
            _ln_apply(tc, nc, xT, mod12T, st1, ada_pp, 0, 1, pln)
        es_a.close()

        # ================= phase B: qkv =====================================
        es_qk = ExitStack()
        pqk8 = es_qk.enter_context(tc.tile_pool(name="pqk8", bufs=1))
        wq8 = pqk8.tile([128, KTP, D], FP8, name="wq8")
        wk8 = pqk8.tile([128, KTP, D], FP8, name="wk8")
        nc.gpsimd.memset(wq8[:, KT, :], 0.0)
        nc.gpsimd.memset(wk8[:, KT, :], 0.0)

        es_att = ExitStack()
        patt = es_att.enter_context(tc.tile_pool(name="patt", bufs=1, side="right"))
        q_all = patt.tile([72, H, NT], FP8, name="q_all")
        k_all = patt.tile([72, H, NT], FP8, name="k_all")
        attn_hs = patt.tile([72, H, NT], FP8, name="attn_hs")
        v_aug = patt.tile([128, NT // 128, H, 97], FP8, name="v_aug")
        nc.gpsimd.memset(v_aug[:, :, :, HD:97], 0.0)
        nc.gpsimd.memset(v_aug[:, :, :, 96:97], 1.0)

        with tc.tile_pool(name="pwst", bufs=1) as pwst, \
             tc.tile_pool(name="ps_v", bufs=3, space="PSUM") as ps_v:
            # q/k/v weight loads in 576-col chunks + fp8 convert (scale WS)
            wv8 = pwst.tile([128, KTP, D], FP8, tag="wv8", bufs=1, name="wv8")
            nc.gpsimd.memset(wv8[:, KT, :], 0.0)
            engs = ["act", "dve", "pool", "dve", "pool", "act"]
            for j, (dst8, c0) in enumerate(((wq8, 0), (wk8, D), (wv8, 2 * D))):
                for half in range(2):
                    msl = slice(half * 576, (half + 1) * 576)
                    wst = pwst.tile([128, KT, 576], F32, tag="wst", bufs=2,
                                    name="wst")
                    nc.sync.dma_start(
                        wst[:, :, :],
                        ins["w_qkv"][:, c0 + half * 576:c0 + (half + 1) * 576]
                        .rearrange("(k p) m -> p k m", p=128),
                    )
                    eng = engs[j * 2 + half]
                    for kk in range(3):
                        ksl = slice(kk * 3, kk * 3 + 3)
                        if eng == "act":
                            nc.scalar.activation(
                                dst8[:, ksl, msl], wst[:, ksl, :],
                                AF.Identity, scale=WS)
                        elif eng == "dve":
                            nc.vector.tensor_scalar_mul(
                                dst8[:, ksl, msl], wst[:, ksl, :], WS)
                        else:
                            nc.gpsimd.tensor_scalar_mul(
                                dst8[:, ksl, msl], wst[:, ksl, :], WS)

            for tt in range(NT // 128):
                tsl = slice(tt * 128, (tt + 1) * 128)
                for si, (c0, c1, h0, h1) in enumerate(V_SLICES):
                    pmv = ps_v.tile([128, 512], F32, tag="mv", name="pmv")
                    for i in range(KTP // 2):
                        nc.tensor.matmul(
                            pmv[:, 0:c1 - c0],
                            mod12T[:, 2 * i:2 * i + 2, tsl],
                            wv8[:, 2 * i:2 * i + 2, c0:c1],
                            start=(i == 0), stop=False, perf_mode=DR,
                            skip_group_check=True,
                        )
                    nc.tensor.matmul(
                        pmv[:, 0:c1 - c0], ones_row[:, :],
                        bv_b[:, c0:c1],
                        start=False, stop=True, skip_group_check=True,
                    )
                    # scatter into v_aug head slots (strided copy)
                    src = pmv[:, 0:c1 - c0].rearrange(
                        "p (h d) -> p h d", d=HD)
                    nc.vector.tensor_scalar_mul(
                        v_aug[:, tt, h0:h1, 0:HD], src, 1.0 / (AS * WS))

        # ================= phase C: attention ===============================
        es_wp = ExitStack()
        pwp8 = es_wp.enter_context(
            tc.tile_pool(name="pwp8", bufs=1, side="right"))
        wp8 = pwp8.tile([72, H, D], FP8, name="wp8")

        with tc.tile_pool(name="p3w", bufs=1) as p3w, \
             tc.tile_pool(name="pexp", bufs=1) as pexp, \
             tc.tile_pool(name="pat3", bufs=1) as pat3, \
             tc.tile_pool(name="ps_qk", bufs=2, space="PSUM") as ps_qk, \
             tc.tile_pool(name="ps_s", bufs=2, space="PSUM") as ps_s, \
             tc.tile_pool(name="ps_av", bufs=2, space="PSUM") as ps_av, \
             tc.tile_pool(name="ps_pa", bufs=1, space="PSUM") as ps_pa:

            def emit_filler(h):
                # late ada chunks (6..17), then w_proj load+convert
                if h < 12:
                    emit_ada_chunk(6 + h, p3w, ps_pa)
                if h == 11:
                    emit_ada_pp_load([2, 3])
                    emit_ada_pp_load([4, 5])

            for h in range(H):
                emit_filler(h)
                pq, pk = {}, {}
                for n in range(2):
                    nsl = slice(n * 512, (n + 1) * 512)
                    pq[n] = ps_qk.tile([72, 512], F32, tag="qk", name="pq")
                    for i in range(KTP // 2):
                        nc.tensor.matmul(
                            pq[n][:, :],
                            wq8[:, 2 * i:2 * i + 2, h * HD:(h + 1) * HD],
                            mod12T[:, 2 * i:2 * i + 2, nsl],
                            start=(i == 0), stop=(i == KTP // 2 - 1),
                            perf_mode=DR,
                        )
                    nc.scalar.activation(
                        q_all[:, h, nsl], pq[n][:, :], AF.Identity,
                        bias=bq_s[:, h:h + 1], scale=QS / (AS * WS),
                    )
                for n in range(2):
                    nsl = slice(n * 512, (n + 1) * 512)
                    pk[n] = ps_qk.tile([72, 512], F32, tag="qk", name="pk")
                    for i in range(KTP // 2):
                        nc.tensor.matmul(
                            pk[n][:, :],
                            wk8[:, 2 * i:2 * i + 2, h * HD:(h + 1) * HD],
                            mod12T[:, 2 * i:2 * i + 2, nsl],
                            start=(i == 0), stop=(i == KTP // 2 - 1),
                            perf_mode=DR,
                        )
                    nc.vector.tensor_scalar(
                        k_all[:, h, nsl], pk[n][:, :], QS / (AS * WS),
                        bk_s[:, h:h + 1], ALU.mult, ALU.add,
                    )
                for n in range(2):
                    nsl = slice(n * 512, (n + 1) * 512)
                    exp_hn = pexp.tile([128, NT // 128, 512], FP8, tag="exp",
                                       bufs=3, name="exp_hn")
                    for kt_i in range(NT // 128):
                        pss = ps_s.tile([128, 512], F32, tag="s", name="pss")
                        nc.tensor.matmul(
                            pss[:, :],
                            k_all[:, h, kt_i * 128:(kt_i + 1) * 128],
                            q_all[:, h, nsl], start=True, stop=True,
                        )
                        nc.scalar.activation(
                            exp_hn[:, kt_i, :], pss[:, :], AF.Exp, scale=ES)
                    pav = ps_av.tile([97, 512], F32, tag="av", name="pav")
                    for i in range(NT // 256):
                        nc.tensor.matmul(
                            pav[:, :],
                            v_aug[:, 2 * i:2 * i + 2, h, :],
                            exp_hn[:, 2 * i:2 * i + 2, :],
                            start=(i == 0), stop=(i == NT // 256 - 1),
                            perf_mode=DR,
                        )
                    den = pat3.tile([1, 512], F32, tag="den", bufs=2,
                                    name="den")
                    nc.vector.tensor_scalar_mul(den[:, :], pav[96:97, :],
                                                1.0 / PS)
                    nc.vector.reciprocal(den[:, :], den[:, :])
                    denB = pat3.tile([72, 512], F32, tag="denB", bufs=2,
                                     name="denB")
                    nc.gpsimd.partition_broadcast(denB[:, :], den[:, :])
                    nc.vector.tensor_mul(
                        attn_hs[:, h, nsl], pav[0:HD, :], denB[:, :])
        es_qk.close()  # wq8/wk8 no longer needed

        # ================= phase D: proj + residual + LN2 ===================
        with tc.tile_pool(name="p4", bufs=1) as p4:
            for i in range(3):
                msl = slice(i * 384, (i + 1) * 384)
                wpst = p4.tile([72, H, 384], F32, tag="wpst", bufs=2,
                               name="wpst")
                nc.sync.dma_start(
                    wpst[:, :, :],
                    ins["w_proj"][:, msl].rearrange("(h p) m -> p h m", p=72),
                )
                for kk in range(2):
                    hsl = slice(kk * 8, kk * 8 + 8)
                    nc.vector.tensor_scalar_mul(
                        wp8[:, hsl, msl], wpst[:, hsl, :], WS)
            with tc.tile_pool(name="ps_mm2", bufs=4, space="PSUM") as ps_mm2:
                for mo in range(KT):
                    for n in range(2):
                        nsl = slice(n * 512, (n + 1) * 512)
                        pm2 = ps_mm2.tile([128, 512], F32, tag="mm2",
                                          name="pm2")
                        for i in range(H // 2):
                            nc.tensor.matmul(
                                pm2[:, :],
                                wp8[:, 2 * i:2 * i + 2,
                                    mo * 128:(mo + 1) * 128],
                                attn_hs[:, 2 * i:2 * i + 2, nsl],
                                start=(i == 0), stop=(i == H // 2 - 1),
                                perf_mode=DR,
                            )
                        t_sb = p4.tile([128, 512], F32, tag="tsb", bufs=3,
                                       name="t_sb")
                        nc.scalar.activation(
                            t_sb[:, :], pm2[:, :], AF.Identity,
                            bias=bproj_pp[:, mo:mo + 1], scale=1.0 / (PS * WS),
                        )
                        nc.vector.scalar_tensor_tensor(
                            xT[:, mo, nsl], t_sb[:, :],
                            ada_pp[:, 2, mo:mo + 1], xT[:, mo, nsl],
                            ALU.mult, ALU.add,
                        )
        es_wp.close()
        es_att.close()
        with tc.tile_pool(name="pst4", bufs=1) as pst4, \
             tc.tile_pool(name="pln4", bufs=1) as pln4, \
             tc.tile_pool(name="ps_st2", bufs=4, space="PSUM") as ps_st2:
            st2 = _ln_stats(tc, nc, xT, ones_col, pst4, pln4, ps_st2)
            _ln_apply(tc, nc, xT, mod12T, st2, ada_pp, 3, 4, pln4)

        # ================= phase E: FFN =====================================
        es_e = ExitStack()
        ph = es_e.enter_context(tc.tile_pool(name="ph", bufs=1))
        hT = ph.tile([128, MH, NT], FP8, name="hT")
        po = es_e.enter_context(tc.tile_pool(name="po", bufs=1))

        with tc.tile_pool(name="ps_f1", bufs=3, space="PSUM") as ps_f1, \
             tc.tile_pool(name="ps_f2", bufs=2, space="PSUM") as ps_f2, \
             tc.tile_pool(name="ps_tro", bufs=2, space="PSUM") as ps_tro:
            # fc1 in 9 chunks of 512 columns (4 m-tiles each)
            with tc.tile_pool(name="p5a", bufs=1) as p5a:
                for ch in range(9):
                    msl = slice(ch * 512, (ch + 1) * 512)
                    f1s = p5a.tile([128, KT, 512], F32, tag="f1s", bufs=2,
                                   name="f1s")
                    nc.sync.dma_start(
                        f1s[:, :, :],
                        ins["w_fc1"][:, msl]
                        .rearrange("(k p) m -> p k m", p=128),
                    )
                    f18 = p5a.tile([128, KTP, 512], FP8, tag="f18", bufs=2,
                                   name="f18")
                    nc.gpsimd.memset(f18[:, KT, :], 0.0)
                    for kk in range(3):
                        ksl = slice(kk * 3, kk * 3 + 3)
                        if ch % 2 == 0:
                            nc.vector.tensor_scalar_mul(f18[:, ksl, :],
                                                        f1s[:, ksl, :], WS)
                        else:
                            nc.gpsimd.tensor_scalar_mul(f18[:, ksl, :],
                                                        f1s[:, ksl, :], WS)
                    for m in range(4):
                        mo = ch * 4 + m
                        for n in range(2):
                            nsl = slice(n * 512, (n + 1) * 512)
                            pf1 = ps_f1.tile([128, 512], F32, tag="f1",
                                             name="pf1")
                            for i in range(KTP // 2):
                                nc.tensor.matmul(
                                    pf1[:, :],
                                    f18[:, 2 * i:2 * i + 2,
                                        m * 128:(m + 1) * 128],
                                    mod12T[:, 2 * i:2 * i + 2, nsl],
                                    start=(i == 0), stop=(i == KTP // 2 - 1),
                                    perf_mode=DR,
                                )
                            nc.scalar.activation(
                                hT[:, mo, nsl], pf1[:, :], AF.Gelu_apprx_tanh,
                                bias=bfc1_pp[:, mo:mo + 1],
                                scale=1.0 / (AS * WS),
                            )
            # fc2 in 9 chunks of 128 columns (1 m-tile each)
            with tc.tile_pool(name="p5b", bufs=1) as p5b:
                for mo in range(KT):
                    f2s = p5b.tile([128, MH, 128], F32, tag="f2s", bufs=2,
                                   name="f2s")
                    nc.sync.dma_start(
                        f2s[:, :, :],
                        ins["w_fc2"][:, mo * 128:(mo + 1) * 128]
                        .rearrange("(k p) m -> p k m", p=128),
                    )
                    f28 = p5b.tile([128, MH, 128], FP8, tag="f28", bufs=2,
                                   name="f28")
                    for kk in range(2):
                        ksl = slice(kk * 18, kk * 18 + 18)
                        if mo % 2 == 0:
                            nc.vector.tensor_scalar_mul(f28[:, ksl, :],
                                                        f2s[:, ksl, :], WS)
                        else:
                            nc.gpsimd.tensor_scalar_mul(f28[:, ksl, :],
                                                        f2s[:, ksl, :], WS)
                    for n in range(2):
                        nsl = slice(n * 512, (n + 1) * 512)
                        pf2 = ps_f2.tile([128, 512], F32, tag="f2", name="pf2")
                        for i in range(MH // 2):
                            nc.tensor.matmul(
                                pf2[:, :], f28[:, 2 * i:2 * i + 2, :],
                                hT[:, 2 * i:2 * i + 2, nsl],
                                start=(i == 0), stop=(i == MH // 2 - 1),
                                perf_mode=DR,
                            )
                        t2 = p5b.tile([128, 512], F32, tag="t2", bufs=3,
                                      name="t2")
                        nc.scalar.activation(
                            t2[:, :], pf2[:, :], AF.Identity,
                            bias=bfc2_pp[:, mo:mo + 1], scale=1.0 / WS,
                        )
                        nc.vector.scalar_tensor_tensor(
                            xT[:, mo, nsl], t2[:, :], ada_pp[:, 5, mo:mo + 1],
                            xT[:, mo, nsl], ALU.mult, ALU.add,
                        )
                    o_slab = po.tile([128, NT // 128, 128], F32, tag="osl",
                                     bufs=2, name="o_slab")
                    for tt in range(NT // 128):
                        pt = ps_tro.tile([128, 128], F32, tag="tro",
                                         name="pt6")
                        nc.tensor.transpose(
                            pt[:, :], xT[:, mo, tt * 128:(tt + 1) * 128],
                            ident[:, :],
                        )
                        dst = o_slab[:, tt, :]
                        if tt % 2 == 0:
                            nc.vector.tensor_copy(dst, pt[:, :])
                        else:
                            nc.scalar.copy(dst, pt[:, :])
                    nc.sync.dma_start(
                        out_dram[:, mo * 128:(mo + 1) * 128]
                        .rearrange("(t p) m -> p t m", p=128),
                        o_slab[:, :, :])
        es_e.close()


_LOCK = threading.Lock()
_PROG = None


def _get_program():
    global _PROG
    with _LOCK:
        if _PROG is None:
            _PROG = _build_program()
    return _PROG


def _make_in_maps(inputs):
    arrs = {k: np.ascontiguousarray(np.asarray(v, dtype=np.float32))
            for k, v in inputs.items()}
    in_maps = []
    for c in range(NCORES):
        m = {k: v for k, v in arrs.items() if k not in ("x", "t_emb")}
        m["x"] = np.ascontiguousarray(arrs["x"][c])
        m["t_emb"] = np.ascontiguousarray(arrs["t_emb"][c])
        in_maps.append(m)
    return in_maps


def kernel(**inputs):
    nc = _get_program()
    res = run_bass_kernel_spmd(nc, _make_in_maps(inputs),
                               core_ids=list(range(NCORES)))
    return np.stack([r["out"] for r in res.results], axis=0)


def kernel_traced(inputs, **kw):
    """test-harness helper: returns full BassKernelResults with trace."""
    nc = _get_program()
    return run_bass_kernel_spmd(
        nc, _make_in_maps(inputs), core_ids=list(range(NCORES)), trace=True,
        **kw
    )


# revision 38
# speedup vs baseline: 1.3772x; 1.0417x over previous
"""DiT block kernel for Trainium2 (Bass/Tile), 8-core data parallel.

Shapes (hardcoded from the problem spec):
  x: (8, 1024, 1152), t_emb: (8, 1152)
  w_qkv (1152, 3456), w_proj (1152, 1152), w_fc1 (1152, 4608),
  w_fc2 (4608, 1152), w_ada (1152, 6912) + biases.

Strategy: batch-parallel across 8 cores (one batch element each, no
collectives). Activations live transposed [D on partitions, tokens free].
The large matmuls (qkv, attention AV, proj, fc1, fc2) run in fp8e4 with
DoubleRow perf mode (two 128-row k-tiles contracted per instruction);
scale factors for fp8 range are folded into the existing activation
bias/scale stages so no extra elementwise work is added.  LayerNorm
statistics reduce over the partition axis via ones-vector f32r matmuls;
softmax runs transposed (keys on partitions) with denominators collected
through a ones-column appended to V and a fused divide.  q/k are produced
per-head directly (M=72 matmuls cost the same per column as M=128), so
attention needs no partition-crossing gather DMAs.  Weights stream
through big staged f32 DMA loads (few, large transfers) and are
converted on-chip; ada (error-sensitive) stays f32r.
"""

import threading
from contextlib import ExitStack

import numpy as np

import concourse.bass as bass
import concourse.mybir as mybir
import concourse.tile as tile
from concourse import bacc
from concourse.bass_utils import run_bass_kernel_spmd
from concourse.masks import make_identity

F32 = mybir.dt.float32
F32R = mybir.dt.float32r
BF16 = mybir.dt.bfloat16
FP8 = mybir.dt.float8e4
AF = mybir.ActivationFunctionType
ALU = mybir.AluOpType
DR = mybir.MatmulPerfMode.DoubleRow

NCORES = 8
D = 1152
NT = 1024
KT = D // 128       # 9
KTP = KT + 1        # padded to even for DoubleRow pairs
H = 16
HD = 72
HID = 4 * D
MH = HID // 128     # 36
EPS = 1e-6
ISC = 1.0 / float(np.sqrt(HD))

# fp8 scale factors
WS = 64.0           # weights
AS = 8.0            # modulated activations (mod1/mod2)
QS = 2.0            # q/k
PS = 4.0            # attention output
ES = ISC / (QS * QS)  # exp() input scale applied to the scores psum

# v output column slices aligned to head boundaries
V_SLICES = [(0, 432, 0, 6), (432, 864, 6, 12), (864, 1152, 12, 16)]


def _r(ap):
    return ap.bitcast(F32R)


def _build_program():
    nc = bacc.Bacc(
        "TRN2", target_bir_lowering=False, debug=False, enable_asserts=False
    )
    ins = {}
    ins["x"] = nc.dram_tensor("x", [NT, D], F32, kind="ExternalInput").ap()
    ins["t_emb"] = nc.dram_tensor("t_emb", [D], F32, kind="ExternalInput").ap()
    for name, shape in [
        ("w_qkv", [D, 3 * D]), ("b_qkv", [3 * D]),
        ("w_proj", [D, D]), ("b_proj", [D]),
        ("w_fc1", [D, HID]), ("b_fc1", [HID]),
        ("w_fc2", [HID, D]), ("b_fc2", [D]),
        ("w_ada", [D, 6 * D]), ("b_ada", [6 * D]),
    ]:
        ins[name] = nc.dram_tensor(name, shape, F32, kind="ExternalInput").ap()
    out_dram = nc.dram_tensor("out", [NT, D], F32, kind="ExternalOutput").ap()

    with tile.TileContext(nc) as tc:
        _body(tc, ins, out_dram)
    nc.compile()
    return nc


def _ln_stats(tc, nc, src, ones_col, pst, pln, ps_st, halves=(0, 1),
              st=None):
    """Return st[n] = [mean; rstd] rows [1, 2, 512] per 512-token half,
    reducing over the partition (D) axis of src [128, KT, NT] f32."""
    ps_x, ps_q = {}, {}
    if st is None:
        st = {}
    for n in halves:
        nsl = slice(n * 512, (n + 1) * 512)
        ps_x[n] = ps_st.tile([1, 512], F32, tag="st", name=f"psx{n}")
        ps_q[n] = ps_st.tile([1, 512], F32, tag="st", name=f"psq{n}")
        for k in range(KT):
            xb = pln.tile([128, 512], BF16, tag="xb", bufs=2, name="xb")
            nc.scalar.copy(xb[:, :], src[:, k, nsl])
            sq = pln.tile([128, 512], BF16, tag="sq", bufs=2, name="sq")
            if n == 0:
                nc.vector.tensor_mul(sq[:, :], src[:, k, nsl], src[:, k, nsl])
            else:
                nc.gpsimd.tensor_mul(sq[:, :], src[:, k, nsl], src[:, k, nsl])
            nc.tensor.matmul(
                ps_x[n][:, :], ones_col[:, :], xb[:, :],
                start=(k == 0), stop=(k == KT - 1), skip_group_check=True,
            )
            nc.tensor.matmul(
                ps_q[n][:, :], ones_col[:, :], sq[:, :],
                start=(k == 0), stop=(k == KT - 1), skip_group_check=True,
            )
    eps_sb = pst.tile([1, 1], F32, tag="eps", bufs=1, name="eps_sb")
    nc.vector.memset(eps_sb[:, :], EPS)
    for n in halves:
        st[n] = pst.tile([1, 2, 512], F32, tag="lnst", bufs=2, name=f"st{n}")
        nc.vector.tensor_scalar_mul(st[n][:, 0, :], ps_x[n][:, :], 1.0 / D)
        work = pst.tile([1, 512], F32, tag="lnwork", bufs=2, name="work")
        nc.vector.tensor_mul(work[:, :], st[n][:, 0, :], st[n][:, 0, :])
        nc.vector.scalar_tensor_tensor(
            st[n][:, 1, :], ps_q[n][:, :], 1.0 / D, work[:, :],
            ALU.mult, ALU.subtract,
        )
        nc.scalar.activation(st[n][:, 1, :], st[n][:, 1, :], AF.Sqrt,
                             bias=eps_sb[:, :], scale=1.0)
        nc.vector.reciprocal(st[n][:, 1, :], st[n][:, 1, :])
    return st


def _ln_apply(tc, nc, src, dst, st, ada_pp, sh_c, sc_c, pln,
              halves=(0, 1)):
    """dst[:,k,nsl] (fp8) = ((src-mean)*rstd) * ada[sc_c] + ada[sh_c]
    (ada params pre-scaled by AS)."""
    for n in halves:
        nsl = slice(n * 512, (n + 1) * 512)
        meanB = pln.tile([128, 512], F32, tag="meanB", bufs=2, name="meanB")
        rstdB = pln.tile([128, 512], F32, tag="rstdB", bufs=2, name="rstdB")
        nc.gpsimd.partition_broadcast(meanB[:, :], st[n][:, 0, :])
        nc.gpsimd.partition_broadcast(rstdB[:, :], st[n][:, 1, :])
        eng0 = nc.vector if n == 0 else nc.gpsimd
        eng1 = nc.gpsimd if n == 0 else nc.vector
        for k in range(KT):
            t1 = pln.tile([128, 512], F32, tag="lnt1", bufs=2, name="t1")
            eng0.tensor_sub(t1[:, :], src[:, k, nsl], meanB[:, :])
            t2 = pln.tile([128, 512], F32, tag="lnt2", bufs=2, name="t2")
            eng0.tensor_mul(t2[:, :], t1[:, :], rstdB[:, :])
            eng1.tensor_scalar(
                dst[:, k, nsl], t2[:, :],
                ada_pp[:, sc_c, k:k + 1], ada_pp[:, sh_c, k:k + 1],
                ALU.mult, ALU.add,
            )


def _body(tc, ins, out_dram):
    nc = tc.nc
    ctx = ExitStack()
    with ctx:
        dram = ctx.enter_context(tc.tile_pool(name="dram", bufs=1, space="DRAM"))
        ada_dr = dram.tile([6 * D], F32)
        w1f8_dr = dram.tile([18, 128, KT, 256], FP8)

        pers = ctx.enter_context(tc.tile_pool(name="pers", bufs=1))
        ident = pers.tile([128, 128], F32)
        make_identity(nc, ident[:, :])
        ones_col = pers.tile([128, 1], BF16)
        nc.vector.memset(ones_col[:, :], 1.0)
        ones_row = pers.tile([1, 128], BF16)
        nc.vector.memset(ones_row[:, :], 1.0)

        t_pp = pers.tile([128, KT], F32)
        nc.sync.dma_start(t_pp[:, :], ins["t_emb"].rearrange("(k p) -> p k", p=128))
        t_pr = pers.tile([128, KT], F32R)
        nc.scalar.activation(t_pr[:, :], t_pp[:, :], AF.Silu)

        bq_s = pers.tile([72, H], F32)
        bk_s = pers.tile([72, H], F32)
        bv_row = pers.tile([1, D], F32)
        bv_b = pers.tile([1, D], BF16)
        bproj_pp = pers.tile([128, KT], F32)
        bfc1_pp = pers.tile([128, MH], F32)
        bfc2_pp = pers.tile([128, KT], F32)
        bada_pp = pers.tile([128, 6, KT], F32)
        ada_pp = pers.tile([128, 6, KT], F32)

        def emit_bias_loads():
            nc.sync.dma_start(
                bq_s[:, :], ins["b_qkv"][0:D].rearrange("(h p) -> p h", p=72))
            nc.sync.dma_start(
                bk_s[:, :], ins["b_qkv"][D:2 * D].rearrange("(h p) -> p h", p=72))
            nc.sync.dma_start(
                bv_row[:, :],
                ins["b_qkv"][2 * D:3 * D].rearrange("(a b) -> a b", a=1))
            # bv enters the v accumulation in (AS*WS)-scaled psum units
            nc.vector.tensor_scalar_mul(bv_b[:, :], bv_row[:, :], AS * WS)
            nc.sync.dma_start(
                bproj_pp[:, :], ins["b_proj"].rearrange("(m p) -> p m", p=128))
            nc.sync.dma_start(
                bfc1_pp[:, :], ins["b_fc1"].rearrange("(m p) -> p m", p=128))
            nc.sync.dma_start(
                bfc2_pp[:, :], ins["b_fc2"].rearrange("(m p) -> p m", p=128))
            nc.sync.dma_start(
                bada_pp[:, :, :],
                ins["b_ada"].rearrange("(c k p) -> p c k", k=KT, p=128))
            # pre-scale q/k biases by QS (folded into the psum->fp8 copies)
            nc.vector.tensor_scalar_mul(bq_s[:, :], bq_s[:, :], QS)
            nc.vector.tensor_scalar_mul(bk_s[:, :], bk_s[:, :], QS)

        xT = pers.tile([128, KT, NT], F32)      # becomes x2T after residual 1
        mod12T = pers.tile([128, KTP, NT], FP8)  # mod1T, later reused as mod2T
        nc.gpsimd.memset(mod12T[:, KT, :], 0.0)  # DoubleRow pad k-tile

        # ================= phase A: x load/transpose, ada, LN1 ==============

        def emit_ada_chunk(c, p1w, ps_pro, ps_bufs=2):
            """chunk c covers w_ada cols [c*384, (c+1)*384); param p=c//3."""
            wst = p1w.tile([128, KT, 384], F32R, tag="adast", bufs=2, name="wst")
            nc.sync.dma_start(
                wst[:, :, :],
                ins["w_ada"][:, c * 384:(c + 1) * 384]
                .rearrange("(k p) m -> p k m", p=128).bitcast(F32R),
            )
            pa = ps_pro.tile([1, 384], F32, tag="psada", bufs=ps_bufs,
                             name="pa")
            for k in range(KT):
                nc.tensor.matmul(
                    pa[:, :], t_pr[:, k:k + 1], wst[:, k, :],
                    start=(k == 0), stop=(k == KT - 1),
                )
            asb = p1w.tile([1, 384], F32, tag="asb", bufs=3, name="asb")
            nc.vector.tensor_copy(asb[:, :], pa[:, :])
            nc.gpsimd.dma_start(
                ada_dr[c * 384:(c + 1) * 384].rearrange("(a b) -> a b", a=1),
                asb[0:1, :],
            )

        def emit_ada_pp_load(cs):
            """Load+finalize ada params cs (list) into ada_pp; params 0/1
            (shift_a/scale_a) and 3/4 are pre-scaled by AS; 1/4 get +1."""
            for c in cs:
                nc.gpsimd.dma_start(
                    ada_pp[:, c, :],
                    ada_dr[c * D:(c + 1) * D].rearrange("(k p) -> p k", p=128),
                )
            lo, hi = min(cs), max(cs) + 1
            nc.vector.tensor_add(ada_pp[:, lo:hi, :], ada_pp[:, lo:hi, :],
                                 bada_pp[:, lo:hi, :])
            for c in cs:
                if c in (1, 4):
                    nc.vector.tensor_scalar_add(ada_pp[:, c, :],
                                                ada_pp[:, c, :], 1.0)
                if c in (0, 1, 3, 4):
                    nc.vector.tensor_scalar_mul(ada_pp[:, c, :],
                                                ada_pp[:, c, :], AS)

        with tc.tile_pool(name="p1w", bufs=1) as p1w, \
             tc.tile_pool(name="pxin", bufs=3) as pxin, \
             tc.tile_pool(name="ps_pro", bufs=2, space="PSUM") as ps_pro, \
             tc.tile_pool(name="ps_tr", bufs=2, space="PSUM") as ps_tr:

            def emit_transpose_block(tt):
                xin = pxin.tile([128, D], F32, tag="xin", name="xin")
                nc.sync.dma_start(
                    xin[:, :], ins["x"][tt * 128:(tt + 1) * 128, :])
                for kd in range(KT):
                    pt = ps_tr.tile([128, 128], F32, tag="ptr", name="pt")
                    nc.tensor.transpose(
                        pt[:, :], xin[:, kd * 128:(kd + 1) * 128], ident[:, :])
                    tsl = slice(tt * 128, (tt + 1) * 128)
                    if kd % 2 == 0:
                        nc.vector.tensor_copy(xT[:, kd, tsl], pt[:, :])
                    else:
                        nc.scalar.copy(xT[:, kd, tsl], pt[:, :])

            for i in range(8):
                emit_transpose_block(i)
                if i == 0:
                    emit_bias_loads()
                if i < 6:
                    emit_ada_chunk(i, p1w, ps_pro)
            emit_ada_pp_load([0, 1])

        # ====== phase B part 1: qkv weight loads + converts (emitted before
        # LN1 so SP streams the loads while ada finishes / LN runs) =========
        es_qk = ExitStack()
        pqk8 = es_qk.enter_context(tc.tile_pool(name="pqk8", bufs=1))
        wq8 = pqk8.tile([128, KTP, D], FP8, name="wq8")
        wk8 = pqk8.tile([128, KTP, D], FP8, name="wk8")
        nc.gpsimd.memset(wq8[:, KT, :], 0.0)
        nc.gpsimd.memset(wk8[:, KT, :], 0.0)

        es_att = ExitStack()
        patt = es_att.enter_context(tc.tile_pool(name="patt", bufs=1, side="right"))
        attn_hs = patt.tile([72, H, NT], FP8, name="attn_hs")
        es_wp = ExitStack()
        pwp8 = es_wp.enter_context(
            tc.tile_pool(name="pwp8", bufs=1, side="right"))
        wp8 = pwp8.tile([72, H, D], FP8, name="wp8")
        es_va = ExitStack()
        pva = es_va.enter_context(tc.tile_pool(name="pva", bufs=1, side="right"))
        v_aug = pva.tile([128, NT // 128, H, 97], FP8, name="v_aug")
        nc.gpsimd.memset(v_aug[:, :, :, HD:97], 0.0)
        nc.gpsimd.memset(v_aug[:, :, :, 96:97], 1.0)

        es_b = ExitStack()
        pwst = es_b.enter_context(tc.tile_pool(name="pwst", bufs=1))
        wv8 = pwst.tile([128, KTP, D], FP8, tag="wv8", bufs=1, name="wv8")
        nc.gpsimd.memset(wv8[:, KT, :], 0.0)
        engs = ["act", "dve", "act", "dve", "act", "dve"]
        for j, (dst8, c0) in enumerate(((wq8, 0), (wk8, D), (wv8, 2 * D))):
            for half in range(2):
                msl = slice(half * 576, (half + 1) * 576)
                wst = pwst.tile([128, KT, 576], F32, tag="wst", bufs=2,
                                name="wst")
                nc.sync.dma_start(
                    wst[:, :, :],
                    ins["w_qkv"][:, c0 + half * 576:c0 + (half + 1) * 576]
                    .rearrange("(k p) m -> p k m", p=128),
                )
                eng = engs[j * 2 + half]
                for kk in range(3):
                    ksl = slice(kk * 3, kk * 3 + 3)
                    if eng == "act":
                        nc.scalar.activation(
                            dst8[:, ksl, msl], wst[:, ksl, :],
                            AF.Identity, scale=WS)
                    elif eng == "dve":
                        nc.vector.tensor_scalar_mul(
                            dst8[:, ksl, msl], wst[:, ksl, :], WS)
                    else:
                        nc.gpsimd.tensor_scalar_mul(
                            dst8[:, ksl, msl], wst[:, ksl, :], WS)

        # ====== LN1 (per-half, interleaved with v matmuls) ==================
        with tc.tile_pool(name="pst", bufs=1) as pst, \
             tc.tile_pool(name="pln", bufs=1) as pln, \
             tc.tile_pool(name="ps_st", bufs=4, space="PSUM") as ps_st, \
             tc.tile_pool(name="ps_v", bufs=3, space="PSUM") as ps_v:

            def v_block(tts):
                for tt in tts:
                    tsl = slice(tt * 128, (tt + 1) * 128)
                    for si, (c0, c1, h0, h1) in enumerate(V_SLICES):
                        pmv = ps_v.tile([128, 512], F32, tag="mv", name="pmv")
                        for i in range(KTP // 2):
                            nc.tensor.matmul(
                                pmv[:, 0:c1 - c0],
                                mod12T[:, 2 * i:2 * i + 2, tsl],
                                wv8[:, 2 * i:2 * i + 2, c0:c1],
                                start=(i == 0), stop=False, perf_mode=DR,
                                skip_group_check=True,
                            )
                        nc.tensor.matmul(
                            pmv[:, 0:c1 - c0], ones_row[:, :],
                            bv_b[:, c0:c1],
                            start=False, stop=True, skip_group_check=True,
                        )
                        vsrc = pmv[:, 0:c1 - c0].rearrange(
                            "p (h d) -> p h d", d=HD)
                        nc.vector.tensor_scalar_mul(
                            v_aug[:, tt, h0:h1, 0:HD], vsrc, 1.0 / (AS * WS))

            st1 = {}
            _ln_stats(tc, nc, xT, ones_col, pst, pln, ps_st, halves=(0,),
                      st=st1)
            _ln_apply(tc, nc, xT, mod12T, st1, ada_pp, 0, 1, pln, halves=(0,))
            _ln_stats(tc, nc, xT, ones_col, pst, pln, ps_st, halves=(1,),
                      st=st1)
            v_block(range(0, 4))
            _ln_apply(tc, nc, xT, mod12T, st1, ada_pp, 0, 1, pln, halves=(1,))
            v_block(range(4, 8))
        es_b.close()

        # ================= phase C: attention ===============================
        with tc.tile_pool(name="p3w", bufs=1) as p3w, \
             tc.tile_pool(name="pexp", bufs=1) as pexp, \
             tc.tile_pool(name="pat3", bufs=1) as pat3, \
             tc.tile_pool(name="ps_qk", bufs=3, space="PSUM") as ps_qk, \
             tc.tile_pool(name="ps_s", bufs=2, space="PSUM") as ps_s, \
             tc.tile_pool(name="ps_av", bufs=2, space="PSUM") as ps_av, \
             tc.tile_pool(name="ps_pa", bufs=1, space="PSUM") as ps_pa:

            def emit_filler(h):
                # late ada chunks (6..17); fc1 fp8 stream-convert to DRAM
                if h < 12:
                    emit_ada_chunk(6 + h, p3w, ps_pa, ps_bufs=1)
                if h == 11:
                    emit_ada_pp_load([2, 3])
                    emit_ada_pp_load([4, 5])
                if 3 <= h < 12:
                    for j in ((h - 3) * 2, (h - 3) * 2 + 1):
                        f1st = p3w.tile([128, KT, 256], F32, tag="f1st",
                                        bufs=2, name="f1st")
                        nc.sync.dma_start(
                            f1st[:, :, :],
                            ins["w_fc1"][:, j * 256:(j + 1) * 256]
                            .rearrange("(k p) m -> p k m", p=128),
                        )
                        f18o = p3w.tile([128, KT, 256], FP8, tag="f18o",
                                        bufs=2, name="f18o")
                        nc.gpsimd.tensor_scalar_mul(
                            f18o[:, :, :], f1st[:, :, :], WS)
                        nc.gpsimd.dma_start(w1f8_dr[j, :, :, :],
                                            f18o[:, :, :])

            for h in range(H):
                emit_filler(h)
                q_h = pat3.tile([72, NT], FP8, tag="qh", bufs=2, name="q_h")
                k_h = pat3.tile([72, NT], FP8, tag="kh", bufs=2, name="k_h")
                for n in range(2):
                    nsl = slice(n * 512, (n + 1) * 512)
                    pq = ps_qk.tile([72, 512], F32, tag="qk", name="pq")
                    for i in range(KTP // 2):
                        nc.tensor.matmul(
                            pq[:, :],
                            wq8[:, 2 * i:2 * i + 2, h * HD:(h + 1) * HD],
                            mod12T[:, 2 * i:2 * i + 2, nsl],
                            start=(i == 0), stop=(i == KTP // 2 - 1),
                            perf_mode=DR,
                        )
                    nc.vector.tensor_scalar(
                        q_h[:, nsl], pq[:, :], QS / (AS * WS),
                        bq_s[:, h:h + 1], ALU.mult, ALU.add,
                    )
                for n in range(2):
                    nsl = slice(n * 512, (n + 1) * 512)
                    pk = ps_qk.tile([72, 512], F32, tag="qk", name="pk")
                    for i in range(KTP // 2):
                        nc.tensor.matmul(
                            pk[:, :],
                            wk8[:, 2 * i:2 * i + 2, h * HD:(h + 1) * HD],
                            mod12T[:, 2 * i:2 * i + 2, nsl],
                            start=(i == 0), stop=(i == KTP // 2 - 1),
                            perf_mode=DR,
                        )
                    nc.vector.tensor_scalar(
                        k_h[:, nsl], pk[:, :], QS / (AS * WS),
                        bk_s[:, h:h + 1], ALU.mult, ALU.add,
                    )
                for n in range(2):
                    nsl = slice(n * 512, (n + 1) * 512)
                    exp_hn = pexp.tile([128, NT // 128, 512], FP8, tag="exp",
                                       bufs=3, name="exp_hn")
                    for kt_i in range(NT // 128):
                        pss = ps_s.tile([128, 512], F32, tag="s", name="pss")
                        nc.tensor.matmul(
                            pss[:, :],
                            k_h[:, kt_i * 128:(kt_i + 1) * 128],
                            q_h[:, nsl], start=True, stop=True,
                        )
                        nc.scalar.activation(
                            exp_hn[:, kt_i, :], pss[:, :], AF.Exp, scale=ES)
                    pav = ps_av.tile([97, 512], F32, tag="av", name="pav")
                    for i in range(NT // 256):
                        nc.tensor.matmul(
                            pav[:, :],
                            v_aug[:, 2 * i:2 * i + 2, h, :],
                            exp_hn[:, 2 * i:2 * i + 2, :],
                            start=(i == 0), stop=(i == NT // 256 - 1),
                            perf_mode=DR,
                        )
                    den = pat3.tile([1, 512], F32, tag="den", bufs=2,
                                    name="den")
                    nc.vector.tensor_scalar_mul(den[:, :], pav[96:97, :],
                                                1.0 / PS)
                    nc.vector.reciprocal(den[:, :], den[:, :])
                    denB = pat3.tile([72, 512], F32, tag="denB", bufs=2,
                                     name="denB")
                    nc.gpsimd.partition_broadcast(denB[:, :], den[:, :])
                    nc.vector.tensor_mul(
                        attn_hs[:, h, nsl], pav[0:HD, :], denB[:, :])
        es_qk.close()  # wq8/wk8 no longer needed
        es_va.close()

        # ================= phase D: proj + residual + LN2 ===================
        es_w2 = ExitStack()
        pw2 = es_w2.enter_context(
            tc.tile_pool(name="pw2", bufs=1, side="right"))
        w2f8 = pw2.tile([128, KT, MH, 128], FP8, name="w2f8")

        with tc.tile_pool(name="p4", bufs=1) as p4, \
             tc.tile_pool(name="pst4", bufs=1) as pst4, \
             tc.tile_pool(name="pln4", bufs=1) as pln4:
            for i in range(6):
                msl = slice(i * 192, (i + 1) * 192)
                wpst = p4.tile([72, H, 192], F32, tag="wpst", bufs=2,
                               name="wpst")
                nc.sync.dma_start(
                    wpst[:, :, :],
                    ins["w_proj"][:, msl].rearrange("(h p) m -> p h m", p=72),
                )
                for kk in range(2):
                    hsl = slice(kk * 8, kk * 8 + 8)
                    nc.vector.tensor_scalar_mul(
                        wp8[:, hsl, msl], wpst[:, hsl, :], WS)

            def emit_fc2_chunk(ch, eng, pool):
                f2s = pool.tile([128, MH, 64], F32, tag="f2s", bufs=2,
                                name="f2s")
                nc.sync.dma_start(
                    f2s[:, :, :],
                    ins["w_fc2"][:, ch * 64:(ch + 1) * 64]
                    .rearrange("(k p) m -> p k m", p=128),
                )
                eng.tensor_scalar_mul(
                    w2f8[:, ch // 2, :, (ch % 2) * 64:(ch % 2 + 1) * 64],
                    f2s[:, :, :], WS)

            st2 = {}
            with tc.tile_pool(name="ps_mm2", bufs=3, space="PSUM") as ps_mm2, \
                 tc.tile_pool(name="ps_st2", bufs=4, space="PSUM") as ps_st2:
                for n in range(2):
                    nsl = slice(n * 512, (n + 1) * 512)
                    for mo in range(KT):
                        if mo < 6:
                            ch = n * 6 + mo
                            eng = nc.vector if ch % 2 else nc.gpsimd
                            emit_fc2_chunk(ch, eng, p4)
                        pm2 = ps_mm2.tile([128, 512], F32, tag="mm2",
                                          name="pm2")
                        for i in range(H // 2):
                            nc.tensor.matmul(
                                pm2[:, :],
                                wp8[:, 2 * i:2 * i + 2,
                                    mo * 128:(mo + 1) * 128],
                                attn_hs[:, 2 * i:2 * i + 2, nsl],
                                start=(i == 0), stop=(i == H // 2 - 1),
                                perf_mode=DR,
                            )
                        t_sb = p4.tile([128, 512], F32, tag="tsb", bufs=2,
                                       name="t_sb")
                        nc.scalar.activation(
                            t_sb[:, :], pm2[:, :], AF.Identity,
                            bias=bproj_pp[:, mo:mo + 1], scale=1.0 / (PS * WS),
                        )
                        nc.vector.scalar_tensor_tensor(
                            xT[:, mo, nsl], t_sb[:, :],
                            ada_pp[:, 2, mo:mo + 1], xT[:, mo, nsl],
                            ALU.mult, ALU.add,
                        )
                    _ln_stats(tc, nc, xT, ones_col, pst4, pln4, ps_st2,
                              halves=(n,), st=st2)
                    _ln_apply(tc, nc, xT, mod12T, st2, ada_pp, 3, 4, pln4,
                              halves=(n,))

        # ================= phase E: FFN =====================================
        es_e = ExitStack()
        ph = es_e.enter_context(tc.tile_pool(name="ph", bufs=1))
        hT = ph.tile([128, MH, NT], FP8, name="hT")
        po = es_e.enter_context(tc.tile_pool(name="po", bufs=1))

        with tc.tile_pool(name="ps_f1", bufs=3, space="PSUM") as ps_f1, \
             tc.tile_pool(name="ps_f2", bufs=2, space="PSUM") as ps_f2, \
             tc.tile_pool(name="ps_tro", bufs=2, space="PSUM") as ps_tro:
            # fc1 in 18 chunks of 256 columns (2 m-tiles each), weights
            # already converted to fp8 in DRAM during the attention window
            with tc.tile_pool(name="p5a", bufs=1) as p5a:
                for ch in range(18):
                    f18 = p5a.tile([128, KTP, 256], FP8, tag="f18", bufs=3,
                                   name="f18")
                    nc.sync.dma_start(f18[:, 0:KT, :], w1f8_dr[ch, :, :, :])
                    nc.gpsimd.memset(f18[:, KT, :], 0.0)
                    if ch < 6:
                        emit_fc2_chunk(12 + ch,
                                       nc.vector if ch % 2 else nc.gpsimd,
                                       p5a)
                    for m in range(2):
                        mo = ch * 2 + m
                        for n in range(2):
                            nsl = slice(n * 512, (n + 1) * 512)
                            pf1 = ps_f1.tile([128, 512], F32, tag="f1",
                                             name="pf1")
                            for i in range(KTP // 2):
                                nc.tensor.matmul(
                                    pf1[:, :],
                                    f18[:, 2 * i:2 * i + 2,
                                        m * 128:(m + 1) * 128],
                                    mod12T[:, 2 * i:2 * i + 2, nsl],
                                    start=(i == 0), stop=(i == KTP // 2 - 1),
                                    perf_mode=DR,
                                )
                            nc.scalar.activation(
                                hT[:, mo, nsl], pf1[:, :], AF.Gelu_apprx_tanh,
                                bias=bfc1_pp[:, mo:mo + 1],
                                scale=1.0 / (AS * WS),
                            )
            # fc2: weights already fp8-resident in SBUF (w2f8)
            with tc.tile_pool(name="p5b", bufs=1) as p5b:
                for mo in range(KT):
                    for n in range(2):
                        nsl = slice(n * 512, (n + 1) * 512)
                        pf2 = ps_f2.tile([128, 512], F32, tag="f2", name="pf2")
                        for i in range(MH // 2):
                            nc.tensor.matmul(
                                pf2[:, :], w2f8[:, mo, 2 * i:2 * i + 2, :],
                                hT[:, 2 * i:2 * i + 2, nsl],
                                start=(i == 0), stop=(i == MH // 2 - 1),
                                perf_mode=DR,
                            )
                        t2 = p5b.tile([128, 512], F32, tag="t2", bufs=3,
                                      name="t2")
                        nc.scalar.activation(
                            t2[:, :], pf2[:, :], AF.Identity,
                            bias=bfc2_pp[:, mo:mo + 1], scale=1.0 / WS,
                        )
                        nc.vector.scalar_tensor_tensor(
                            xT[:, mo, nsl], t2[:, :], ada_pp[:, 5, mo:mo + 1],
                            xT[:, mo, nsl], ALU.mult, ALU.add,
                        )
                    o_slab = po.tile([128, NT // 128, 128], F32, tag="osl",
                                     bufs=2, name="o_slab")
                    for tt in range(NT // 128):
                        pt = ps_tro.tile([128, 128], F32, tag="tro",
                                         name="pt6")
                        nc.tensor.transpose(
                            pt[:, :], xT[:, mo, tt * 128:(tt + 1) * 128],
                            ident[:, :],
                        )
                        dst = o_slab[:, tt, :]
                        if tt % 2 == 0:
                            nc.vector.tensor_copy(dst, pt[:, :])
                        else:
                            nc.scalar.copy(dst, pt[:, :])
                    nc.gpsimd.dma_start(
                        out_dram[:, mo * 128:(mo + 1) * 128]
                        .rearrange("(t p) m -> p t m", p=128),
                        o_slab[:, :, :])
        es_w2.close()
        es_wp.close()
        es_att.close()
        es_e.close()


_LOCK = threading.Lock()
_PROG = None


def _get_program():
    global _PROG
    with _LOCK:
        if _PROG is None:
            _PROG = _build_program()
    return _PROG


def _make_in_maps(inputs):
    arrs = {k: np.ascontiguousarray(np.asarray(v, dtype=np.float32))
            for k, v in inputs.items()}
    in_maps = []
    for c in range(NCORES):
        m = {k: v for k, v in arrs.items() if k not in ("x", "t_emb")}
        m["x"] = np.ascontiguousarray(arrs["x"][c])
        m["t_emb"] = np.ascontiguousarray(arrs["t_emb"][c])
        in_maps.append(m)
    return in_maps


def kernel(**inputs):
    nc = _get_program()
    res = run_bass_kernel_spmd(nc, _make_in_maps(inputs),
                               core_ids=list(range(NCORES)))
    return np.stack([r["out"] for r in res.results], axis=0)


def kernel_traced(inputs, **kw):
    """test-harness helper: returns full BassKernelResults with trace."""
    nc = _get_program()
    return run_bass_kernel_spmd(
        nc, _make_in_maps(inputs), core_ids=list(range(NCORES)), trace=True,
        **kw
    )


# revision 57
# speedup vs baseline: 1.4126x; 1.0257x over previous
"""DiT block kernel for Trainium2 (Bass/Tile), 8-core data parallel.

Shapes (hardcoded from the problem spec):
  x: (8, 1024, 1152), t_emb: (8, 1152)
  w_qkv (1152, 3456), w_proj (1152, 1152), w_fc1 (1152, 4608),
  w_fc2 (4608, 1152), w_ada (1152, 6912) + biases.

Strategy: batch-parallel across 8 cores (one batch element each, no
collectives). Activations live transposed [D on partitions, tokens free].
The large matmuls (qkv, attention AV, proj, fc1, fc2) run in fp8e4 with
DoubleRow perf mode (two 128-row k-tiles contracted per instruction);
scale factors for fp8 range are folded into the existing activation
bias/scale stages so no extra elementwise work is added.  LayerNorm
statistics reduce over the partition axis via ones-vector f32r matmuls;
softmax runs transposed (keys on partitions) with denominators collected
through a ones-column appended to V and a fused divide.  q/k are produced
per-head directly (M=72 matmuls cost the same per column as M=128), so
attention needs no partition-crossing gather DMAs.  Weights stream
through big staged f32 DMA loads (few, large transfers) and are
converted on-chip; ada (error-sensitive) stays f32r.
"""

import threading
from contextlib import ExitStack

import numpy as np

import concourse.bass as bass
import concourse.mybir as mybir
import concourse.tile as tile
from concourse import bacc
from concourse.bass_utils import run_bass_kernel_spmd
from concourse.masks import make_identity

F32 = mybir.dt.float32
F32R = mybir.dt.float32r
BF16 = mybir.dt.bfloat16
FP8 = mybir.dt.float8e4
AF = mybir.ActivationFunctionType
ALU = mybir.AluOpType
DR = mybir.MatmulPerfMode.DoubleRow

NCORES = 8
D = 1152
NT = 1024
KT = D // 128       # 9
KTP = KT + 1        # padded to even for DoubleRow pairs
H = 16
HD = 72
HID = 4 * D
MH = HID // 128     # 36
EPS = 1e-6
ISC = 1.0 / float(np.sqrt(HD))

# fp8 scale factors
WS = 64.0           # weights
AS = 8.0            # modulated activations (mod1/mod2)
QS = 2.0            # q/k
PS = 4.0            # attention output
ES = ISC / (QS * QS)  # exp() input scale applied to the scores psum

# v output column slices aligned to head boundaries
V_SLICES = [(0, 432, 0, 6), (432, 864, 6, 12), (864, 1152, 12, 16)]


def _r(ap):
    return ap.bitcast(F32R)


def _build_program():
    nc = bacc.Bacc(
        "TRN2", target_bir_lowering=False, debug=False, enable_asserts=False
    )
    ins = {}
    ins["x"] = nc.dram_tensor("x", [NT, D], F32, kind="ExternalInput").ap()
    ins["t_emb"] = nc.dram_tensor("t_emb", [D], F32, kind="ExternalInput").ap()
    for name, shape in [
        ("w_qkv", [D, 3 * D]), ("b_qkv", [3 * D]),
        ("w_proj", [D, D]), ("b_proj", [D]),
        ("w_fc1", [D, HID]), ("b_fc1", [HID]),
        ("w_fc2", [HID, D]), ("b_fc2", [D]),
        ("w_ada", [D, 6 * D]), ("b_ada", [6 * D]),
    ]:
        ins[name] = nc.dram_tensor(name, shape, F32, kind="ExternalInput").ap()
    out_dram = nc.dram_tensor("out", [NT, D], F32, kind="ExternalOutput").ap()

    with tile.TileContext(nc) as tc:
        _body(tc, ins, out_dram)
    nc.compile()
    return nc


def _ln_stats(tc, nc, src, ones_col, pst, pln, ps_st, halves=(0, 1),
              st=None):
    """Return st[n] = [mean; rstd] rows [1, 2, 512] per 512-token half,
    reducing over the partition (D) axis of src [128, KT, NT] f32."""
    ps_x, ps_q = {}, {}
    if st is None:
        st = {}
    for n in halves:
        nsl = slice(n * 512, (n + 1) * 512)
        ps_x[n] = ps_st.tile([1, 512], F32, tag="st", name=f"psx{n}")
        ps_q[n] = ps_st.tile([1, 512], F32, tag="st", name=f"psq{n}")
        for k in range(KT):
            xb = pln.tile([128, 512], BF16, tag="xb", bufs=2, name="xb")
            nc.scalar.copy(xb[:, :], src[:, k, nsl])
            sq = pln.tile([128, 512], BF16, tag="sq", bufs=2, name="sq")
            if n == 0:
                nc.vector.tensor_mul(sq[:, :], src[:, k, nsl], src[:, k, nsl])
            else:
                nc.gpsimd.tensor_mul(sq[:, :], src[:, k, nsl], src[:, k, nsl])
            nc.tensor.matmul(
                ps_x[n][:, :], ones_col[:, :], xb[:, :],
                start=(k == 0), stop=(k == KT - 1), skip_group_check=True,
            )
            nc.tensor.matmul(
                ps_q[n][:, :], ones_col[:, :], sq[:, :],
                start=(k == 0), stop=(k == KT - 1), skip_group_check=True,
            )
    eps_sb = pst.tile([1, 1], F32, tag="eps", bufs=1, name="eps_sb")
    nc.vector.memset(eps_sb[:, :], EPS)
    for n in halves:
        st[n] = pst.tile([1, 2, 512], F32, tag="lnst", bufs=2, name=f"st{n}")
        nc.vector.tensor_scalar_mul(st[n][:, 0, :], ps_x[n][:, :], 1.0 / D)
        work = pst.tile([1, 512], F32, tag="lnwork", bufs=2, name="work")
        nc.vector.tensor_mul(work[:, :], st[n][:, 0, :], st[n][:, 0, :])
        nc.vector.scalar_tensor_tensor(
            st[n][:, 1, :], ps_q[n][:, :], 1.0 / D, work[:, :],
            ALU.mult, ALU.subtract,
        )
        nc.scalar.activation(st[n][:, 1, :], st[n][:, 1, :], AF.Sqrt,
                             bias=eps_sb[:, :], scale=1.0)
        nc.vector.reciprocal(st[n][:, 1, :], st[n][:, 1, :])
    return st


def _ln_apply(tc, nc, src, dst, st, ada_pp, sh_c, sc_c, pln,
              halves=(0, 1)):
    """dst[:,k,nsl] (fp8) = ((src-mean)*rstd) * ada[sc_c] + ada[sh_c]
    (ada params pre-scaled by AS)."""
    for n in halves:
        nsl = slice(n * 512, (n + 1) * 512)
        meanB = pln.tile([128, 512], F32, tag="meanB", bufs=2, name="meanB")
        rstdB = pln.tile([128, 512], F32, tag="rstdB", bufs=2, name="rstdB")
        nc.gpsimd.partition_broadcast(meanB[:, :], st[n][:, 0, :])
        nc.gpsimd.partition_broadcast(rstdB[:, :], st[n][:, 1, :])
        eng0 = nc.vector if n == 0 else nc.gpsimd
        eng1 = nc.gpsimd if n == 0 else nc.vector
        for k in range(KT):
            t1 = pln.tile([128, 512], F32, tag="lnt1", bufs=2, name="t1")
            eng0.tensor_sub(t1[:, :], src[:, k, nsl], meanB[:, :])
            t2 = pln.tile([128, 512], F32, tag="lnt2", bufs=2, name="t2")
            eng0.tensor_mul(t2[:, :], t1[:, :], rstdB[:, :])
            eng1.tensor_scalar(
                dst[:, k, nsl], t2[:, :],
                ada_pp[:, sc_c, k:k + 1], ada_pp[:, sh_c, k:k + 1],
                ALU.mult, ALU.add,
            )


def _body(tc, ins, out_dram):
    nc = tc.nc
    ctx = ExitStack()
    with ctx:
        dram = ctx.enter_context(tc.tile_pool(name="dram", bufs=1, space="DRAM"))
        ada_dr = dram.tile([6 * D], F32)
        w1f8_dr = dram.tile([18, 128, KT, 256], FP8)

        pers = ctx.enter_context(tc.tile_pool(name="pers", bufs=1))
        ident = pers.tile([128, 128], F32)
        make_identity(nc, ident[:, :])
        ones_col = pers.tile([128, 1], BF16)
        nc.vector.memset(ones_col[:, :], 1.0)
        ones_row = pers.tile([1, 128], BF16)
        nc.vector.memset(ones_row[:, :], 1.0)

        t_pp = pers.tile([128, KT], F32)
        nc.sync.dma_start(t_pp[:, :], ins["t_emb"].rearrange("(k p) -> p k", p=128))
        t_pr = pers.tile([128, KT], F32R)
        nc.scalar.activation(t_pr[:, :], t_pp[:, :], AF.Silu)

        bq_s = pers.tile([72, H], F32)
        bk_s = pers.tile([72, H], F32)
        bv_row = pers.tile([1, D], F32)
        bv_b = pers.tile([1, D], BF16)
        bproj_pp = pers.tile([128, KT], F32)
        bfc1_pp = pers.tile([128, MH], F32)
        bfc2_pp = pers.tile([128, KT], F32)
        bada_pp = pers.tile([128, 6, KT], F32)
        ada_pp = pers.tile([128, 6, KT], F32)

        def emit_bias_loads():
            nc.sync.dma_start(
                bq_s[:, :], ins["b_qkv"][0:D].rearrange("(h p) -> p h", p=72))
            nc.sync.dma_start(
                bk_s[:, :], ins["b_qkv"][D:2 * D].rearrange("(h p) -> p h", p=72))
            nc.sync.dma_start(
                bv_row[:, :],
                ins["b_qkv"][2 * D:3 * D].rearrange("(a b) -> a b", a=1))
            # bv enters the v accumulation in (AS*WS)-scaled psum units
            nc.vector.tensor_scalar_mul(bv_b[:, :], bv_row[:, :], AS * WS)
            nc.sync.dma_start(
                bproj_pp[:, :], ins["b_proj"].rearrange("(m p) -> p m", p=128))
            nc.sync.dma_start(
                bfc1_pp[:, :], ins["b_fc1"].rearrange("(m p) -> p m", p=128))
            nc.sync.dma_start(
                bfc2_pp[:, :], ins["b_fc2"].rearrange("(m p) -> p m", p=128))
            nc.sync.dma_start(
                bada_pp[:, :, :],
                ins["b_ada"].rearrange("(c k p) -> p c k", k=KT, p=128))
            # pre-scale q/k biases by QS (folded into the psum->fp8 copies)
            nc.vector.tensor_scalar_mul(bq_s[:, :], bq_s[:, :], QS)
            nc.vector.tensor_scalar_mul(bk_s[:, :], bk_s[:, :], QS)

        xT = pers.tile([128, KT, NT], F32)      # becomes x2T after residual 1
        mod12T = pers.tile([128, KTP, NT], FP8)  # mod1T, later reused as mod2T
        nc.gpsimd.memset(mod12T[:, KT, :], 0.0)  # DoubleRow pad k-tile

        # ================= phase A: x load/transpose, ada, LN1 ==============

        def emit_ada_chunk(c, p1w, ps_pro, ps_bufs=2):
            """chunk c covers w_ada cols [c*384, (c+1)*384); param p=c//3."""
            wst = p1w.tile([128, KT, 384], F32R, tag="adast", bufs=2, name="wst")
            nc.sync.dma_start(
                wst[:, :, :],
                ins["w_ada"][:, c * 384:(c + 1) * 384]
                .rearrange("(k p) m -> p k m", p=128).bitcast(F32R),
            )
            pa = ps_pro.tile([1, 384], F32, tag="psada", bufs=ps_bufs,
                             name="pa")
            for k in range(KT):
                nc.tensor.matmul(
                    pa[:, :], t_pr[:, k:k + 1], wst[:, k, :],
                    start=(k == 0), stop=(k == KT - 1),
                )
            asb = p1w.tile([1, 384], F32, tag="asb", bufs=3, name="asb")
            nc.vector.tensor_copy(asb[:, :], pa[:, :])
            nc.gpsimd.dma_start(
                ada_dr[c * 384:(c + 1) * 384].rearrange("(a b) -> a b", a=1),
                asb[0:1, :],
            )

        def emit_ada_pp_load(cs):
            """Load+finalize ada params cs (list) into ada_pp; params 0/1
            (shift_a/scale_a) and 3/4 are pre-scaled by AS; 1/4 get +1."""
            for c in cs:
                nc.gpsimd.dma_start(
                    ada_pp[:, c, :],
                    ada_dr[c * D:(c + 1) * D].rearrange("(k p) -> p k", p=128),
                )
            lo, hi = min(cs), max(cs) + 1
            nc.vector.tensor_add(ada_pp[:, lo:hi, :], ada_pp[:, lo:hi, :],
                                 bada_pp[:, lo:hi, :])
            for c in cs:
                if c in (1, 4):
                    nc.vector.tensor_scalar_add(ada_pp[:, c, :],
                                                ada_pp[:, c, :], 1.0)
                if c in (0, 1, 3, 4):
                    nc.vector.tensor_scalar_mul(ada_pp[:, c, :],
                                                ada_pp[:, c, :], AS)

        with tc.tile_pool(name="p1w", bufs=1) as p1w, \
             tc.tile_pool(name="pxin", bufs=3) as pxin, \
             tc.tile_pool(name="ps_pro", bufs=2, space="PSUM") as ps_pro, \
             tc.tile_pool(name="ps_tr", bufs=2, space="PSUM") as ps_tr:

            def emit_transpose_block(tt):
                xin = pxin.tile([128, D], F32, tag="xin", name="xin")
                nc.sync.dma_start(
                    xin[:, :], ins["x"][tt * 128:(tt + 1) * 128, :])
                for kd in range(KT):
                    pt = ps_tr.tile([128, 128], F32, tag="ptr", name="pt")
                    nc.tensor.transpose(
                        pt[:, :], xin[:, kd * 128:(kd + 1) * 128], ident[:, :])
                    tsl = slice(tt * 128, (tt + 1) * 128)
                    if kd % 2 == 0:
                        nc.vector.tensor_copy(xT[:, kd, tsl], pt[:, :])
                    else:
                        nc.scalar.copy(xT[:, kd, tsl], pt[:, :])

            for i in range(8):
                emit_transpose_block(i)
                if i == 0:
                    emit_bias_loads()
                if i < 6:
                    emit_ada_chunk(i, p1w, ps_pro)
            emit_ada_pp_load([0, 1])

        # ====== phase B part 1: qkv weight loads + converts (emitted before
        # LN1 so SP streams the loads while ada finishes / LN runs) =========
        es_qk = ExitStack()
        pqk8 = es_qk.enter_context(tc.tile_pool(name="pqk8", bufs=1))
        wq8 = pqk8.tile([128, KTP, D], FP8, name="wq8")
        wk8 = pqk8.tile([128, KTP, D], FP8, name="wk8")
        nc.gpsimd.memset(wq8[:, KT, :], 0.0)
        nc.gpsimd.memset(wk8[:, KT, :], 0.0)

        es_att = ExitStack()
        patt = es_att.enter_context(tc.tile_pool(name="patt", bufs=1, side="right"))
        attn_hs = patt.tile([72, H, NT], FP8, name="attn_hs")
        es_wp = ExitStack()
        pwp8 = es_wp.enter_context(
            tc.tile_pool(name="pwp8", bufs=1, side="right"))
        wp8 = pwp8.tile([72, H, D], FP8, name="wp8")
        es_va = ExitStack()
        pva = es_va.enter_context(tc.tile_pool(name="pva", bufs=1, side="right"))
        v_aug = pva.tile([128, NT // 128, H, 97], FP8, name="v_aug")
        nc.gpsimd.memset(v_aug[:, :, :, HD:97], 0.0)
        nc.gpsimd.memset(v_aug[:, :, :, 96:97], 1.0)

        es_b = ExitStack()
        pwst = es_b.enter_context(tc.tile_pool(name="pwst", bufs=1))
        wv8 = pwst.tile([128, KTP, D], FP8, tag="wv8", bufs=1, name="wv8")
        nc.gpsimd.memset(wv8[:, KT, :], 0.0)
        engs = ["act", "dve", "act", "dve", "act", "dve"]
        for j, (dst8, c0) in enumerate(((wq8, 0), (wk8, D), (wv8, 2 * D))):
            for half in range(2):
                msl = slice(half * 576, (half + 1) * 576)
                wst = pwst.tile([128, KT, 576], F32, tag="wst", bufs=2,
                                name="wst")
                nc.sync.dma_start(
                    wst[:, :, :],
                    ins["w_qkv"][:, c0 + half * 576:c0 + (half + 1) * 576]
                    .rearrange("(k p) m -> p k m", p=128),
                )
                eng = engs[j * 2 + half]
                for kk in range(3):
                    ksl = slice(kk * 3, kk * 3 + 3)
                    if eng == "act":
                        nc.scalar.activation(
                            dst8[:, ksl, msl], wst[:, ksl, :],
                            AF.Identity, scale=WS)
                    elif eng == "dve":
                        nc.vector.tensor_scalar_mul(
                            dst8[:, ksl, msl], wst[:, ksl, :], WS)
                    else:
                        nc.gpsimd.tensor_scalar_mul(
                            dst8[:, ksl, msl], wst[:, ksl, :], WS)

        # ====== LN1 (per-half, interleaved with v matmuls) ==================
        with tc.tile_pool(name="pst", bufs=1) as pst, \
             tc.tile_pool(name="pln", bufs=1) as pln, \
             tc.tile_pool(name="ps_st", bufs=4, space="PSUM") as ps_st, \
             tc.tile_pool(name="ps_v", bufs=3, space="PSUM") as ps_v:

            def v_block(tts):
                for tt in tts:
                    tsl = slice(tt * 128, (tt + 1) * 128)
                    for si, (c0, c1, h0, h1) in enumerate(V_SLICES):
                        pmv = ps_v.tile([128, 512], F32, tag="mv", name="pmv")
                        for i in range(KTP // 2):
                            nc.tensor.matmul(
                                pmv[:, 0:c1 - c0],
                                mod12T[:, 2 * i:2 * i + 2, tsl],
                                wv8[:, 2 * i:2 * i + 2, c0:c1],
                                start=(i == 0), stop=False, perf_mode=DR,
                                skip_group_check=True,
                            )
                        nc.tensor.matmul(
                            pmv[:, 0:c1 - c0], ones_row[:, :],
                            bv_b[:, c0:c1],
                            start=False, stop=True, skip_group_check=True,
                        )
                        vsrc = pmv[:, 0:c1 - c0].rearrange(
                            "p (h d) -> p h d", d=HD)
                        nc.vector.tensor_scalar_mul(
                            v_aug[:, tt, h0:h1, 0:HD], vsrc, 1.0 / (AS * WS))

            st1 = {}
            _ln_stats(tc, nc, xT, ones_col, pst, pln, ps_st, halves=(0,),
                      st=st1)
            _ln_apply(tc, nc, xT, mod12T, st1, ada_pp, 0, 1, pln, halves=(0,))
            _ln_stats(tc, nc, xT, ones_col, pst, pln, ps_st, halves=(1,),
                      st=st1)
            v_block(range(0, 4))
            _ln_apply(tc, nc, xT, mod12T, st1, ada_pp, 0, 1, pln, halves=(1,))
            v_block(range(4, 8))
        es_b.close()

        # ================= phase C: attention ===============================
        with tc.tile_pool(name="p3w", bufs=1) as p3w, \
             tc.tile_pool(name="pexp", bufs=1) as pexp, \
             tc.tile_pool(name="pat3", bufs=1) as pat3, \
             tc.tile_pool(name="ps_qk", bufs=2, space="PSUM") as ps_qk, \
             tc.tile_pool(name="ps_s", bufs=2, space="PSUM") as ps_s, \
             tc.tile_pool(name="ps_av", bufs=1, space="PSUM") as ps_av, \
             tc.tile_pool(name="ps_pa", bufs=1, space="PSUM") as ps_pa:

            def emit_filler(h):
                # late ada chunks (6..17); fc1 fp8 stream-convert to DRAM
                if h < 12:
                    emit_ada_chunk(6 + h, p3w, ps_pa, ps_bufs=1)
                if h == 11:
                    emit_ada_pp_load([2, 3])
                    emit_ada_pp_load([4, 5])
                if 3 <= h < 12:
                    for j in ((h - 3) * 2, (h - 3) * 2 + 1):
                        f1st = p3w.tile([128, KT, 256], F32, tag="f1st",
                                        bufs=2, name="f1st")
                        nc.sync.dma_start(
                            f1st[:, :, :],
                            ins["w_fc1"][:, j * 256:(j + 1) * 256]
                            .rearrange("(k p) m -> p k m", p=128),
                        )
                        f18o = p3w.tile([128, KT, 256], FP8, tag="f18o",
                                        bufs=2, name="f18o")
                        nc.gpsimd.tensor_scalar_mul(
                            f18o[:, :, :], f1st[:, :, :], WS)
                        nc.gpsimd.dma_start(w1f8_dr[j, :, :, :],
                                            f18o[:, :, :])

            for h in range(H):
                emit_filler(h)
                q_h = pat3.tile([72, NT], FP8, tag="qh", bufs=2, name="q_h")
                k_h = pat3.tile([72, NT], FP8, tag="kh", bufs=2, name="k_h")
                for n in range(2):
                    nsl = slice(n * 512, (n + 1) * 512)
                    pq = ps_qk.tile([72, 512], F32, tag="qk", name="pq")
                    for i in range(KTP // 2):
                        nc.tensor.matmul(
                            pq[:, :],
                            wq8[:, 2 * i:2 * i + 2, h * HD:(h + 1) * HD],
                            mod12T[:, 2 * i:2 * i + 2, nsl],
                            start=(i == 0), stop=(i == KTP // 2 - 1),
                            perf_mode=DR,
                        )
                    nc.vector.tensor_scalar(
                        q_h[:, nsl], pq[:, :], QS / (AS * WS),
                        bq_s[:, h:h + 1], ALU.mult, ALU.add,
                    )
                for n in range(2):
                    nsl = slice(n * 512, (n + 1) * 512)
                    pk = ps_qk.tile([72, 512], F32, tag="qk", name="pk")
                    for i in range(KTP // 2):
                        nc.tensor.matmul(
                            pk[:, :],
                            wk8[:, 2 * i:2 * i + 2, h * HD:(h + 1) * HD],
                            mod12T[:, 2 * i:2 * i + 2, nsl],
                            start=(i == 0), stop=(i == KTP // 2 - 1),
                            perf_mode=DR,
                        )
                    nc.vector.tensor_scalar(
                        k_h[:, nsl], pk[:, :], QS / (AS * WS),
                        bk_s[:, h:h + 1], ALU.mult, ALU.add,
                    )
                for n in range(2):
                    nsl = slice(n * 512, (n + 1) * 512)
                    exp_hn = pexp.tile([128, NT // 128, 512], FP8, tag="exp",
                                       bufs=3, name="exp_hn")
                    for kp in range(NT // 256):
                        pss = ps_s.tile([128, 2, 512], F32, tag="s",
                                        name="pss")
                        for j in range(2):
                            kt_i = 2 * kp + j
                            nc.tensor.matmul(
                                pss[:, j, :],
                                k_h[:, kt_i * 128:(kt_i + 1) * 128],
                                q_h[:, nsl], start=True, stop=True,
                            )
                        nc.scalar.activation(
                            exp_hn[:, 2 * kp:2 * kp + 2, :],
                            pss[:, :, :], AF.Exp, scale=ES)
                    pav = ps_av.tile([97, 512], F32, tag="av", name="pav")
                    for i in range(NT // 256):
                        nc.tensor.matmul(
                            pav[:, :],
                            v_aug[:, 2 * i:2 * i + 2, h, :],
                            exp_hn[:, 2 * i:2 * i + 2, :],
                            start=(i == 0), stop=(i == NT // 256 - 1),
                            perf_mode=DR,
                        )
                    den = pat3.tile([1, 512], F32, tag="den", bufs=2,
                                    name="den")
                    nc.vector.tensor_scalar_mul(den[:, :], pav[96:97, :],
                                                1.0 / PS)
                    nc.vector.reciprocal(den[:, :], den[:, :])
                    denB = pat3.tile([72, 512], F32, tag="denB", bufs=2,
                                     name="denB")
                    nc.gpsimd.partition_broadcast(denB[:, :], den[:, :])
                    nc.vector.tensor_mul(
                        attn_hs[:, h, nsl], pav[0:HD, :], denB[:, :])
        es_qk.close()  # wq8/wk8 no longer needed
        es_va.close()

        # ================= phase D: proj + residual + LN2 ===================
        es_w2 = ExitStack()
        pw2 = es_w2.enter_context(
            tc.tile_pool(name="pw2", bufs=1, side="right"))
        w2f8 = pw2.tile([128, KT, MH, 128], FP8, name="w2f8")

        with tc.tile_pool(name="p4", bufs=1) as p4, \
             tc.tile_pool(name="pst4", bufs=1) as pst4, \
             tc.tile_pool(name="pln4", bufs=1) as pln4:
            for i in range(6):
                msl = slice(i * 192, (i + 1) * 192)
                wpst = p4.tile([72, H, 192], F32, tag="wpst", bufs=2,
                               name="wpst")
                nc.sync.dma_start(
                    wpst[:, :, :],
                    ins["w_proj"][:, msl].rearrange("(h p) m -> p h m", p=72),
                )
                for kk in range(2):
                    hsl = slice(kk * 8, kk * 8 + 8)
                    nc.vector.tensor_scalar_mul(
                        wp8[:, hsl, msl], wpst[:, hsl, :], WS)

            def emit_fc2_chunk(ch, eng, pool):
                f2s = pool.tile([128, MH, 64], F32, tag="f2s", bufs=2,
                                name="f2s")
                nc.sync.dma_start(
                    f2s[:, :, :],
                    ins["w_fc2"][:, ch * 64:(ch + 1) * 64]
                    .rearrange("(k p) m -> p k m", p=128),
                )
                eng.tensor_scalar_mul(
                    w2f8[:, ch // 2, :, (ch % 2) * 64:(ch % 2 + 1) * 64],
                    f2s[:, :, :], WS)

            st2 = {}
            with tc.tile_pool(name="ps_mm2", bufs=3, space="PSUM") as ps_mm2, \
                 tc.tile_pool(name="ps_st2", bufs=4, space="PSUM") as ps_st2:
                for n in range(2):
                    nsl = slice(n * 512, (n + 1) * 512)
                    for mo in range(KT):
                        if mo < 6:
                            ch = n * 6 + mo
                            eng = nc.vector if ch % 2 else nc.gpsimd
                            emit_fc2_chunk(ch, eng, p4)
                        pm2 = ps_mm2.tile([128, 512], F32, tag="mm2",
                                          name="pm2")
                        for i in range(H // 2):
                            nc.tensor.matmul(
                                pm2[:, :],
                                wp8[:, 2 * i:2 * i + 2,
                                    mo * 128:(mo + 1) * 128],
                                attn_hs[:, 2 * i:2 * i + 2, nsl],
                                start=(i == 0), stop=(i == H // 2 - 1),
                                perf_mode=DR,
                            )
                        t_sb = p4.tile([128, 512], F32, tag="tsb", bufs=2,
                                       name="t_sb")
                        nc.scalar.activation(
                            t_sb[:, :], pm2[:, :], AF.Identity,
                            bias=bproj_pp[:, mo:mo + 1], scale=1.0 / (PS * WS),
                        )
                        nc.vector.scalar_tensor_tensor(
                            xT[:, mo, nsl], t_sb[:, :],
                            ada_pp[:, 2, mo:mo + 1], xT[:, mo, nsl],
                            ALU.mult, ALU.add,
                        )
                    _ln_stats(tc, nc, xT, ones_col, pst4, pln4, ps_st2,
                              halves=(n,), st=st2)
                    _ln_apply(tc, nc, xT, mod12T, st2, ada_pp, 3, 4, pln4,
                              halves=(n,))

        # ================= phase E: FFN =====================================
        es_e = ExitStack()
        ph = es_e.enter_context(tc.tile_pool(name="ph", bufs=1))
        hT = ph.tile([128, MH, NT], FP8, name="hT")
        po = es_e.enter_context(tc.tile_pool(name="po", bufs=1))

        with tc.tile_pool(name="ps_f1", bufs=3, space="PSUM") as ps_f1, \
             tc.tile_pool(name="ps_f2", bufs=2, space="PSUM") as ps_f2, \
             tc.tile_pool(name="ps_tro", bufs=2, space="PSUM") as ps_tro:
            # fc1 in 18 chunks of 256 columns (2 m-tiles each), weights
            # already converted to fp8 in DRAM during the attention window
            with tc.tile_pool(name="p5a", bufs=1) as p5a:
                for ch in range(18):
                    f18 = p5a.tile([128, KTP, 256], FP8, tag="f18", bufs=3,
                                   name="f18")
                    nc.sync.dma_start(f18[:, 0:KT, :], w1f8_dr[ch, :, :, :])
                    nc.gpsimd.memset(f18[:, KT, :], 0.0)
                    if ch < 6:
                        emit_fc2_chunk(12 + ch,
                                       nc.vector if ch % 2 else nc.gpsimd,
                                       p5a)
                    for m in range(2):
                        mo = ch * 2 + m
                        for n in range(2):
                            nsl = slice(n * 512, (n + 1) * 512)
                            pf1 = ps_f1.tile([128, 512], F32, tag="f1",
                                             name="pf1")
                            for i in range(KTP // 2):
                                nc.tensor.matmul(
                                    pf1[:, :],
                                    f18[:, 2 * i:2 * i + 2,
                                        m * 128:(m + 1) * 128],
                                    mod12T[:, 2 * i:2 * i + 2, nsl],
                                    start=(i == 0), stop=(i == KTP // 2 - 1),
                                    perf_mode=DR,
                                )
                            nc.scalar.activation(
                                hT[:, mo, nsl], pf1[:, :], AF.Gelu_apprx_tanh,
                                bias=bfc1_pp[:, mo:mo + 1],
                                scale=1.0 / (AS * WS),
                            )
            # fc2: weights already fp8-resident in SBUF (w2f8)
            with tc.tile_pool(name="p5b", bufs=1) as p5b:
                for mo in range(KT):
                    for n in range(2):
                        nsl = slice(n * 512, (n + 1) * 512)
                        pf2 = ps_f2.tile([128, 512], F32, tag="f2", name="pf2")
                        for i in range(MH // 2):
                            nc.tensor.matmul(
                                pf2[:, :], w2f8[:, mo, 2 * i:2 * i + 2, :],
                                hT[:, 2 * i:2 * i + 2, nsl],
                                start=(i == 0), stop=(i == MH // 2 - 1),
                                perf_mode=DR,
                            )
                        t2 = p5b.tile([128, 512], F32, tag="t2", bufs=3,
                                      name="t2")
                        nc.scalar.activation(
                            t2[:, :], pf2[:, :], AF.Identity,
                            bias=bfc2_pp[:, mo:mo + 1], scale=1.0 / WS,
                        )
                        nc.vector.scalar_tensor_tensor(
                            xT[:, mo, nsl], t2[:, :], ada_pp[:, 5, mo:mo + 1],
                            xT[:, mo, nsl], ALU.mult, ALU.add,
                        )
                    o_slab = po.tile([128, NT // 128, 128], F32, tag="osl",
                                     bufs=2, name="o_slab")
                    for tt in range(NT // 128):
                        pt = ps_tro.tile([128, 128], F32, tag="tro",
                                         name="pt6")
                        nc.tensor.transpose(
                            pt[:, :], xT[:, mo, tt * 128:(tt + 1) * 128],
                            ident[:, :],
                        )
                        dst = o_slab[:, tt, :]
                        if tt % 2 == 0:
                            nc.vector.tensor_copy(dst, pt[:, :])
                        else:
                            nc.scalar.copy(dst, pt[:, :])
                    nc.gpsimd.dma_start(
                        out_dram[:, mo * 128:(mo + 1) * 128]
                        .rearrange("(t p) m -> p t m", p=128),
                        o_slab[:, :, :])
        es_w2.close()
        es_wp.close()
        es_att.close()
        es_e.close()


_LOCK = threading.Lock()
_PROG = None


def _get_program():
    global _PROG
    with _LOCK:
        if _PROG is None:
            _PROG = _build_program()
    return _PROG


def _make_in_maps(inputs):
    arrs = {k: np.ascontiguousarray(np.asarray(v, dtype=np.float32))
            for k, v in inputs.items()}
    in_maps = []
    for c in range(NCORES):
        m = {k: v for k, v in arrs.items() if k not in ("x", "t_emb")}
        m["x"] = np.ascontiguousarray(arrs["x"][c])
        m["t_emb"] = np.ascontiguousarray(arrs["t_emb"][c])
        in_maps.append(m)
    return in_maps


def kernel(**inputs):
    nc = _get_program()
    res = run_bass_kernel_spmd(nc, _make_in_maps(inputs),
                               core_ids=list(range(NCORES)))
    return np.stack([r["out"] for r in res.results], axis=0)


def kernel_traced(inputs, **kw):
    """test-harness helper: returns full BassKernelResults with trace."""
    nc = _get_program()
    return run_bass_kernel_spmd(
        nc, _make_in_maps(inputs), core_ids=list(range(NCORES)), trace=True,
        **kw
    )


# revision 64
# speedup vs baseline: 1.5241x; 1.0789x over previous
"""DiT block kernel for Trainium2 (Bass/Tile), 8-core data parallel.

Shapes (hardcoded from the problem spec):
  x: (8, 1024, 1152), t_emb: (8, 1152)
  w_qkv (1152, 3456), w_proj (1152, 1152), w_fc1 (1152, 4608),
  w_fc2 (4608, 1152), w_ada (1152, 6912) + biases.

Strategy: batch-parallel across 8 cores (one batch element each, no
collectives). Activations live transposed [D on partitions, tokens free].
The large matmuls (qkv, attention AV, proj, fc1, fc2) run in fp8e4 with
DoubleRow perf mode (two 128-row k-tiles contracted per instruction);
scale factors for fp8 range are folded into the existing activation
bias/scale stages so no extra elementwise work is added.  LayerNorm
statistics reduce over the partition axis via ones-vector f32r matmuls;
softmax runs transposed (keys on partitions) with denominators collected
through a ones-column appended to V and a fused divide.  q/k are produced
per-head directly (M=72 matmuls cost the same per column as M=128), so
attention needs no partition-crossing gather DMAs.  Weights stream
through big staged f32 DMA loads (few, large transfers) and are
converted on-chip; ada (error-sensitive) stays f32r.
"""

import threading
from contextlib import ExitStack

import numpy as np

import concourse.bass as bass
import concourse.mybir as mybir
import concourse.tile as tile
from concourse import bacc
from concourse.bass_utils import run_bass_kernel_spmd
from concourse.masks import make_identity

F32 = mybir.dt.float32
F32R = mybir.dt.float32r
BF16 = mybir.dt.bfloat16
FP8 = mybir.dt.float8e4
AF = mybir.ActivationFunctionType
ALU = mybir.AluOpType
DR = mybir.MatmulPerfMode.DoubleRow

NCORES = 8
D = 1152
NT = 1024
KT = D // 128       # 9
KTP = KT + 1        # padded to even for DoubleRow pairs
H = 16
HD = 72
HID = 4 * D
MH = HID // 128     # 36
EPS = 1e-6
ISC = 1.0 / float(np.sqrt(HD))

# fp8 scale factors
WS = 64.0           # weights
AS = 8.0            # modulated activations (mod1/mod2)
QS = 2.0            # q/k
PS = 4.0            # attention output
ES = ISC / (QS * QS)  # exp() input scale applied to the scores psum

# v output column slices aligned to head boundaries
V_SLICES = [(0, 432, 0, 6), (432, 864, 6, 12), (864, 1152, 12, 16)]


def _r(ap):
    return ap.bitcast(F32R)


def _build_program():
    nc = bacc.Bacc(
        "TRN2", target_bir_lowering=False, debug=False, enable_asserts=False
    )
    ins = {}
    ins["x"] = nc.dram_tensor("x", [NT, D], F32, kind="ExternalInput").ap()
    ins["t_emb"] = nc.dram_tensor("t_emb", [D], F32, kind="ExternalInput").ap()
    for name, shape in [
        ("w_qkv", [D, 3 * D]), ("b_qkv", [3 * D]),
        ("w_proj", [D, D]), ("b_proj", [D]),
        ("w_fc1", [D, HID]), ("b_fc1", [HID]),
        ("w_fc2", [HID, D]), ("b_fc2", [D]),
        ("w_ada", [D, 6 * D]), ("b_ada", [6 * D]),
    ]:
        ins[name] = nc.dram_tensor(name, shape, F32, kind="ExternalInput").ap()
    out_dram = nc.dram_tensor("out", [NT, D], F32, kind="ExternalOutput").ap()

    with tile.TileContext(nc) as tc:
        _body(tc, ins, out_dram)
    nc.compile()
    return nc


def _ln_stats(tc, nc, src, ones_col, pst, pln, ps_st, halves=(0, 1),
              st=None):
    """Return st[n] = [mean; rstd] rows [1, 2, 512] per 512-token half,
    reducing over the partition (D) axis of src [128, KT, NT] f32."""
    ps_x, ps_q = {}, {}
    if st is None:
        st = {}
    for n in halves:
        nsl = slice(n * 512, (n + 1) * 512)
        ps_x[n] = ps_st.tile([1, 512], F32, tag="st", name=f"psx{n}")
        ps_q[n] = ps_st.tile([1, 512], F32, tag="st", name=f"psq{n}")
        for k in range(KT):
            xb = pln.tile([128, 512], BF16, tag="xb", bufs=2, name="xb")
            nc.scalar.copy(xb[:, :], src[:, k, nsl])
            sq = pln.tile([128, 512], BF16, tag="sq", bufs=2, name="sq")
            nc.vector.tensor_mul(sq[:, :], src[:, k, nsl], src[:, k, nsl])
            nc.tensor.matmul(
                ps_x[n][:, :], ones_col[:, :], xb[:, :],
                start=(k == 0), stop=(k == KT - 1), skip_group_check=True,
            )
            nc.tensor.matmul(
                ps_q[n][:, :], ones_col[:, :], sq[:, :],
                start=(k == 0), stop=(k == KT - 1), skip_group_check=True,
            )
    eps_sb = pst.tile([1, 1], F32, tag="eps", bufs=1, name="eps_sb")
    nc.vector.memset(eps_sb[:, :], EPS)
    for n in halves:
        st[n] = pst.tile([1, 2, 512], F32, tag="lnst", bufs=2, name=f"st{n}")
        nc.vector.tensor_scalar_mul(st[n][:, 0, :], ps_x[n][:, :], 1.0 / D)
        work = pst.tile([1, 512], F32, tag="lnwork", bufs=2, name="work")
        nc.vector.tensor_mul(work[:, :], st[n][:, 0, :], st[n][:, 0, :])
        nc.vector.scalar_tensor_tensor(
            st[n][:, 1, :], ps_q[n][:, :], 1.0 / D, work[:, :],
            ALU.mult, ALU.subtract,
        )
        nc.scalar.activation(st[n][:, 1, :], st[n][:, 1, :], AF.Sqrt,
                             bias=eps_sb[:, :], scale=1.0)
        nc.vector.reciprocal(st[n][:, 1, :], st[n][:, 1, :])
    return st


def _ln_apply(tc, nc, src, dst, st, ada_pp, sh_c, sc_c, pln,
              halves=(0, 1)):
    """dst[:,k,nsl] (fp8) = ((src-mean)*rstd) * ada[sc_c] + ada[sh_c]
    (ada params pre-scaled by AS)."""
    for n in halves:
        nsl = slice(n * 512, (n + 1) * 512)
        meanB = pln.tile([128, 512], F32, tag="meanB", bufs=2, name="meanB")
        rstdB = pln.tile([128, 512], F32, tag="rstdB", bufs=2, name="rstdB")
        nc.gpsimd.partition_broadcast(meanB[:, :], st[n][:, 0, :])
        nc.gpsimd.partition_broadcast(rstdB[:, :], st[n][:, 1, :])
        for k in range(KT):
            t1 = pln.tile([128, 512], F32, tag="lnt1", bufs=3, name="t1")
            nc.vector.tensor_sub(t1[:, :], src[:, k, nsl], meanB[:, :])
            nc.vector.tensor_mul(t1[:, :], t1[:, :], rstdB[:, :])
            nc.gpsimd.tensor_scalar(
                dst[:, k, nsl], t1[:, :],
                ada_pp[:, sc_c, k:k + 1], ada_pp[:, sh_c, k:k + 1],
                ALU.mult, ALU.add,
            )


def _body(tc, ins, out_dram):
    nc = tc.nc
    ctx = ExitStack()
    with ctx:
        dram = ctx.enter_context(tc.tile_pool(name="dram", bufs=1, space="DRAM"))
        ada_dr = dram.tile([6 * D], F32)
        w1f8_dr = dram.tile([18, 128, KT, 256], FP8)

        pers = ctx.enter_context(tc.tile_pool(name="pers", bufs=1))
        ident = pers.tile([128, 128], F32)
        make_identity(nc, ident[:, :])
        ones_col = pers.tile([128, 1], BF16)
        nc.vector.memset(ones_col[:, :], 1.0)
        ones_row = pers.tile([1, 128], BF16)
        nc.vector.memset(ones_row[:, :], 1.0)

        t_pp = pers.tile([128, KT], F32)
        nc.sync.dma_start(t_pp[:, :], ins["t_emb"].rearrange("(k p) -> p k", p=128))
        t_pr = pers.tile([128, KT], F32R)
        nc.scalar.activation(t_pr[:, :], t_pp[:, :], AF.Silu)

        bq_s = pers.tile([72, H], F32)
        bk_s = pers.tile([72, H], F32)
        bv_row = pers.tile([1, D], F32)
        bv_b = pers.tile([1, D], BF16)
        bproj_pp = pers.tile([128, KT], F32)
        bfc1_pp = pers.tile([128, MH], F32)
        bfc2_pp = pers.tile([128, KT], F32)
        bada_pp = pers.tile([128, 6, KT], F32)
        ada_pp = pers.tile([128, 6, KT], F32)

        def emit_bias_loads():
            nc.sync.dma_start(
                bq_s[:, :], ins["b_qkv"][0:D].rearrange("(h p) -> p h", p=72))
            nc.sync.dma_start(
                bk_s[:, :], ins["b_qkv"][D:2 * D].rearrange("(h p) -> p h", p=72))
            nc.sync.dma_start(
                bv_row[:, :],
                ins["b_qkv"][2 * D:3 * D].rearrange("(a b) -> a b", a=1))
            # bv enters the v accumulation in (AS*WS)-scaled psum units
            nc.vector.tensor_scalar_mul(bv_b[:, :], bv_row[:, :], AS * WS)
            nc.sync.dma_start(
                bproj_pp[:, :], ins["b_proj"].rearrange("(m p) -> p m", p=128))
            nc.sync.dma_start(
                bfc1_pp[:, :], ins["b_fc1"].rearrange("(m p) -> p m", p=128))
            nc.sync.dma_start(
                bfc2_pp[:, :], ins["b_fc2"].rearrange("(m p) -> p m", p=128))
            nc.sync.dma_start(
                bada_pp[:, :, :],
                ins["b_ada"].rearrange("(c k p) -> p c k", k=KT, p=128))
            # pre-scale q/k biases by QS (folded into the psum->fp8 copies)
            nc.vector.tensor_scalar_mul(bq_s[:, :], bq_s[:, :], QS)
            nc.vector.tensor_scalar_mul(bk_s[:, :], bk_s[:, :], QS)

        xT = pers.tile([128, KT, NT], F32)      # becomes x2T after residual 1
        mod12T = pers.tile([128, KTP, NT], FP8)  # mod1T, later reused as mod2T
        nc.gpsimd.memset(mod12T[:, KT, :], 0.0)  # DoubleRow pad k-tile

        # ================= phase A: x load/transpose, ada, LN1 ==============

        def emit_ada_chunk(c, p1w, ps_pro, ps_bufs=2):
            """chunk c covers w_ada cols [c*384, (c+1)*384); param p=c//3."""
            wst = p1w.tile([128, KT, 384], F32R, tag="adast", bufs=2, name="wst")
            nc.sync.dma_start(
                wst[:, :, :],
                ins["w_ada"][:, c * 384:(c + 1) * 384]
                .rearrange("(k p) m -> p k m", p=128).bitcast(F32R),
            )
            pa = ps_pro.tile([1, 384], F32, tag="psada", bufs=ps_bufs,
                             name="pa")
            for k in range(KT):
                nc.tensor.matmul(
                    pa[:, :], t_pr[:, k:k + 1], wst[:, k, :],
                    start=(k == 0), stop=(k == KT - 1),
                )
            asb = p1w.tile([1, 384], F32, tag="asb", bufs=3, name="asb")
            nc.vector.tensor_copy(asb[:, :], pa[:, :])
            nc.scalar.dma_start(
                ada_dr[c * 384:(c + 1) * 384].rearrange("(a b) -> a b", a=1),
                asb[0:1, :],
            )

        def emit_ada_pp_load(cs):
            """Load+finalize ada params cs (list) into ada_pp; params 0/1
            (shift_a/scale_a) and 3/4 are pre-scaled by AS; 1/4 get +1."""
            for c in cs:
                nc.scalar.dma_start(
                    ada_pp[:, c, :],
                    ada_dr[c * D:(c + 1) * D].rearrange("(k p) -> p k", p=128),
                )
            lo, hi = min(cs), max(cs) + 1
            nc.vector.tensor_add(ada_pp[:, lo:hi, :], ada_pp[:, lo:hi, :],
                                 bada_pp[:, lo:hi, :])
            for c in cs:
                if c in (1, 4):
                    nc.vector.tensor_scalar_add(ada_pp[:, c, :],
                                                ada_pp[:, c, :], 1.0)
                if c in (0, 1, 3, 4):
                    nc.vector.tensor_scalar_mul(ada_pp[:, c, :],
                                                ada_pp[:, c, :], AS)

        with tc.tile_pool(name="p1w", bufs=1) as p1w, \
             tc.tile_pool(name="pxin", bufs=3) as pxin, \
             tc.tile_pool(name="ps_pro", bufs=2, space="PSUM") as ps_pro, \
             tc.tile_pool(name="ps_tr", bufs=2, space="PSUM") as ps_tr:

            def emit_transpose_block(tt):
                xin = pxin.tile([128, D], F32, tag="xin", name="xin")
                nc.sync.dma_start(
                    xin[:, :], ins["x"][tt * 128:(tt + 1) * 128, :])
                for kd in range(KT):
                    pt = ps_tr.tile([128, 128], F32, tag="ptr", name="pt")
                    nc.tensor.transpose(
                        pt[:, :], xin[:, kd * 128:(kd + 1) * 128], ident[:, :])
                    tsl = slice(tt * 128, (tt + 1) * 128)
                    if kd % 2 == 0:
                        nc.vector.tensor_copy(xT[:, kd, tsl], pt[:, :])
                    else:
                        nc.scalar.copy(xT[:, kd, tsl], pt[:, :])

            for i in range(8):
                emit_transpose_block(i)
                if i == 0:
                    emit_bias_loads()
                if i < 6:
                    emit_ada_chunk(i, p1w, ps_pro)
            emit_ada_pp_load([0, 1])

        # ====== phase B part 1: qkv weight loads + converts (emitted before
        # LN1 so SP streams the loads while ada finishes / LN runs) =========
        es_qk = ExitStack()
        pqk8 = es_qk.enter_context(tc.tile_pool(name="pqk8", bufs=1))
        wq8 = pqk8.tile([128, KTP, D], FP8, name="wq8")
        wk8 = pqk8.tile([128, KTP, D], FP8, name="wk8")
        nc.gpsimd.memset(wq8[:, KT, :], 0.0)
        nc.gpsimd.memset(wk8[:, KT, :], 0.0)

        es_att = ExitStack()
        patt = es_att.enter_context(tc.tile_pool(name="patt", bufs=1, side="right"))
        attn_hs = patt.tile([72, H, NT], FP8, name="attn_hs")
        es_wp = ExitStack()
        pwp8 = es_wp.enter_context(
            tc.tile_pool(name="pwp8", bufs=1, side="right"))
        wp8 = pwp8.tile([72, H, D], FP8, name="wp8")
        es_va = ExitStack()
        pva = es_va.enter_context(tc.tile_pool(name="pva", bufs=1, side="right"))
        v_aug = pva.tile([128, NT // 128, H, 97], FP8, name="v_aug")
        nc.gpsimd.memset(v_aug[:, :, :, HD:97], 0.0)
        nc.gpsimd.memset(v_aug[:, :, :, 96:97], 1.0)

        es_b = ExitStack()
        pwst = es_b.enter_context(tc.tile_pool(name="pwst", bufs=1))
        wv8 = pwst.tile([128, KTP, D], FP8, tag="wv8", bufs=1, name="wv8")
        nc.gpsimd.memset(wv8[:, KT, :], 0.0)
        engs = ["act", "dve", "act", "dve", "act", "dve"]
        for j, (dst8, c0) in enumerate(((wq8, 0), (wk8, D), (wv8, 2 * D))):
            for half in range(2):
                msl = slice(half * 576, (half + 1) * 576)
                wst = pwst.tile([128, KT, 576], F32, tag="wst", bufs=2,
                                name="wst")
                nc.sync.dma_start(
                    wst[:, :, :],
                    ins["w_qkv"][:, c0 + half * 576:c0 + (half + 1) * 576]
                    .rearrange("(k p) m -> p k m", p=128),
                )
                eng = engs[j * 2 + half]
                for kk in range(3):
                    ksl = slice(kk * 3, kk * 3 + 3)
                    if eng == "act":
                        nc.scalar.activation(
                            dst8[:, ksl, msl], wst[:, ksl, :],
                            AF.Identity, scale=WS)
                    elif eng == "dve":
                        nc.vector.tensor_scalar_mul(
                            dst8[:, ksl, msl], wst[:, ksl, :], WS)
                    else:
                        nc.gpsimd.tensor_scalar_mul(
                            dst8[:, ksl, msl], wst[:, ksl, :], WS)

        # ====== LN1 (per-half, interleaved with v matmuls) ==================
        with tc.tile_pool(name="pst", bufs=1) as pst, \
             tc.tile_pool(name="pln", bufs=1) as pln, \
             tc.tile_pool(name="ps_st", bufs=4, space="PSUM") as ps_st, \
             tc.tile_pool(name="ps_v", bufs=3, space="PSUM") as ps_v:

            def v_block(tts):
                for tt in tts:
                    tsl = slice(tt * 128, (tt + 1) * 128)
                    for si, (c0, c1, h0, h1) in enumerate(V_SLICES):
                        pmv = ps_v.tile([128, 512], F32, tag="mv", name="pmv")
                        for i in range(KTP // 2):
                            nc.tensor.matmul(
                                pmv[:, 0:c1 - c0],
                                mod12T[:, 2 * i:2 * i + 2, tsl],
                                wv8[:, 2 * i:2 * i + 2, c0:c1],
                                start=(i == 0), stop=False, perf_mode=DR,
                                skip_group_check=True,
                            )
                        nc.tensor.matmul(
                            pmv[:, 0:c1 - c0], ones_row[:, :],
                            bv_b[:, c0:c1],
                            start=False, stop=True, skip_group_check=True,
                        )
                        vsrc = pmv[:, 0:c1 - c0].rearrange(
                            "p (h d) -> p h d", d=HD)
                        nc.vector.tensor_scalar_mul(
                            v_aug[:, tt, h0:h1, 0:HD], vsrc, 1.0 / (AS * WS))

            st1 = {}
            _ln_stats(tc, nc, xT, ones_col, pst, pln, ps_st, halves=(0,),
                      st=st1)
            _ln_apply(tc, nc, xT, mod12T, st1, ada_pp, 0, 1, pln, halves=(0,))
            _ln_stats(tc, nc, xT, ones_col, pst, pln, ps_st, halves=(1,),
                      st=st1)
            v_block(range(0, 4))
            _ln_apply(tc, nc, xT, mod12T, st1, ada_pp, 0, 1, pln, halves=(1,))
            v_block(range(4, 8))
        es_b.close()

        # ================= phase C: attention ===============================
        with tc.tile_pool(name="p3w", bufs=1) as p3w, \
             tc.tile_pool(name="pexp", bufs=1) as pexp, \
             tc.tile_pool(name="pat3", bufs=1) as pat3, \
             tc.tile_pool(name="ps_qk", bufs=2, space="PSUM") as ps_qk, \
             tc.tile_pool(name="ps_s", bufs=2, space="PSUM") as ps_s, \
             tc.tile_pool(name="ps_av", bufs=1, space="PSUM") as ps_av, \
             tc.tile_pool(name="ps_pa", bufs=1, space="PSUM") as ps_pa:

            def emit_fc1_stream(j):
                f1st = p3w.tile([128, KT, 256], F32, tag="f1st",
                                bufs=2, name="f1st")
                nc.sync.dma_start(
                    f1st[:, :, :],
                    ins["w_fc1"][:, j * 256:(j + 1) * 256]
                    .rearrange("(k p) m -> p k m", p=128),
                )
                f18o = p3w.tile([128, KT, 256], FP8, tag="f18o",
                                bufs=2, name="f18o")
                nc.gpsimd.tensor_scalar_mul(
                    f18o[:, :, :], f1st[:, :, :], WS)
                nc.scalar.dma_start(w1f8_dr[j, :, :, :], f18o[:, :, :])

            def emit_filler(h):
                # late ada chunks; fc1 fp8 stream-convert to DRAM
                if h % 4 != 3:
                    emit_ada_chunk(6 + h - h // 4, p3w, ps_pa, ps_bufs=1)
                if h == 15:
                    emit_ada_pp_load([2, 3])
                    emit_ada_pp_load([4, 5])
                if 2 <= h:
                    js = ([2 * h - 4, 2 * h - 3] if h < 6
                          else [h + 2])
                    for j in js:
                        emit_fc1_stream(j)

            for h in range(H):
                emit_filler(h)
                q_h = pat3.tile([72, NT], FP8, tag="qh", bufs=2, name="q_h")
                k_h = pat3.tile([72, NT], FP8, tag="kh", bufs=2, name="k_h")
                for n in range(2):
                    nsl = slice(n * 512, (n + 1) * 512)
                    pq = ps_qk.tile([72, 512], F32, tag="qk", name="pq")
                    for i in range(KTP // 2):
                        nc.tensor.matmul(
                            pq[:, :],
                            wq8[:, 2 * i:2 * i + 2, h * HD:(h + 1) * HD],
                            mod12T[:, 2 * i:2 * i + 2, nsl],
                            start=(i == 0), stop=(i == KTP // 2 - 1),
                            perf_mode=DR,
                        )
                    nc.vector.tensor_scalar(
                        q_h[:, nsl], pq[:, :], QS / (AS * WS),
                        bq_s[:, h:h + 1], ALU.mult, ALU.add,
                    )
                for n in range(2):
                    nsl = slice(n * 512, (n + 1) * 512)
                    pk = ps_qk.tile([72, 512], F32, tag="qk", name="pk")
                    for i in range(KTP // 2):
                        nc.tensor.matmul(
                            pk[:, :],
                            wk8[:, 2 * i:2 * i + 2, h * HD:(h + 1) * HD],
                            mod12T[:, 2 * i:2 * i + 2, nsl],
                            start=(i == 0), stop=(i == KTP // 2 - 1),
                            perf_mode=DR,
                        )
                    nc.vector.tensor_scalar(
                        k_h[:, nsl], pk[:, :], QS / (AS * WS),
                        bk_s[:, h:h + 1], ALU.mult, ALU.add,
                    )
                for n in range(2):
                    nsl = slice(n * 512, (n + 1) * 512)
                    exp_hn = pexp.tile([128, NT // 128, 512], FP8, tag="exp",
                                       bufs=3, name="exp_hn")
                    for kp in range(NT // 256):
                        pss = ps_s.tile([128, 2, 512], F32, tag="s",
                                        name="pss")
                        for j in range(2):
                            kt_i = 2 * kp + j
                            nc.tensor.matmul(
                                pss[:, j, :],
                                k_h[:, kt_i * 128:(kt_i + 1) * 128],
                                q_h[:, nsl], start=True, stop=True,
                            )
                        nc.scalar.activation(
                            exp_hn[:, 2 * kp:2 * kp + 2, :],
                            pss[:, :, :], AF.Exp, scale=ES)
                    pav = ps_av.tile([97, 512], F32, tag="av", name="pav")
                    for i in range(NT // 256):
                        nc.tensor.matmul(
                            pav[:, :],
                            v_aug[:, 2 * i:2 * i + 2, h, :],
                            exp_hn[:, 2 * i:2 * i + 2, :],
                            start=(i == 0), stop=(i == NT // 256 - 1),
                            perf_mode=DR,
                        )
                    den = pat3.tile([1, 512], F32, tag="den", bufs=2,
                                    name="den")
                    nc.vector.tensor_scalar_mul(den[:, :], pav[96:97, :],
                                                1.0 / PS)
                    nc.vector.reciprocal(den[:, :], den[:, :])
                    denB = pat3.tile([72, 512], F32, tag="denB", bufs=2,
                                     name="denB")
                    nc.gpsimd.partition_broadcast(denB[:, :], den[:, :])
                    nc.vector.tensor_mul(
                        attn_hs[:, h, nsl], pav[0:HD, :], denB[:, :])
        es_qk.close()  # wq8/wk8 no longer needed
        es_va.close()

        # ================= phase D: proj + residual + LN2 ===================
        es_w2 = ExitStack()
        pw2 = es_w2.enter_context(
            tc.tile_pool(name="pw2", bufs=1, side="right"))
        w2f8 = pw2.tile([128, KT, MH, 128], FP8, name="w2f8")

        with tc.tile_pool(name="p4", bufs=1) as p4, \
             tc.tile_pool(name="pst4", bufs=1) as pst4, \
             tc.tile_pool(name="pln4", bufs=1) as pln4:
            for i in range(6):
                msl = slice(i * 192, (i + 1) * 192)
                wpst = p4.tile([72, H, 192], F32, tag="wpst", bufs=2,
                               name="wpst")
                nc.sync.dma_start(
                    wpst[:, :, :],
                    ins["w_proj"][:, msl].rearrange("(h p) m -> p h m", p=72),
                )
                for kk in range(2):
                    hsl = slice(kk * 8, kk * 8 + 8)
                    nc.vector.tensor_scalar_mul(
                        wp8[:, hsl, msl], wpst[:, hsl, :], WS)

            def emit_fc2_chunk(ch, eng, pool):
                f2s = pool.tile([128, MH, 64], F32, tag="f2s", bufs=2,
                                name="f2s")
                nc.sync.dma_start(
                    f2s[:, :, :],
                    ins["w_fc2"][:, ch * 64:(ch + 1) * 64]
                    .rearrange("(k p) m -> p k m", p=128),
                )
                eng.tensor_scalar_mul(
                    w2f8[:, ch // 2, :, (ch % 2) * 64:(ch % 2 + 1) * 64],
                    f2s[:, :, :], WS)

            st2 = {}
            with tc.tile_pool(name="ps_mm2", bufs=3, space="PSUM") as ps_mm2, \
                 tc.tile_pool(name="ps_st2", bufs=4, space="PSUM") as ps_st2:
                for n in range(2):
                    nsl = slice(n * 512, (n + 1) * 512)
                    for mo in range(KT):
                        if mo < 6:
                            ch = n * 6 + mo
                            eng = nc.vector if ch % 2 else nc.gpsimd
                            emit_fc2_chunk(ch, eng, p4)
                        pm2 = ps_mm2.tile([128, 512], F32, tag="mm2",
                                          name="pm2")
                        for i in range(H // 2):
                            nc.tensor.matmul(
                                pm2[:, :],
                                wp8[:, 2 * i:2 * i + 2,
                                    mo * 128:(mo + 1) * 128],
                                attn_hs[:, 2 * i:2 * i + 2, nsl],
                                start=(i == 0), stop=(i == H // 2 - 1),
                                perf_mode=DR,
                            )
                        t_sb = p4.tile([128, 512], F32, tag="tsb", bufs=2,
                                       name="t_sb")
                        nc.scalar.activation(
                            t_sb[:, :], pm2[:, :], AF.Identity,
                            bias=bproj_pp[:, mo:mo + 1], scale=1.0 / (PS * WS),
                        )
                        nc.vector.scalar_tensor_tensor(
                            xT[:, mo, nsl], t_sb[:, :],
                            ada_pp[:, 2, mo:mo + 1], xT[:, mo, nsl],
                            ALU.mult, ALU.add,
                        )
                    _ln_stats(tc, nc, xT, ones_col, pst4, pln4, ps_st2,
                              halves=(n,), st=st2)
                    _ln_apply(tc, nc, xT, mod12T, st2, ada_pp, 3, 4, pln4,
                              halves=(n,))

        # ================= phase E: FFN =====================================
        es_e = ExitStack()
        ph = es_e.enter_context(tc.tile_pool(name="ph", bufs=1))
        hT = ph.tile([128, MH, NT], FP8, name="hT")
        po = es_e.enter_context(tc.tile_pool(name="po", bufs=1))

        with tc.tile_pool(name="ps_f1", bufs=3, space="PSUM") as ps_f1, \
             tc.tile_pool(name="ps_f2", bufs=2, space="PSUM") as ps_f2, \
             tc.tile_pool(name="ps_tro", bufs=2, space="PSUM") as ps_tro:
            # fc1 in 18 chunks of 256 columns (2 m-tiles each), weights
            # already converted to fp8 in DRAM during the attention window
            with tc.tile_pool(name="p5a", bufs=1) as p5a:
                for ch in range(18):
                    f18 = p5a.tile([128, KTP, 256], FP8, tag="f18", bufs=3,
                                   name="f18")
                    nc.sync.dma_start(f18[:, 0:KT, :], w1f8_dr[ch, :, :, :])
                    nc.gpsimd.memset(f18[:, KT, :], 0.0)
                    if ch < 6:
                        emit_fc2_chunk(12 + ch,
                                       nc.vector if ch % 2 else nc.gpsimd,
                                       p5a)
                    for m in range(2):
                        mo = ch * 2 + m
                        for n in range(2):
                            nsl = slice(n * 512, (n + 1) * 512)
                            pf1 = ps_f1.tile([128, 512], F32, tag="f1",
                                             name="pf1")
                            for i in range(KTP // 2):
                                nc.tensor.matmul(
                                    pf1[:, :],
                                    f18[:, 2 * i:2 * i + 2,
                                        m * 128:(m + 1) * 128],
                                    mod12T[:, 2 * i:2 * i + 2, nsl],
                                    start=(i == 0), stop=(i == KTP // 2 - 1),
                                    perf_mode=DR,
                                )
                            nc.scalar.activation(
                                hT[:, mo, nsl], pf1[:, :], AF.Gelu_apprx_tanh,
                                bias=bfc1_pp[:, mo:mo + 1],
                                scale=1.0 / (AS * WS),
                            )
            # fc2: weights already fp8-resident in SBUF (w2f8)
            with tc.tile_pool(name="p5b", bufs=1) as p5b:
                for mo in range(KT):
                    for n in range(2):
                        nsl = slice(n * 512, (n + 1) * 512)
                        pf2 = ps_f2.tile([128, 512], F32, tag="f2", name="pf2")
                        for i in range(MH // 2):
                            nc.tensor.matmul(
                                pf2[:, :], w2f8[:, mo, 2 * i:2 * i + 2, :],
                                hT[:, 2 * i:2 * i + 2, nsl],
                                start=(i == 0), stop=(i == MH // 2 - 1),
                                perf_mode=DR,
                            )
                        t2 = p5b.tile([128, 512], F32, tag="t2", bufs=3,
                                      name="t2")
                        nc.scalar.activation(
                            t2[:, :], pf2[:, :], AF.Identity,
                            bias=bfc2_pp[:, mo:mo + 1], scale=1.0 / WS,
                        )
                        nc.vector.scalar_tensor_tensor(
                            xT[:, mo, nsl], t2[:, :], ada_pp[:, 5, mo:mo + 1],
                            xT[:, mo, nsl], ALU.mult, ALU.add,
                        )
                    o_slab = po.tile([128, NT // 128, 128], F32, tag="osl",
                                     bufs=2, name="o_slab")
                    for tt in range(NT // 128):
                        pt = ps_tro.tile([128, 128], F32, tag="tro",
                                         name="pt6")
                        nc.tensor.transpose(
                            pt[:, :], xT[:, mo, tt * 128:(tt + 1) * 128],
                            ident[:, :],
                        )
                        dst = o_slab[:, tt, :]
                        if tt % 2 == 0:
                            nc.vector.tensor_copy(dst, pt[:, :])
                        else:
                            nc.scalar.copy(dst, pt[:, :])
                    nc.scalar.dma_start(
                        out_dram[:, mo * 128:(mo + 1) * 128]
                        .rearrange("(t p) m -> p t m", p=128),
                        o_slab[:, :, :])
        es_w2.close()
        es_wp.close()
        es_att.close()
        es_e.close()


_LOCK = threading.Lock()
_PROG = None


def _get_program():
    global _PROG
    with _LOCK:
        if _PROG is None:
            _PROG = _build_program()
    return _PROG


def _make_in_maps(inputs):
    arrs = {k: np.ascontiguousarray(np.asarray(v, dtype=np.float32))
            for k, v in inputs.items()}
    in_maps = []
    for c in range(NCORES):
        m = {k: v for k, v in arrs.items() if k not in ("x", "t_emb")}
        m["x"] = np.ascontiguousarray(arrs["x"][c])
        m["t_emb"] = np.ascontiguousarray(arrs["t_emb"][c])
        in_maps.append(m)
    return in_maps


def kernel(**inputs):
    nc = _get_program()
    res = run_bass_kernel_spmd(nc, _make_in_maps(inputs),
                               core_ids=list(range(NCORES)))
    return np.stack([r["out"] for r in res.results], axis=0)


def kernel_traced(inputs, **kw):
    """test-harness helper: returns full BassKernelResults with trace."""
    nc = _get_program()
    return run_bass_kernel_spmd(
        nc, _make_in_maps(inputs), core_ids=list(range(NCORES)), trace=True,
        **kw
    )


# revision 65
# speedup vs baseline: 1.5253x; 1.0008x over previous
"""DiT block kernel for Trainium2 (Bass/Tile), 8-core data parallel.

Shapes (hardcoded from the problem spec):
  x: (8, 1024, 1152), t_emb: (8, 1152)
  w_qkv (1152, 3456), w_proj (1152, 1152), w_fc1 (1152, 4608),
  w_fc2 (4608, 1152), w_ada (1152, 6912) + biases.

Strategy: batch-parallel across 8 cores (one batch element each, no
collectives). Activations live transposed [D on partitions, tokens free].
The large matmuls (qkv, attention AV, proj, fc1, fc2) run in fp8e4 with
DoubleRow perf mode (two 128-row k-tiles contracted per instruction);
scale factors for fp8 range are folded into the existing activation
bias/scale stages so no extra elementwise work is added.  LayerNorm
statistics reduce over the partition axis via ones-vector f32r matmuls;
softmax runs transposed (keys on partitions) with denominators collected
through a ones-column appended to V and a fused divide.  q/k are produced
per-head directly (M=72 matmuls cost the same per column as M=128), so
attention needs no partition-crossing gather DMAs.  Weights stream
through big staged f32 DMA loads (few, large transfers) and are
converted on-chip; ada (error-sensitive) stays f32r.
"""

import threading
from contextlib import ExitStack

import numpy as np

import concourse.bass as bass
import concourse.mybir as mybir
import concourse.tile as tile
from concourse import bacc
from concourse.bass_utils import run_bass_kernel_spmd
from concourse.masks import make_identity

F32 = mybir.dt.float32
F32R = mybir.dt.float32r
BF16 = mybir.dt.bfloat16
FP8 = mybir.dt.float8e4
AF = mybir.ActivationFunctionType
ALU = mybir.AluOpType
DR = mybir.MatmulPerfMode.DoubleRow

NCORES = 8
D = 1152
NT = 1024
KT = D // 128       # 9
KTP = KT + 1        # padded to even for DoubleRow pairs
H = 16
HD = 72
HID = 4 * D
MH = HID // 128     # 36
EPS = 1e-6
ISC = 1.0 / float(np.sqrt(HD))

# fp8 scale factors
WS = 64.0           # weights
AS = 8.0            # modulated activations (mod1/mod2)
QS = 2.0            # q/k
PS = 4.0            # attention output
ES = ISC / (QS * QS)  # exp() input scale applied to the scores psum

# v output column slices aligned to head boundaries
V_SLICES = [(0, 432, 0, 6), (432, 864, 6, 12), (864, 1152, 12, 16)]


def _r(ap):
    return ap.bitcast(F32R)


def _build_program():
    nc = bacc.Bacc(
        "TRN2", target_bir_lowering=False, debug=False, enable_asserts=False
    )
    ins = {}
    ins["x"] = nc.dram_tensor("x", [NT, D], F32, kind="ExternalInput").ap()
    ins["t_emb"] = nc.dram_tensor("t_emb", [D], F32, kind="ExternalInput").ap()
    for name, shape in [
        ("w_qkv", [D, 3 * D]), ("b_qkv", [3 * D]),
        ("w_proj", [D, D]), ("b_proj", [D]),
        ("w_fc1", [D, HID]), ("b_fc1", [HID]),
        ("w_fc2", [HID, D]), ("b_fc2", [D]),
        ("w_ada", [D, 6 * D]), ("b_ada", [6 * D]),
    ]:
        ins[name] = nc.dram_tensor(name, shape, F32, kind="ExternalInput").ap()
    out_dram = nc.dram_tensor("out", [NT, D], F32, kind="ExternalOutput").ap()

    with tile.TileContext(nc) as tc:
        _body(tc, ins, out_dram)
    nc.compile()
    return nc


def _ln_stats(tc, nc, src, ones_col, pst, pln, ps_st, halves=(0, 1),
              st=None):
    """Return st[n] = [mean; rstd] rows [1, 2, 512] per 512-token half,
    reducing over the partition (D) axis of src [128, KT, NT] f32."""
    ps_x, ps_q = {}, {}
    if st is None:
        st = {}
    for n in halves:
        nsl = slice(n * 512, (n + 1) * 512)
        ps_x[n] = ps_st.tile([1, 512], F32, tag="st", name=f"psx{n}")
        ps_q[n] = ps_st.tile([1, 512], F32, tag="st", name=f"psq{n}")
        for k in range(KT):
            xb = pln.tile([128, 512], BF16, tag="xb", bufs=2, name="xb")
            nc.scalar.copy(xb[:, :], src[:, k, nsl])
            sq = pln.tile([128, 512], BF16, tag="sq", bufs=2, name="sq")
            nc.vector.tensor_mul(sq[:, :], src[:, k, nsl], src[:, k, nsl])
            nc.tensor.matmul(
                ps_x[n][:, :], ones_col[:, :], xb[:, :],
                start=(k == 0), stop=(k == KT - 1), skip_group_check=True,
            )
            nc.tensor.matmul(
                ps_q[n][:, :], ones_col[:, :], sq[:, :],
                start=(k == 0), stop=(k == KT - 1), skip_group_check=True,
            )
    eps_sb = pst.tile([1, 1], F32, tag="eps", bufs=1, name="eps_sb")
    nc.vector.memset(eps_sb[:, :], EPS)
    for n in halves:
        st[n] = pst.tile([1, 2, 512], F32, tag="lnst", bufs=2, name=f"st{n}")
        nc.vector.tensor_scalar_mul(st[n][:, 0, :], ps_x[n][:, :], 1.0 / D)
        work = pst.tile([1, 512], F32, tag="lnwork", bufs=2, name="work")
        nc.vector.tensor_mul(work[:, :], st[n][:, 0, :], st[n][:, 0, :])
        nc.vector.scalar_tensor_tensor(
            st[n][:, 1, :], ps_q[n][:, :], 1.0 / D, work[:, :],
            ALU.mult, ALU.subtract,
        )
        nc.scalar.activation(st[n][:, 1, :], st[n][:, 1, :], AF.Sqrt,
                             bias=eps_sb[:, :], scale=1.0)
        nc.vector.reciprocal(st[n][:, 1, :], st[n][:, 1, :])
    return st


def _ln_apply(tc, nc, src, dst, st, ada_pp, sh_c, sc_c, pln,
              halves=(0, 1)):
    """dst[:,k,nsl] (fp8) = ((src-mean)*rstd) * ada[sc_c] + ada[sh_c]
    (ada params pre-scaled by AS)."""
    for n in halves:
        nsl = slice(n * 512, (n + 1) * 512)
        meanB = pln.tile([128, 512], F32, tag="meanB", bufs=2, name="meanB")
        rstdB = pln.tile([128, 512], F32, tag="rstdB", bufs=2, name="rstdB")
        nc.gpsimd.partition_broadcast(meanB[:, :], st[n][:, 0, :])
        nc.gpsimd.partition_broadcast(rstdB[:, :], st[n][:, 1, :])
        for k in range(KT):
            t1 = pln.tile([128, 512], F32, tag="lnt1", bufs=3, name="t1")
            nc.vector.tensor_sub(t1[:, :], src[:, k, nsl], meanB[:, :])
            nc.vector.tensor_mul(t1[:, :], t1[:, :], rstdB[:, :])
            nc.gpsimd.tensor_scalar(
                dst[:, k, nsl], t1[:, :],
                ada_pp[:, sc_c, k:k + 1], ada_pp[:, sh_c, k:k + 1],
                ALU.mult, ALU.add,
            )


def _body(tc, ins, out_dram):
    nc = tc.nc
    ctx = ExitStack()
    with ctx:
        dram = ctx.enter_context(tc.tile_pool(name="dram", bufs=1, space="DRAM"))
        ada_dr = dram.tile([6 * D], F32)
        w1f8_dr = dram.tile([18, 128, KT, 256], FP8)

        pers = ctx.enter_context(tc.tile_pool(name="pers", bufs=1))
        ident = pers.tile([128, 128], F32)
        make_identity(nc, ident[:, :])
        ones_col = pers.tile([128, 1], BF16)
        nc.vector.memset(ones_col[:, :], 1.0)
        ones_row = pers.tile([1, 128], BF16)
        nc.vector.memset(ones_row[:, :], 1.0)

        t_pp = pers.tile([128, KT], F32)
        nc.sync.dma_start(t_pp[:, :], ins["t_emb"].rearrange("(k p) -> p k", p=128))
        t_pr = pers.tile([128, KT], F32R)
        nc.scalar.activation(t_pr[:, :], t_pp[:, :], AF.Silu)

        bq_s = pers.tile([72, H], F32)
        bk_s = pers.tile([72, H], F32)
        bv_row = pers.tile([1, D], F32)
        bv_b = pers.tile([1, D], BF16)
        bproj_pp = pers.tile([128, KT], F32)
        bfc1_pp = pers.tile([128, MH], F32)
        bfc2_pp = pers.tile([128, KT], F32)
        bada_pp = pers.tile([128, 6, KT], F32)
        ada_pp = pers.tile([128, 6, KT], F32)

        def emit_bias_loads():
            nc.sync.dma_start(
                bq_s[:, :], ins["b_qkv"][0:D].rearrange("(h p) -> p h", p=72))
            nc.sync.dma_start(
                bk_s[:, :], ins["b_qkv"][D:2 * D].rearrange("(h p) -> p h", p=72))
            nc.sync.dma_start(
                bv_row[:, :],
                ins["b_qkv"][2 * D:3 * D].rearrange("(a b) -> a b", a=1))
            # bv enters the v accumulation in (AS*WS)-scaled psum units
            nc.vector.tensor_scalar_mul(bv_b[:, :], bv_row[:, :], AS * WS)
            nc.sync.dma_start(
                bproj_pp[:, :], ins["b_proj"].rearrange("(m p) -> p m", p=128))
            nc.sync.dma_start(
                bfc1_pp[:, :], ins["b_fc1"].rearrange("(m p) -> p m", p=128))
            nc.sync.dma_start(
                bfc2_pp[:, :], ins["b_fc2"].rearrange("(m p) -> p m", p=128))
            nc.sync.dma_start(
                bada_pp[:, :, :],
                ins["b_ada"].rearrange("(c k p) -> p c k", k=KT, p=128))
            # pre-scale q/k biases by QS (folded into the psum->fp8 copies)
            nc.vector.tensor_scalar_mul(bq_s[:, :], bq_s[:, :], QS)
            nc.vector.tensor_scalar_mul(bk_s[:, :], bk_s[:, :], QS)

        xT = pers.tile([128, KT, NT], F32)      # becomes x2T after residual 1
        mod12T = pers.tile([128, KTP, NT], FP8)  # mod1T, later reused as mod2T
        nc.gpsimd.memset(mod12T[:, KT, :], 0.0)  # DoubleRow pad k-tile

        # ================= phase A: x load/transpose, ada, LN1 ==============

        def emit_ada_chunk(c, p1w, ps_pro, ps_bufs=2):
            """chunk c covers w_ada cols [c*384, (c+1)*384); param p=c//3."""
            wst = p1w.tile([128, KT, 384], F32R, tag="adast", bufs=2, name="wst")
            nc.sync.dma_start(
                wst[:, :, :],
                ins["w_ada"][:, c * 384:(c + 1) * 384]
                .rearrange("(k p) m -> p k m", p=128).bitcast(F32R),
            )
            pa = ps_pro.tile([1, 384], F32, tag="psada", bufs=ps_bufs,
                             name="pa")
            for k in range(KT):
                nc.tensor.matmul(
                    pa[:, :], t_pr[:, k:k + 1], wst[:, k, :],
                    start=(k == 0), stop=(k == KT - 1),
                )
            asb = p1w.tile([1, 384], F32, tag="asb", bufs=3, name="asb")
            nc.vector.tensor_copy(asb[:, :], pa[:, :])
            nc.scalar.dma_start(
                ada_dr[c * 384:(c + 1) * 384].rearrange("(a b) -> a b", a=1),
                asb[0:1, :],
            )

        def emit_ada_pp_load(cs):
            """Load+finalize ada params cs (list) into ada_pp; params 0/1
            (shift_a/scale_a) and 3/4 are pre-scaled by AS; 1/4 get +1."""
            for c in cs:
                nc.scalar.dma_start(
                    ada_pp[:, c, :],
                    ada_dr[c * D:(c + 1) * D].rearrange("(k p) -> p k", p=128),
                )
            lo, hi = min(cs), max(cs) + 1
            nc.vector.tensor_add(ada_pp[:, lo:hi, :], ada_pp[:, lo:hi, :],
                                 bada_pp[:, lo:hi, :])
            for c in cs:
                if c in (1, 4):
                    nc.vector.tensor_scalar_add(ada_pp[:, c, :],
                                                ada_pp[:, c, :], 1.0)
                if c in (0, 1, 3, 4):
                    nc.vector.tensor_scalar_mul(ada_pp[:, c, :],
                                                ada_pp[:, c, :], AS)

        with tc.tile_pool(name="p1w", bufs=1) as p1w, \
             tc.tile_pool(name="pxin", bufs=3) as pxin, \
             tc.tile_pool(name="ps_pro", bufs=2, space="PSUM") as ps_pro, \
             tc.tile_pool(name="ps_tr", bufs=2, space="PSUM") as ps_tr:

            def emit_transpose_block(tt):
                xin = pxin.tile([128, D], F32, tag="xin", name="xin")
                nc.sync.dma_start(
                    xin[:, :], ins["x"][tt * 128:(tt + 1) * 128, :])
                for kd in range(KT):
                    pt = ps_tr.tile([128, 128], F32, tag="ptr", name="pt")
                    nc.tensor.transpose(
                        pt[:, :], xin[:, kd * 128:(kd + 1) * 128], ident[:, :])
                    tsl = slice(tt * 128, (tt + 1) * 128)
                    if kd % 2 == 0:
                        nc.vector.tensor_copy(xT[:, kd, tsl], pt[:, :])
                    else:
                        nc.scalar.copy(xT[:, kd, tsl], pt[:, :])

            for i in range(8):
                emit_transpose_block(i)
                if i == 0:
                    emit_bias_loads()
                if i < 6:
                    emit_ada_chunk(i, p1w, ps_pro)
            emit_ada_pp_load([0, 1])

        # ====== phase B part 1: qkv weight loads + converts (emitted before
        # LN1 so SP streams the loads while ada finishes / LN runs) =========
        es_qk = ExitStack()
        pqk8 = es_qk.enter_context(tc.tile_pool(name="pqk8", bufs=1))
        wq8 = pqk8.tile([128, KTP, D], FP8, name="wq8")
        wk8 = pqk8.tile([128, KTP, D], FP8, name="wk8")
        nc.gpsimd.memset(wq8[:, KT, :], 0.0)
        nc.gpsimd.memset(wk8[:, KT, :], 0.0)

        es_att = ExitStack()
        patt = es_att.enter_context(tc.tile_pool(name="patt", bufs=1, side="right"))
        attn_hs = patt.tile([72, H, NT], FP8, name="attn_hs")
        es_wp = ExitStack()
        pwp8 = es_wp.enter_context(
            tc.tile_pool(name="pwp8", bufs=1, side="right"))
        wp8 = pwp8.tile([72, H, D], FP8, name="wp8")
        es_va = ExitStack()
        pva = es_va.enter_context(tc.tile_pool(name="pva", bufs=1, side="right"))
        v_aug = pva.tile([128, NT // 128, H, 97], FP8, name="v_aug")
        nc.gpsimd.memset(v_aug[:, :, :, HD:97], 0.0)
        nc.gpsimd.memset(v_aug[:, :, :, 96:97], 1.0)

        es_b = ExitStack()
        pwst = es_b.enter_context(tc.tile_pool(name="pwst", bufs=1))
        wv8 = pwst.tile([128, KTP, D], FP8, tag="wv8", bufs=1, name="wv8")
        nc.gpsimd.memset(wv8[:, KT, :], 0.0)
        engs = ["act", "dve", "act", "dve", "act", "dve"]
        for j, (dst8, c0) in enumerate(((wq8, 0), (wk8, D), (wv8, 2 * D))):
            for half in range(2):
                msl = slice(half * 576, (half + 1) * 576)
                wst = pwst.tile([128, KT, 576], F32, tag="wst", bufs=2,
                                name="wst")
                nc.sync.dma_start(
                    wst[:, :, :],
                    ins["w_qkv"][:, c0 + half * 576:c0 + (half + 1) * 576]
                    .rearrange("(k p) m -> p k m", p=128),
                )
                eng = engs[j * 2 + half]
                for kk in range(3):
                    ksl = slice(kk * 3, kk * 3 + 3)
                    if eng == "act":
                        nc.scalar.activation(
                            dst8[:, ksl, msl], wst[:, ksl, :],
                            AF.Identity, scale=WS)
                    elif eng == "dve":
                        nc.vector.tensor_scalar_mul(
                            dst8[:, ksl, msl], wst[:, ksl, :], WS)
                    else:
                        nc.gpsimd.tensor_scalar_mul(
                            dst8[:, ksl, msl], wst[:, ksl, :], WS)

        # ====== LN1 (per-half, interleaved with v matmuls) ==================
        with tc.tile_pool(name="pst", bufs=1) as pst, \
             tc.tile_pool(name="pln", bufs=1) as pln, \
             tc.tile_pool(name="ps_st", bufs=4, space="PSUM") as ps_st, \
             tc.tile_pool(name="ps_v", bufs=3, space="PSUM") as ps_v:

            def v_block(tts):
                for tt in tts:
                    tsl = slice(tt * 128, (tt + 1) * 128)
                    for si, (c0, c1, h0, h1) in enumerate(V_SLICES):
                        pmv = ps_v.tile([128, 512], F32, tag="mv", name="pmv")
                        for i in range(KTP // 2):
                            nc.tensor.matmul(
                                pmv[:, 0:c1 - c0],
                                mod12T[:, 2 * i:2 * i + 2, tsl],
                                wv8[:, 2 * i:2 * i + 2, c0:c1],
                                start=(i == 0), stop=False, perf_mode=DR,
                                skip_group_check=True,
                            )
                        nc.tensor.matmul(
                            pmv[:, 0:c1 - c0], ones_row[:, :],
                            bv_b[:, c0:c1],
                            start=False, stop=True, skip_group_check=True,
                        )
                        vsrc = pmv[:, 0:c1 - c0].rearrange(
                            "p (h d) -> p h d", d=HD)
                        nc.vector.tensor_scalar_mul(
                            v_aug[:, tt, h0:h1, 0:HD], vsrc, 1.0 / (AS * WS))

            st1 = {}
            _ln_stats(tc, nc, xT, ones_col, pst, pln, ps_st, halves=(0,),
                      st=st1)
            _ln_apply(tc, nc, xT, mod12T, st1, ada_pp, 0, 1, pln, halves=(0,))
            _ln_stats(tc, nc, xT, ones_col, pst, pln, ps_st, halves=(1,),
                      st=st1)
            v_block(range(0, 4))
            _ln_apply(tc, nc, xT, mod12T, st1, ada_pp, 0, 1, pln, halves=(1,))
            v_block(range(4, 8))
        es_b.close()

        # ================= phase C: attention ===============================
        with tc.tile_pool(name="p3w", bufs=1) as p3w, \
             tc.tile_pool(name="pexp", bufs=1) as pexp, \
             tc.tile_pool(name="pat3", bufs=1) as pat3, \
             tc.tile_pool(name="ps_qk", bufs=2, space="PSUM") as ps_qk, \
             tc.tile_pool(name="ps_s", bufs=2, space="PSUM") as ps_s, \
             tc.tile_pool(name="ps_av", bufs=1, space="PSUM") as ps_av, \
             tc.tile_pool(name="ps_pa", bufs=1, space="PSUM") as ps_pa:

            def emit_fc1_stream(j):
                f1st = p3w.tile([128, KT, 256], F32, tag="f1st",
                                bufs=2, name="f1st")
                nc.sync.dma_start(
                    f1st[:, :, :],
                    ins["w_fc1"][:, j * 256:(j + 1) * 256]
                    .rearrange("(k p) m -> p k m", p=128),
                )
                f18o = p3w.tile([128, KT, 256], FP8, tag="f18o",
                                bufs=2, name="f18o")
                nc.gpsimd.tensor_scalar_mul(
                    f18o[:, :, :], f1st[:, :, :], WS)
                nc.scalar.dma_start(w1f8_dr[j, :, :, :], f18o[:, :, :])

            def emit_filler(h):
                # late ada chunks; fc1 fp8 stream-convert to DRAM
                if h % 4 != 3:
                    emit_ada_chunk(6 + h - h // 4, p3w, ps_pa, ps_bufs=1)
                if h == 15:
                    emit_ada_pp_load([2, 3])
                    emit_ada_pp_load([4, 5])
                if 2 <= h:
                    js = ([2 * h - 4, 2 * h - 3] if h < 6
                          else [h + 2])
                    for j in js:
                        emit_fc1_stream(j)

            for h in range(H):
                emit_filler(h)
                q_h = pat3.tile([72, NT], FP8, tag="qh", bufs=2, name="q_h")
                k_h = pat3.tile([72, NT], FP8, tag="kh", bufs=2, name="k_h")
                for n in range(2):
                    nsl = slice(n * 512, (n + 1) * 512)
                    pq = ps_qk.tile([72, 512], F32, tag="qk", name="pq")
                    for i in range(KTP // 2):
                        nc.tensor.matmul(
                            pq[:, :],
                            wq8[:, 2 * i:2 * i + 2, h * HD:(h + 1) * HD],
                            mod12T[:, 2 * i:2 * i + 2, nsl],
                            start=(i == 0), stop=(i == KTP // 2 - 1),
                            perf_mode=DR,
                        )
                    nc.vector.tensor_scalar(
                        q_h[:, nsl], pq[:, :], QS / (AS * WS),
                        bq_s[:, h:h + 1], ALU.mult, ALU.add,
                    )
                for n in range(2):
                    nsl = slice(n * 512, (n + 1) * 512)
                    pk = ps_qk.tile([72, 512], F32, tag="qk", name="pk")
                    for i in range(KTP // 2):
                        nc.tensor.matmul(
                            pk[:, :],
                            wk8[:, 2 * i:2 * i + 2, h * HD:(h + 1) * HD],
                            mod12T[:, 2 * i:2 * i + 2, nsl],
                            start=(i == 0), stop=(i == KTP // 2 - 1),
                            perf_mode=DR,
                        )
                    nc.vector.tensor_scalar(
                        k_h[:, nsl], pk[:, :], QS / (AS * WS),
                        bk_s[:, h:h + 1], ALU.mult, ALU.add,
                    )
                for n in range(2):
                    nsl = slice(n * 512, (n + 1) * 512)
                    exp_hn = pexp.tile([128, NT // 128, 512], FP8, tag="exp",
                                       bufs=3, name="exp_hn")
                    for kp in range(NT // 256):
                        pss = ps_s.tile([128, 2, 512], F32, tag="s",
                                        name="pss")
                        for j in range(2):
                            kt_i = 2 * kp + j
                            nc.tensor.matmul(
                                pss[:, j, :],
                                k_h[:, kt_i * 128:(kt_i + 1) * 128],
                                q_h[:, nsl], start=True, stop=True,
                            )
                        nc.scalar.activation(
                            exp_hn[:, 2 * kp:2 * kp + 2, :],
                            pss[:, :, :], AF.Exp, scale=ES)
                    pav = ps_av.tile([97, 512], F32, tag="av", name="pav")
                    for i in range(NT // 256):
                        nc.tensor.matmul(
                            pav[:, :],
                            v_aug[:, 2 * i:2 * i + 2, h, :],
                            exp_hn[:, 2 * i:2 * i + 2, :],
                            start=(i == 0), stop=(i == NT // 256 - 1),
                            perf_mode=DR,
                        )
                    den = pat3.tile([1, 512], F32, tag="den", bufs=2,
                                    name="den")
                    nc.vector.tensor_scalar_mul(den[:, :], pav[96:97, :],
                                                1.0 / PS)
                    nc.vector.reciprocal(den[:, :], den[:, :])
                    denB = pat3.tile([72, 512], F32, tag="denB", bufs=2,
                                     name="denB")
                    nc.gpsimd.partition_broadcast(denB[:, :], den[:, :])
                    nc.vector.tensor_mul(
                        attn_hs[:, h, nsl], pav[0:HD, :], denB[:, :])
        es_qk.close()  # wq8/wk8 no longer needed
        es_va.close()

        # ================= phase D: proj + residual + LN2 ===================
        es_w2 = ExitStack()
        pw2 = es_w2.enter_context(
            tc.tile_pool(name="pw2", bufs=1, side="right"))
        w2f8 = pw2.tile([128, KT, MH, 128], FP8, name="w2f8")

        with tc.tile_pool(name="p4", bufs=1) as p4, \
             tc.tile_pool(name="pst4", bufs=1) as pst4, \
             tc.tile_pool(name="pln4", bufs=1) as pln4:
            for i in range(6):
                msl = slice(i * 192, (i + 1) * 192)
                wpst = p4.tile([72, H, 192], F32, tag="wpst", bufs=2,
                               name="wpst")
                nc.sync.dma_start(
                    wpst[:, :, :],
                    ins["w_proj"][:, msl].rearrange("(h p) m -> p h m", p=72),
                )
                for kk in range(2):
                    hsl = slice(kk * 8, kk * 8 + 8)
                    nc.vector.tensor_scalar_mul(
                        wp8[:, hsl, msl], wpst[:, hsl, :], WS)

            def emit_fc2_chunk(ch, eng, pool):
                f2s = pool.tile([128, MH, 64], F32, tag="f2s", bufs=2,
                                name="f2s")
                nc.sync.dma_start(
                    f2s[:, :, :],
                    ins["w_fc2"][:, ch * 64:(ch + 1) * 64]
                    .rearrange("(k p) m -> p k m", p=128),
                )
                eng.tensor_scalar_mul(
                    w2f8[:, ch // 2, :, (ch % 2) * 64:(ch % 2 + 1) * 64],
                    f2s[:, :, :], WS)

            st2 = {}
            with tc.tile_pool(name="ps_mm2", bufs=3, space="PSUM") as ps_mm2, \
                 tc.tile_pool(name="ps_st2", bufs=4, space="PSUM") as ps_st2:
                for n in range(2):
                    nsl = slice(n * 512, (n + 1) * 512)
                    for mo in range(KT):
                        if mo < 6:
                            ch = n * 6 + mo
                            eng = nc.vector if ch % 2 else nc.gpsimd
                            emit_fc2_chunk(ch, eng, p4)
                        pm2 = ps_mm2.tile([128, 512], F32, tag="mm2",
                                          name="pm2")
                        for i in range(H // 2):
                            nc.tensor.matmul(
                                pm2[:, :],
                                wp8[:, 2 * i:2 * i + 2,
                                    mo * 128:(mo + 1) * 128],
                                attn_hs[:, 2 * i:2 * i + 2, nsl],
                                start=(i == 0), stop=(i == H // 2 - 1),
                                perf_mode=DR,
                            )
                        t_sb = p4.tile([128, 512], F32, tag="tsb", bufs=2,
                                       name="t_sb")
                        nc.scalar.activation(
                            t_sb[:, :], pm2[:, :], AF.Identity,
                            bias=bproj_pp[:, mo:mo + 1], scale=1.0 / (PS * WS),
                        )
                        nc.vector.scalar_tensor_tensor(
                            xT[:, mo, nsl], t_sb[:, :],
                            ada_pp[:, 2, mo:mo + 1], xT[:, mo, nsl],
                            ALU.mult, ALU.add,
                        )
                    _ln_stats(tc, nc, xT, ones_col, pst4, pln4, ps_st2,
                              halves=(n,), st=st2)
                    _ln_apply(tc, nc, xT, mod12T, st2, ada_pp, 3, 4, pln4,
                              halves=(n,))

        # ================= phase E: FFN =====================================
        es_e = ExitStack()
        ph = es_e.enter_context(tc.tile_pool(name="ph", bufs=1))
        hT = ph.tile([128, MH, NT], FP8, name="hT")
        po = es_e.enter_context(tc.tile_pool(name="po", bufs=1))

        with tc.tile_pool(name="ps_f1", bufs=3, space="PSUM") as ps_f1, \
             tc.tile_pool(name="ps_f2", bufs=2, space="PSUM") as ps_f2, \
             tc.tile_pool(name="ps_tro", bufs=2, space="PSUM") as ps_tro:
            # fc1 in 18 chunks of 256 columns (2 m-tiles each), weights
            # already converted to fp8 in DRAM during the attention window
            with tc.tile_pool(name="p5a", bufs=1) as p5a:
                for ch in range(18):
                    f18 = p5a.tile([128, KTP, 256], FP8, tag="f18", bufs=3,
                                   name="f18")
                    nc.sync.dma_start(f18[:, 0:KT, :], w1f8_dr[ch, :, :, :])
                    nc.gpsimd.memset(f18[:, KT, :], 0.0)
                    if ch < 6:
                        emit_fc2_chunk(12 + ch,
                                       nc.vector if ch % 2 else nc.gpsimd,
                                       p5a)
                    for m in range(2):
                        mo = ch * 2 + m
                        for n in range(2):
                            nsl = slice(n * 512, (n + 1) * 512)
                            pf1 = ps_f1.tile([128, 512], F32, tag="f1",
                                             name="pf1")
                            for i in range(KTP // 2):
                                nc.tensor.matmul(
                                    pf1[:, :],
                                    f18[:, 2 * i:2 * i + 2,
                                        m * 128:(m + 1) * 128],
                                    mod12T[:, 2 * i:2 * i + 2, nsl],
                                    start=(i == 0), stop=(i == KTP // 2 - 1),
                                    perf_mode=DR,
                                )
                            nc.scalar.activation(
                                hT[:, mo, nsl], pf1[:, :], AF.Gelu_apprx_tanh,
                                bias=bfc1_pp[:, mo:mo + 1],
                                scale=1.0 / (AS * WS),
                            )
            # fc2: weights already fp8-resident in SBUF (w2f8)
            with tc.tile_pool(name="p5b", bufs=1) as p5b:
                for mo in range(KT):
                    for n in range(2):
                        nsl = slice(n * 512, (n + 1) * 512)
                        pf2 = ps_f2.tile([128, 512], F32, tag="f2", name="pf2")
                        for i in range(MH // 2):
                            nc.tensor.matmul(
                                pf2[:, :], w2f8[:, mo, 2 * i:2 * i + 2, :],
                                hT[:, 2 * i:2 * i + 2, nsl],
                                start=(i == 0), stop=(i == MH // 2 - 1),
                                perf_mode=DR,
                            )
                        t2 = p5b.tile([128, 512], F32, tag="t2", bufs=3,
                                      name="t2")
                        nc.scalar.activation(
                            t2[:, :], pf2[:, :], AF.Identity,
                            bias=bfc2_pp[:, mo:mo + 1], scale=1.0 / WS,
                        )
                        nc.vector.scalar_tensor_tensor(
                            xT[:, mo, nsl], t2[:, :], ada_pp[:, 5, mo:mo + 1],
                            xT[:, mo, nsl], ALU.mult, ALU.add,
                        )
                    o_slab = po.tile([128, NT // 128, 128], F32, tag="osl",
                                     bufs=2, name="o_slab")
                    for tt in range(NT // 128):
                        pt = ps_tro.tile([128, 128], F32, tag="tro",
                                         name="pt6")
                        nc.tensor.transpose(
                            pt[:, :], xT[:, mo, tt * 128:(tt + 1) * 128],
                            ident[:, :],
                        )
                        dst = o_slab[:, tt, :]
                        nc.vector.tensor_copy(dst, pt[:, :])
                    nc.scalar.dma_start(
                        out_dram[:, mo * 128:(mo + 1) * 128]
                        .rearrange("(t p) m -> p t m", p=128),
                        o_slab[:, :, :])
        es_w2.close()
        es_wp.close()
        es_att.close()
        es_e.close()


_LOCK = threading.Lock()
_PROG = None


def _get_program():
    global _PROG
    with _LOCK:
        if _PROG is None:
            _PROG = _build_program()
    return _PROG


def _make_in_maps(inputs):
    arrs = {k: np.ascontiguousarray(np.asarray(v, dtype=np.float32))
            for k, v in inputs.items()}
    in_maps = []
    for c in range(NCORES):
        m = {k: v for k, v in arrs.items() if k not in ("x", "t_emb")}
        m["x"] = np.ascontiguousarray(arrs["x"][c])
        m["t_emb"] = np.ascontiguousarray(arrs["t_emb"][c])
        in_maps.append(m)
    return in_maps


def kernel(**inputs):
    nc = _get_program()
    res = run_bass_kernel_spmd(nc, _make_in_maps(inputs),
                               core_ids=list(range(NCORES)))
    return np.stack([r["out"] for r in res.results], axis=0)


def kernel_traced(inputs, **kw):
    """test-harness helper: returns full BassKernelResults with trace."""
    nc = _get_program()
    return run_bass_kernel_spmd(
        nc, _make_in_maps(inputs), core_ids=list(range(NCORES)), trace=True,
        **kw
    )


# revision 75
# speedup vs baseline: 1.5514x; 1.0171x over previous
"""DiT block kernel for Trainium2 (Bass/Tile), 8-core data parallel.

Shapes (hardcoded from the problem spec):
  x: (8, 1024, 1152), t_emb: (8, 1152)
  w_qkv (1152, 3456), w_proj (1152, 1152), w_fc1 (1152, 4608),
  w_fc2 (4608, 1152), w_ada (1152, 6912) + biases.

Strategy: batch-parallel across 8 cores (one batch element each, no
collectives). Activations live transposed [D on partitions, tokens free].
The large matmuls (qkv, attention AV, proj, fc1, fc2) run in fp8e4 with
DoubleRow perf mode (two 128-row k-tiles contracted per instruction);
scale factors for fp8 range are folded into the existing activation
bias/scale stages so no extra elementwise work is added.  LayerNorm
statistics reduce over the partition axis via ones-vector f32r matmuls;
softmax runs transposed (keys on partitions) with denominators collected
through a ones-column appended to V and a fused divide.  q/k are produced
per-head directly (M=72 matmuls cost the same per column as M=128), so
attention needs no partition-crossing gather DMAs.  Weights stream
through big staged f32 DMA loads (few, large transfers) and are
converted on-chip; ada (error-sensitive) stays f32r.
"""

import threading
from contextlib import ExitStack

import numpy as np

import concourse.bass as bass
import concourse.mybir as mybir
import concourse.tile as tile
from concourse import bacc
from concourse.bass_utils import run_bass_kernel_spmd
from concourse.masks import make_identity

F32 = mybir.dt.float32
F32R = mybir.dt.float32r
BF16 = mybir.dt.bfloat16
FP8 = mybir.dt.float8e4
AF = mybir.ActivationFunctionType
ALU = mybir.AluOpType
DR = mybir.MatmulPerfMode.DoubleRow

NCORES = 8
D = 1152
NT = 1024
KT = D // 128       # 9
KTP = KT + 1        # padded to even for DoubleRow pairs
H = 16
HD = 72
HID = 4 * D
MH = HID // 128     # 36
EPS = 1e-6
ISC = 1.0 / float(np.sqrt(HD))

# fp8 scale factors
WS = 64.0           # weights
AS = 8.0            # modulated activations (mod1/mod2)
QS = 2.0            # q/k
PS = 4.0            # attention output
ES = ISC / (QS * QS)  # exp() input scale applied to the scores psum

# v output column slices aligned to head boundaries
V_SLICES = [(0, 432, 0, 6), (432, 864, 6, 12), (864, 1152, 12, 16)]


def _r(ap):
    return ap.bitcast(F32R)


def _build_program():
    nc = bacc.Bacc(
        "TRN2", target_bir_lowering=False, debug=False, enable_asserts=False
    )
    ins = {}
    ins["x"] = nc.dram_tensor("x", [NT, D], F32, kind="ExternalInput").ap()
    ins["t_emb"] = nc.dram_tensor("t_emb", [D], F32, kind="ExternalInput").ap()
    for name, shape in [
        ("w_qkv", [D, 3 * D]), ("b_qkv", [3 * D]),
        ("w_proj", [D, D]), ("b_proj", [D]),
        ("w_fc1", [D, HID]), ("b_fc1", [HID]),
        ("w_fc2", [HID, D]), ("b_fc2", [D]),
        ("w_ada", [D, 6 * D]), ("b_ada", [6 * D]),
    ]:
        ins[name] = nc.dram_tensor(name, shape, F32, kind="ExternalInput").ap()
    out_dram = nc.dram_tensor("out", [NT, D], F32, kind="ExternalOutput").ap()

    with tile.TileContext(nc) as tc:
        _body(tc, ins, out_dram)
    nc.compile()
    return nc


def _ln_stats(tc, nc, src, ones_col, pst, pln, ps_st, halves=(0, 1),
              st=None):
    """Return st[n] = [mean; rstd] rows [1, 2, 512] per 512-token half,
    reducing over the partition (D) axis of src [128, KT, NT] f32."""
    ps_x, ps_q = {}, {}
    if st is None:
        st = {}
    for n in halves:
        nsl = slice(n * 512, (n + 1) * 512)
        ps_x[n] = ps_st.tile([1, 512], F32, tag="st", name=f"psx{n}")
        ps_q[n] = ps_st.tile([1, 512], F32, tag="st", name=f"psq{n}")
        for k in range(KT):
            xb = pln.tile([128, 512], BF16, tag="xb", bufs=2, name="xb")
            nc.scalar.copy(xb[:, :], src[:, k, nsl])
            sq = pln.tile([128, 512], BF16, tag="sq", bufs=2, name="sq")
            nc.vector.tensor_mul(sq[:, :], src[:, k, nsl], src[:, k, nsl])
            nc.tensor.matmul(
                ps_x[n][:, :], ones_col[:, :], xb[:, :],
                start=(k == 0), stop=(k == KT - 1), skip_group_check=True,
            )
            nc.tensor.matmul(
                ps_q[n][:, :], ones_col[:, :], sq[:, :],
                start=(k == 0), stop=(k == KT - 1), skip_group_check=True,
            )
    eps_sb = pst.tile([1, 1], F32, tag="eps", bufs=1, name="eps_sb")
    nc.vector.memset(eps_sb[:, :], EPS)
    for n in halves:
        st[n] = pst.tile([1, 2, 512], F32, tag="lnst", bufs=2, name=f"st{n}")
        nc.vector.tensor_scalar_mul(st[n][:, 0, :], ps_x[n][:, :], 1.0 / D)
        work = pst.tile([1, 512], F32, tag="lnwork", bufs=2, name="work")
        nc.vector.tensor_mul(work[:, :], st[n][:, 0, :], st[n][:, 0, :])
        nc.vector.scalar_tensor_tensor(
            st[n][:, 1, :], ps_q[n][:, :], 1.0 / D, work[:, :],
            ALU.mult, ALU.subtract,
        )
        nc.scalar.activation(st[n][:, 1, :], st[n][:, 1, :], AF.Sqrt,
                             bias=eps_sb[:, :], scale=1.0)
        nc.vector.reciprocal(st[n][:, 1, :], st[n][:, 1, :])
    return st


def _ln_apply(tc, nc, src, dst, st, ada_pp, sh_c, sc_c, pln,
              halves=(0, 1)):
    """dst[:,k,nsl] (fp8) = ((src-mean)*rstd) * ada[sc_c] + ada[sh_c]
    (ada params pre-scaled by AS)."""
    for n in halves:
        nsl = slice(n * 512, (n + 1) * 512)
        meanB = pln.tile([128, 512], F32, tag="meanB", bufs=2, name="meanB")
        rstdB = pln.tile([128, 512], F32, tag="rstdB", bufs=2, name="rstdB")
        nc.gpsimd.partition_broadcast(meanB[:, :], st[n][:, 0, :])
        nc.gpsimd.partition_broadcast(rstdB[:, :], st[n][:, 1, :])
        for k in range(KT):
            t1 = pln.tile([128, 512], F32, tag="lnt1", bufs=3, name="t1")
            nc.vector.tensor_sub(t1[:, :], src[:, k, nsl], meanB[:, :])
            nc.vector.tensor_mul(t1[:, :], t1[:, :], rstdB[:, :])
            nc.gpsimd.tensor_scalar(
                dst[:, k, nsl], t1[:, :],
                ada_pp[:, sc_c, k:k + 1], ada_pp[:, sh_c, k:k + 1],
                ALU.mult, ALU.add,
            )


def _body(tc, ins, out_dram):
    nc = tc.nc
    ctx = ExitStack()
    with ctx:
        dram = ctx.enter_context(tc.tile_pool(name="dram", bufs=1, space="DRAM"))
        ada_dr = dram.tile([6 * D], F32)
        w1f8_dr = dram.tile([18, 128, KT, 256], FP8)

        pers = ctx.enter_context(tc.tile_pool(name="pers", bufs=1))
        ident = pers.tile([128, 128], F32)
        make_identity(nc, ident[:, :])
        ones_col = pers.tile([128, 1], BF16)
        nc.vector.memset(ones_col[:, :], 1.0)
        ones_row = pers.tile([1, 128], BF16)
        nc.vector.memset(ones_row[:, :], 1.0)

        t_pp = pers.tile([128, KT], F32)
        nc.sync.dma_start(t_pp[:, :], ins["t_emb"].rearrange("(k p) -> p k", p=128))
        t_pr = pers.tile([128, KT], F32R)
        nc.scalar.activation(t_pr[:, :], t_pp[:, :], AF.Silu)

        bq_s = pers.tile([72, H], F32)
        bk_s = pers.tile([72, H], F32)
        bv_row = pers.tile([1, D], F32)
        bv_b = pers.tile([1, D], BF16)
        bproj_pp = pers.tile([128, KT], F32)
        bfc1_pp = pers.tile([128, MH], F32)
        bfc2_pp = pers.tile([128, KT], F32)
        bada_pp = pers.tile([128, 6, KT], F32)
        ada_pp = pers.tile([128, 6, KT], F32)

        def emit_bias_loads():
            nc.sync.dma_start(
                bq_s[:, :], ins["b_qkv"][0:D].rearrange("(h p) -> p h", p=72))
            nc.sync.dma_start(
                bk_s[:, :], ins["b_qkv"][D:2 * D].rearrange("(h p) -> p h", p=72))
            nc.sync.dma_start(
                bv_row[:, :],
                ins["b_qkv"][2 * D:3 * D].rearrange("(a b) -> a b", a=1))
            # bv enters the v accumulation in (AS*WS)-scaled psum units
            nc.vector.tensor_scalar_mul(bv_b[:, :], bv_row[:, :], AS * WS)
            nc.sync.dma_start(
                bproj_pp[:, :], ins["b_proj"].rearrange("(m p) -> p m", p=128))
            nc.sync.dma_start(
                bfc1_pp[:, :], ins["b_fc1"].rearrange("(m p) -> p m", p=128))
            nc.sync.dma_start(
                bfc2_pp[:, :], ins["b_fc2"].rearrange("(m p) -> p m", p=128))
            nc.sync.dma_start(
                bada_pp[:, :, :],
                ins["b_ada"].rearrange("(c k p) -> p c k", k=KT, p=128))
            # pre-scale q/k biases by QS (folded into the psum->fp8 copies)
            nc.vector.tensor_scalar_mul(bq_s[:, :], bq_s[:, :], QS)
            nc.vector.tensor_scalar_mul(bk_s[:, :], bk_s[:, :], QS)

        xT = pers.tile([128, KT, NT], F32)      # becomes x2T after residual 1
        mod12T = pers.tile([128, KTP, NT], FP8)  # mod1T, later reused as mod2T
        nc.gpsimd.memset(mod12T[:, KT, :], 0.0)  # DoubleRow pad k-tile

        # ================= phase A: x load/transpose, ada, LN1 ==============

        def emit_ada_chunk(c, p1w, ps_pro, ps_bufs=2):
            """chunk c covers w_ada cols [c*384, (c+1)*384); param p=c//3."""
            wst = p1w.tile([128, KT, 384], F32R, tag="adast", bufs=2, name="wst")
            nc.sync.dma_start(
                wst[:, :, :],
                ins["w_ada"][:, c * 384:(c + 1) * 384]
                .rearrange("(k p) m -> p k m", p=128).bitcast(F32R),
            )
            pa = ps_pro.tile([1, 384], F32, tag="psada", bufs=ps_bufs,
                             name="pa")
            for k in range(KT):
                nc.tensor.matmul(
                    pa[:, :], t_pr[:, k:k + 1], wst[:, k, :],
                    start=(k == 0), stop=(k == KT - 1),
                )
            asb = p1w.tile([1, 384], F32, tag="asb", bufs=3, name="asb")
            nc.vector.tensor_copy(asb[:, :], pa[:, :])
            nc.scalar.dma_start(
                ada_dr[c * 384:(c + 1) * 384].rearrange("(a b) -> a b", a=1),
                asb[0:1, :],
            )

        def emit_ada_pp_load(cs):
            """Load+finalize ada params cs (list) into ada_pp; params 0/1
            (shift_a/scale_a) and 3/4 are pre-scaled by AS; 1/4 get +1."""
            for c in cs:
                nc.scalar.dma_start(
                    ada_pp[:, c, :],
                    ada_dr[c * D:(c + 1) * D].rearrange("(k p) -> p k", p=128),
                )
            lo, hi = min(cs), max(cs) + 1
            nc.vector.tensor_add(ada_pp[:, lo:hi, :], ada_pp[:, lo:hi, :],
                                 bada_pp[:, lo:hi, :])
            for c in cs:
                if c in (1, 4):
                    nc.vector.tensor_scalar_add(ada_pp[:, c, :],
                                                ada_pp[:, c, :], 1.0)
                if c in (0, 1, 3, 4):
                    nc.vector.tensor_scalar_mul(ada_pp[:, c, :],
                                                ada_pp[:, c, :], AS)

        with tc.tile_pool(name="p1w", bufs=1) as p1w, \
             tc.tile_pool(name="pxin", bufs=3) as pxin, \
             tc.tile_pool(name="ps_pro", bufs=2, space="PSUM") as ps_pro, \
             tc.tile_pool(name="ps_tr", bufs=2, space="PSUM") as ps_tr:

            def emit_transpose_block(tt):
                xin = pxin.tile([128, D], F32, tag="xin", name="xin")
                nc.sync.dma_start(
                    xin[:, :], ins["x"][tt * 128:(tt + 1) * 128, :])
                for kd in range(KT):
                    pt = ps_tr.tile([128, 128], F32, tag="ptr", name="pt")
                    nc.tensor.transpose(
                        pt[:, :], xin[:, kd * 128:(kd + 1) * 128], ident[:, :])
                    tsl = slice(tt * 128, (tt + 1) * 128)
                    if kd % 2 == 0:
                        nc.vector.tensor_copy(xT[:, kd, tsl], pt[:, :])
                    else:
                        nc.scalar.copy(xT[:, kd, tsl], pt[:, :])

            for i in range(8):
                emit_transpose_block(i)
                if i == 0:
                    emit_bias_loads()
                if i < 6:
                    emit_ada_chunk(i, p1w, ps_pro)
            emit_ada_pp_load([0, 1])

        # ====== phase B part 1: qkv weight loads + converts (emitted before
        # LN1 so SP streams the loads while ada finishes / LN runs) =========
        es_qk = ExitStack()
        pqk8 = es_qk.enter_context(tc.tile_pool(name="pqk8", bufs=1))
        wq8 = pqk8.tile([128, KTP, D], FP8, name="wq8")
        wk8 = pqk8.tile([128, KTP, D], FP8, name="wk8")
        nc.gpsimd.memset(wq8[:, KT, :], 0.0)
        nc.gpsimd.memset(wk8[:, KT, :], 0.0)

        es_att = ExitStack()
        patt = es_att.enter_context(tc.tile_pool(name="patt", bufs=1, side="right"))
        attn_hs = patt.tile([72, H, NT], FP8, name="attn_hs")
        es_wp = ExitStack()
        pwp8 = es_wp.enter_context(
            tc.tile_pool(name="pwp8", bufs=1, side="right"))
        wp8 = pwp8.tile([72, H, D], FP8, name="wp8")
        es_va = ExitStack()
        pva = es_va.enter_context(tc.tile_pool(name="pva", bufs=1, side="right"))
        v_aug = pva.tile([128, NT // 128, H, 97], FP8, name="v_aug")
        nc.gpsimd.memset(v_aug[:, :, :, HD:97], 0.0)
        nc.gpsimd.memset(v_aug[:, :, :, 96:97], 1.0)

        es_b = ExitStack()
        pwst = es_b.enter_context(tc.tile_pool(name="pwst", bufs=1))
        wv8 = pwst.tile([128, KTP, D], FP8, tag="wv8", bufs=1, name="wv8")
        nc.gpsimd.memset(wv8[:, KT, :], 0.0)
        engs = ["act", "dve", "act", "dve", "act", "dve"]
        for j, (dst8, c0) in enumerate(((wq8, 0), (wk8, D), (wv8, 2 * D))):
            for half in range(2):
                msl = slice(half * 576, (half + 1) * 576)
                wst = pwst.tile([128, KT, 576], F32, tag="wst", bufs=2,
                                name="wst")
                nc.sync.dma_start(
                    wst[:, :, :],
                    ins["w_qkv"][:, c0 + half * 576:c0 + (half + 1) * 576]
                    .rearrange("(k p) m -> p k m", p=128),
                )
                eng = engs[j * 2 + half]
                for kk in range(3):
                    ksl = slice(kk * 3, kk * 3 + 3)
                    if eng == "act":
                        nc.scalar.activation(
                            dst8[:, ksl, msl], wst[:, ksl, :],
                            AF.Identity, scale=WS)
                    elif eng == "dve":
                        nc.vector.tensor_scalar_mul(
                            dst8[:, ksl, msl], wst[:, ksl, :], WS)
                    else:
                        nc.gpsimd.tensor_scalar_mul(
                            dst8[:, ksl, msl], wst[:, ksl, :], WS)

        # ====== LN1 (per-half, interleaved with v matmuls) ==================
        with tc.tile_pool(name="pst", bufs=1) as pst, \
             tc.tile_pool(name="pln", bufs=1) as pln, \
             tc.tile_pool(name="ps_st", bufs=4, space="PSUM") as ps_st, \
             tc.tile_pool(name="ps_v", bufs=3, space="PSUM") as ps_v:

            def v_block(tts):
                for tt in tts:
                    tsl = slice(tt * 128, (tt + 1) * 128)
                    for si, (c0, c1, h0, h1) in enumerate(V_SLICES):
                        pmv = ps_v.tile([128, 512], F32, tag="mv", name="pmv")
                        for i in range(KTP // 2):
                            nc.tensor.matmul(
                                pmv[:, 0:c1 - c0],
                                mod12T[:, 2 * i:2 * i + 2, tsl],
                                wv8[:, 2 * i:2 * i + 2, c0:c1],
                                start=(i == 0), stop=False, perf_mode=DR,
                                skip_group_check=True,
                            )
                        nc.tensor.matmul(
                            pmv[:, 0:c1 - c0], ones_row[:, :],
                            bv_b[:, c0:c1],
                            start=False, stop=True, skip_group_check=True,
                        )
                        vsrc = pmv[:, 0:c1 - c0].rearrange(
                            "p (h d) -> p h d", d=HD)
                        nc.vector.tensor_scalar_mul(
                            v_aug[:, tt, h0:h1, 0:HD], vsrc, 1.0 / (AS * WS))

            st1 = {}
            _ln_stats(tc, nc, xT, ones_col, pst, pln, ps_st, halves=(0,),
                      st=st1)
            _ln_apply(tc, nc, xT, mod12T, st1, ada_pp, 0, 1, pln, halves=(0,))
            _ln_stats(tc, nc, xT, ones_col, pst, pln, ps_st, halves=(1,),
                      st=st1)
            v_block(range(0, 4))
            _ln_apply(tc, nc, xT, mod12T, st1, ada_pp, 0, 1, pln, halves=(1,))
            v_block(range(4, 8))
        es_b.close()

        # ================= phase C: attention ===============================
        with tc.tile_pool(name="p3w", bufs=1) as p3w, \
             tc.tile_pool(name="pexp", bufs=1) as pexp, \
             tc.tile_pool(name="pat3", bufs=1) as pat3, \
             tc.tile_pool(name="ps_qk", bufs=2, space="PSUM") as ps_qk, \
             tc.tile_pool(name="ps_s", bufs=2, space="PSUM") as ps_s, \
             tc.tile_pool(name="ps_av", bufs=1, space="PSUM") as ps_av, \
             tc.tile_pool(name="ps_pa", bufs=1, space="PSUM") as ps_pa:

            def emit_fc1_stream(j):
                f1st = p3w.tile([128, KT, 256], F32, tag="f1st",
                                bufs=2, name="f1st")
                nc.sync.dma_start(
                    f1st[:, :, :],
                    ins["w_fc1"][:, j * 256:(j + 1) * 256]
                    .rearrange("(k p) m -> p k m", p=128),
                )
                f18o = p3w.tile([128, KT, 256], FP8, tag="f18o",
                                bufs=2, name="f18o")
                nc.gpsimd.tensor_scalar_mul(
                    f18o[:, :, :], f1st[:, :, :], WS)
                nc.scalar.dma_start(w1f8_dr[j, :, :, :], f18o[:, :, :])

            def emit_wp_stream(c):
                # reuse the f1st staging tag: [128, KT*256] bytes == 16*144
                wpt = p3w.tile([128, KT, 256], F32, tag="f1st", bufs=2,
                               name="wpt")
                wpv = (wpt[:, :, :].rearrange("p k m -> p (k m)")[0:72, :]
                       .rearrange("p (h m) -> p h m", h=H))
                msl = slice(c * 144, (c + 1) * 144)
                nc.sync.dma_start(
                    wpv[:, :, :],
                    ins["w_proj"][:, msl].rearrange("(h p) m -> p h m", p=72),
                )
                nc.vector.tensor_scalar_mul(
                    wp8[:, :, msl], wpv[:, :, :], WS)

            def emit_filler(h):
                # late ada chunks; fc1 fp8 stream-convert to DRAM
                if h % 4 != 3:
                    emit_ada_chunk(6 + h - h // 4, p3w, ps_pa, ps_bufs=1)
                if h == 15:
                    emit_ada_pp_load([2, 3])
                    emit_ada_pp_load([4, 5])
                if 2 <= h:
                    js = ([2 * h - 4, 2 * h - 3] if h < 6
                          else [h + 2])
                    for j in js:
                        emit_fc1_stream(j)

            for h in range(H):
                emit_filler(h)
                q_h = pat3.tile([72, NT], FP8, tag="qh", bufs=2, name="q_h")
                k_h = pat3.tile([72, NT], FP8, tag="kh", bufs=2, name="k_h")
                for n in range(2):
                    nsl = slice(n * 512, (n + 1) * 512)
                    pq = ps_qk.tile([72, 512], F32, tag="qk", name="pq")
                    for i in range(KTP // 2):
                        nc.tensor.matmul(
                            pq[:, :],
                            wq8[:, 2 * i:2 * i + 2, h * HD:(h + 1) * HD],
                            mod12T[:, 2 * i:2 * i + 2, nsl],
                            start=(i == 0), stop=(i == KTP // 2 - 1),
                            perf_mode=DR,
                        )
                    nc.vector.tensor_scalar(
                        q_h[:, nsl], pq[:, :], QS / (AS * WS),
                        bq_s[:, h:h + 1], ALU.mult, ALU.add,
                    )
                for n in range(2):
                    nsl = slice(n * 512, (n + 1) * 512)
                    pk = ps_qk.tile([72, 512], F32, tag="qk", name="pk")
                    for i in range(KTP // 2):
                        nc.tensor.matmul(
                            pk[:, :],
                            wk8[:, 2 * i:2 * i + 2, h * HD:(h + 1) * HD],
                            mod12T[:, 2 * i:2 * i + 2, nsl],
                            start=(i == 0), stop=(i == KTP // 2 - 1),
                            perf_mode=DR,
                        )
                    nc.vector.tensor_scalar(
                        k_h[:, nsl], pk[:, :], QS / (AS * WS),
                        bk_s[:, h:h + 1], ALU.mult, ALU.add,
                    )
                for n in range(2):
                    nsl = slice(n * 512, (n + 1) * 512)
                    exp_hn = pexp.tile([128, NT // 128, 512], FP8, tag="exp",
                                       bufs=3, name="exp_hn")
                    for kp in range(NT // 256):
                        pss = ps_s.tile([128, 2, 512], F32, tag="s",
                                        name="pss")
                        for j in range(2):
                            kt_i = 2 * kp + j
                            nc.tensor.matmul(
                                pss[:, j, :],
                                k_h[:, kt_i * 128:(kt_i + 1) * 128],
                                q_h[:, nsl], start=True, stop=True,
                            )
                        nc.scalar.activation(
                            exp_hn[:, 2 * kp:2 * kp + 2, :],
                            pss[:, :, :], AF.Exp, scale=ES)
                    pav = ps_av.tile([97, 512], F32, tag="av", name="pav")
                    for i in range(NT // 256):
                        nc.tensor.matmul(
                            pav[:, :],
                            v_aug[:, 2 * i:2 * i + 2, h, :],
                            exp_hn[:, 2 * i:2 * i + 2, :],
                            start=(i == 0), stop=(i == NT // 256 - 1),
                            perf_mode=DR,
                        )
                    den = pat3.tile([1, 512], F32, tag="den", bufs=2,
                                    name="den")
                    nc.vector.tensor_scalar_mul(den[:, :], pav[96:97, :],
                                                1.0 / PS)
                    nc.vector.reciprocal(den[:, :], den[:, :])
                    denB = pat3.tile([72, 512], F32, tag="denB", bufs=2,
                                     name="denB")
                    nc.gpsimd.partition_broadcast(denB[:, :], den[:, :])
                    nc.vector.tensor_mul(
                        attn_hs[:, h, nsl], pav[0:HD, :], denB[:, :])
        es_qk.close()  # wq8/wk8 no longer needed
        es_va.close()

        # ================= phase D: proj + residual + LN2 ===================
        es_w2 = ExitStack()
        pw2 = es_w2.enter_context(
            tc.tile_pool(name="pw2", bufs=1, side="right"))
        w2f8 = pw2.tile([128, KT, MH, 128], FP8, name="w2f8")

        with tc.tile_pool(name="p4", bufs=1) as p4, \
             tc.tile_pool(name="pst4", bufs=1) as pst4, \
             tc.tile_pool(name="pln4", bufs=1) as pln4:
            for i in range(6):
                msl = slice(i * 192, (i + 1) * 192)
                wpst = p4.tile([72, H, 192], F32, tag="wpst", bufs=2,
                               name="wpst")
                nc.sync.dma_start(
                    wpst[:, :, :],
                    ins["w_proj"][:, msl].rearrange("(h p) m -> p h m", p=72),
                )
                for kk in range(2):
                    hsl = slice(kk * 8, kk * 8 + 8)
                    nc.vector.tensor_scalar_mul(
                        wp8[:, hsl, msl], wpst[:, hsl, :], WS)

            def emit_fc2_chunk(ch, eng, pool):
                f2s = pool.tile([128, MH, 64], F32, tag="f2s", bufs=2,
                                name="f2s")
                nc.sync.dma_start(
                    f2s[:, :, :],
                    ins["w_fc2"][:, ch * 64:(ch + 1) * 64]
                    .rearrange("(k p) m -> p k m", p=128),
                )
                eng.tensor_scalar_mul(
                    w2f8[:, ch // 2, :, (ch % 2) * 64:(ch % 2 + 1) * 64],
                    f2s[:, :, :], WS)

            st2 = {}
            with tc.tile_pool(name="ps_mm2", bufs=3, space="PSUM") as ps_mm2, \
                 tc.tile_pool(name="ps_st2", bufs=4, space="PSUM") as ps_st2:
                for n in range(2):
                    nsl = slice(n * 512, (n + 1) * 512)
                    for mo in range(KT):
                        if mo < 4:
                            ch = n * 4 + mo
                            eng = nc.vector if ch % 2 else nc.gpsimd
                            emit_fc2_chunk(ch, eng, p4)
                        pm2 = ps_mm2.tile([128, 512], F32, tag="mm2",
                                          name="pm2")
                        for i in range(H // 2):
                            nc.tensor.matmul(
                                pm2[:, :],
                                wp8[:, 2 * i:2 * i + 2,
                                    mo * 128:(mo + 1) * 128],
                                attn_hs[:, 2 * i:2 * i + 2, nsl],
                                start=(i == 0), stop=(i == H // 2 - 1),
                                perf_mode=DR,
                            )
                        t_sb = p4.tile([128, 512], F32, tag="tsb", bufs=2,
                                       name="t_sb")
                        nc.scalar.activation(
                            t_sb[:, :], pm2[:, :], AF.Identity,
                            bias=bproj_pp[:, mo:mo + 1], scale=1.0 / (PS * WS),
                        )
                        nc.vector.scalar_tensor_tensor(
                            xT[:, mo, nsl], t_sb[:, :],
                            ada_pp[:, 2, mo:mo + 1], xT[:, mo, nsl],
                            ALU.mult, ALU.add,
                        )
                    _ln_stats(tc, nc, xT, ones_col, pst4, pln4, ps_st2,
                              halves=(n,), st=st2)
                    _ln_apply(tc, nc, xT, mod12T, st2, ada_pp, 3, 4, pln4,
                              halves=(n,))

        # ================= phase E: FFN =====================================
        es_e = ExitStack()
        ph = es_e.enter_context(tc.tile_pool(name="ph", bufs=1))
        hT = ph.tile([128, MH, NT], FP8, name="hT")
        po = es_e.enter_context(tc.tile_pool(name="po", bufs=1))

        with tc.tile_pool(name="ps_f1", bufs=3, space="PSUM") as ps_f1, \
             tc.tile_pool(name="ps_f2", bufs=2, space="PSUM") as ps_f2, \
             tc.tile_pool(name="ps_tro", bufs=2, space="PSUM") as ps_tro:
            # fc1 in 18 chunks of 256 columns (2 m-tiles each), weights
            # already converted to fp8 in DRAM during the attention window
            with tc.tile_pool(name="p5a", bufs=1) as p5a:
                for ch in range(18):
                    f18 = p5a.tile([128, KTP, 256], FP8, tag="f18", bufs=3,
                                   name="f18")
                    nc.sync.dma_start(f18[:, 0:KT, :], w1f8_dr[ch, :, :, :])
                    nc.gpsimd.memset(f18[:, KT, :], 0.0)
                    if ch < 10:
                        emit_fc2_chunk(8 + ch,
                                       nc.vector if ch % 2 else nc.gpsimd,
                                       p5a)
                    for m in range(2):
                        mo = ch * 2 + m
                        for n in range(2):
                            nsl = slice(n * 512, (n + 1) * 512)
                            pf1 = ps_f1.tile([128, 512], F32, tag="f1",
                                             name="pf1")
                            for i in range(KTP // 2):
                                nc.tensor.matmul(
                                    pf1[:, :],
                                    f18[:, 2 * i:2 * i + 2,
                                        m * 128:(m + 1) * 128],
                                    mod12T[:, 2 * i:2 * i + 2, nsl],
                                    start=(i == 0), stop=(i == KTP // 2 - 1),
                                    perf_mode=DR,
                                )
                            nc.scalar.activation(
                                hT[:, mo, nsl], pf1[:, :], AF.Gelu_apprx_tanh,
                                bias=bfc1_pp[:, mo:mo + 1],
                                scale=1.0 / (AS * WS),
                            )
            # fc2: weights already fp8-resident in SBUF (w2f8)
            with tc.tile_pool(name="p5b", bufs=1) as p5b:
                for mo in range(KT):
                    for n in range(2):
                        nsl = slice(n * 512, (n + 1) * 512)
                        pf2 = ps_f2.tile([128, 512], F32, tag="f2", name="pf2")
                        for i in range(MH // 2):
                            nc.tensor.matmul(
                                pf2[:, :], w2f8[:, mo, 2 * i:2 * i + 2, :],
                                hT[:, 2 * i:2 * i + 2, nsl],
                                start=(i == 0), stop=(i == MH // 2 - 1),
                                perf_mode=DR,
                            )
                        t2 = p5b.tile([128, 512], F32, tag="t2", bufs=3,
                                      name="t2")
                        nc.scalar.activation(
                            t2[:, :], pf2[:, :], AF.Identity,
                            bias=bfc2_pp[:, mo:mo + 1], scale=1.0 / WS,
                        )
                        nc.vector.scalar_tensor_tensor(
                            xT[:, mo, nsl], t2[:, :], ada_pp[:, 5, mo:mo + 1],
                            xT[:, mo, nsl], ALU.mult, ALU.add,
                        )
                    o_slab = po.tile([128, NT // 128, 128], F32, tag="osl",
                                     bufs=2, name="o_slab")
                    for tt in range(NT // 128):
                        pt = ps_tro.tile([128, 128], F32, tag="tro",
                                         name="pt6")
                        nc.tensor.transpose(
                            pt[:, :], xT[:, mo, tt * 128:(tt + 1) * 128],
                            ident[:, :],
                        )
                        dst = o_slab[:, tt, :]
                        nc.vector.tensor_copy(dst, pt[:, :])
                    nc.scalar.dma_start(
                        out_dram[:, mo * 128:(mo + 1) * 128]
                        .rearrange("(t p) m -> p t m", p=128),
                        o_slab[:, :, :])
        es_w2.close()
        es_wp.close()
        es_att.close()
        es_e.close()


_LOCK = threading.Lock()
_PROG = None


def _get_program():
    global _PROG
    with _LOCK:
        if _PROG is None:
            _PROG = _build_program()
    return _PROG


def _make_in_maps(inputs):
    arrs = {k: np.ascontiguousarray(np.asarray(v, dtype=np.float32))
            for k, v in inputs.items()}
    in_maps = []
    for c in range(NCORES):
        m = {k: v for k, v in arrs.items() if k not in ("x", "t_emb")}
        m["x"] = np.ascontiguousarray(arrs["x"][c])
        m["t_emb"] = np.ascontiguousarray(arrs["t_emb"][c])
        in_maps.append(m)
    return in_maps


def kernel(**inputs):
    nc = _get_program()
    res = run_bass_kernel_spmd(nc, _make_in_maps(inputs),
                               core_ids=list(range(NCORES)))
    return np.stack([r["out"] for r in res.results], axis=0)


def kernel_traced(inputs, **kw):
    """test-harness helper: returns full BassKernelResults with trace."""
    nc = _get_program()
    return run_bass_kernel_spmd(
        nc, _make_in_maps(inputs), core_ids=list(range(NCORES)), trace=True,
        **kw
    )


# revision 84
# speedup vs baseline: 1.5972x; 1.0296x over previous
"""DiT block kernel for Trainium2 (Bass/Tile), 8-core data parallel.

Shapes (hardcoded from the problem spec):
  x: (8, 1024, 1152), t_emb: (8, 1152)
  w_qkv (1152, 3456), w_proj (1152, 1152), w_fc1 (1152, 4608),
  w_fc2 (4608, 1152), w_ada (1152, 6912) + biases.

Strategy: batch-parallel across 8 cores (one batch element each, no
collectives). Activations live transposed [D on partitions, tokens free].
The large matmuls (qkv, attention AV, proj, fc1, fc2) run in fp8e4 with
DoubleRow perf mode (two 128-row k-tiles contracted per instruction);
scale factors for fp8 range are folded into the existing activation
bias/scale stages so no extra elementwise work is added.  LayerNorm
statistics reduce over the partition axis via ones-vector f32r matmuls;
softmax runs transposed (keys on partitions) with denominators collected
through a ones-column appended to V and a fused divide.  q/k are produced
per-head directly (M=72 matmuls cost the same per column as M=128), so
attention needs no partition-crossing gather DMAs.  Weights stream
through big staged f32 DMA loads (few, large transfers) and are
converted on-chip; ada (error-sensitive) stays f32r.
"""

import threading
from contextlib import ExitStack

import numpy as np

import concourse.bass as bass
import concourse.mybir as mybir
import concourse.tile as tile
from concourse import bacc
from concourse.bass_utils import run_bass_kernel_spmd
from concourse.masks import make_identity

F32 = mybir.dt.float32
F32R = mybir.dt.float32r
BF16 = mybir.dt.bfloat16
FP8 = mybir.dt.float8e4
AF = mybir.ActivationFunctionType
ALU = mybir.AluOpType
DR = mybir.MatmulPerfMode.DoubleRow

NCORES = 8
D = 1152
NT = 1024
KT = D // 128       # 9
KTP = KT + 1        # padded to even for DoubleRow pairs
H = 16
HD = 72
HID = 4 * D
MH = HID // 128     # 36
EPS = 1e-6
ISC = 1.0 / float(np.sqrt(HD))

# fp8 scale factors
WS = 64.0           # weights
AS = 8.0            # modulated activations (mod1/mod2)
QS = 2.0            # q/k
PS = 4.0            # attention output
ES = ISC / (QS * QS)  # exp() input scale applied to the scores psum

# v output column slices aligned to head boundaries
V_SLICES = [(0, 432, 0, 6), (432, 864, 6, 12), (864, 1152, 12, 16)]


def _r(ap):
    return ap.bitcast(F32R)


def _build_program():
    nc = bacc.Bacc(
        "TRN2", target_bir_lowering=False, debug=False, enable_asserts=False
    )
    ins = {}
    ins["x"] = nc.dram_tensor("x", [NT, D], F32, kind="ExternalInput").ap()
    ins["t_emb"] = nc.dram_tensor("t_emb", [D], F32, kind="ExternalInput").ap()
    for name, shape in [
        ("w_qkv", [D, 3 * D]), ("b_qkv", [3 * D]),
        ("w_proj", [D, D]), ("b_proj", [D]),
        ("w_fc1", [D, HID]), ("b_fc1", [HID]),
        ("w_fc2", [HID, D]), ("b_fc2", [D]),
        ("w_ada", [D, 6 * D]), ("b_ada", [6 * D]),
    ]:
        ins[name] = nc.dram_tensor(name, shape, F32, kind="ExternalInput").ap()
    out_dram = nc.dram_tensor("out", [NT, D], F32, kind="ExternalOutput").ap()

    with tile.TileContext(nc) as tc:
        _body(tc, ins, out_dram)
    nc.compile()
    return nc


def _ln_stats(tc, nc, src, ones_col, pst, pln, ps_st, halves=(0, 1),
              st=None):
    """Return st[n] = [mean; rstd] rows [1, 2, 512] per 512-token half,
    reducing over the partition (D) axis of src [128, KT, NT] f32."""
    ps_x, ps_q = {}, {}
    if st is None:
        st = {}
    for n in halves:
        nsl = slice(n * 512, (n + 1) * 512)
        ps_x[n] = ps_st.tile([1, 512], F32, tag="st", name=f"psx{n}")
        ps_q[n] = ps_st.tile([1, 512], F32, tag="st", name=f"psq{n}")
        for k in range(KT):
            xb = pln.tile([128, 512], BF16, tag="xb", bufs=2, name="xb")
            nc.scalar.copy(xb[:, :], src[:, k, nsl])
            sq = pln.tile([128, 512], BF16, tag="sq", bufs=2, name="sq")
            nc.vector.tensor_mul(sq[:, :], src[:, k, nsl], src[:, k, nsl])
            nc.tensor.matmul(
                ps_x[n][:, :], ones_col[:, :], xb[:, :],
                start=(k == 0), stop=(k == KT - 1), skip_group_check=True,
            )
            nc.tensor.matmul(
                ps_q[n][:, :], ones_col[:, :], sq[:, :],
                start=(k == 0), stop=(k == KT - 1), skip_group_check=True,
            )
    eps_sb = pst.tile([1, 1], F32, tag="eps", bufs=1, name="eps_sb")
    nc.vector.memset(eps_sb[:, :], EPS)
    for n in halves:
        st[n] = pst.tile([1, 2, 512], F32, tag="lnst", bufs=2, name=f"st{n}")
        nc.vector.tensor_scalar_mul(st[n][:, 0, :], ps_x[n][:, :], 1.0 / D)
        work = pst.tile([1, 512], F32, tag="lnwork", bufs=2, name="work")
        nc.vector.tensor_mul(work[:, :], st[n][:, 0, :], st[n][:, 0, :])
        nc.vector.scalar_tensor_tensor(
            st[n][:, 1, :], ps_q[n][:, :], 1.0 / D, work[:, :],
            ALU.mult, ALU.subtract,
        )
        nc.scalar.activation(st[n][:, 1, :], st[n][:, 1, :], AF.Sqrt,
                             bias=eps_sb[:, :], scale=1.0)
        nc.vector.reciprocal(st[n][:, 1, :], st[n][:, 1, :])
    return st


def _ln_apply(tc, nc, src, dst, st, ada_pp, sh_c, sc_c, pln,
              halves=(0, 1)):
    """dst[:,k,nsl] (fp8) = ((src-mean)*rstd) * ada[sc_c] + ada[sh_c]
    (ada params pre-scaled by AS)."""
    for n in halves:
        nsl = slice(n * 512, (n + 1) * 512)
        meanB = pln.tile([128, 512], F32, tag="meanB", bufs=2, name="meanB")
        rstdB = pln.tile([128, 512], F32, tag="rstdB", bufs=2, name="rstdB")
        nc.gpsimd.partition_broadcast(meanB[:, :], st[n][:, 0, :])
        nc.gpsimd.partition_broadcast(rstdB[:, :], st[n][:, 1, :])
        for k in range(KT):
            t1 = pln.tile([128, 512], F32, tag="lnt1", bufs=3, name="t1")
            nc.vector.tensor_sub(t1[:, :], src[:, k, nsl], meanB[:, :])
            nc.vector.tensor_mul(t1[:, :], t1[:, :], rstdB[:, :])
            nc.gpsimd.tensor_scalar(
                dst[:, k, nsl], t1[:, :],
                ada_pp[:, sc_c, k:k + 1], ada_pp[:, sh_c, k:k + 1],
                ALU.mult, ALU.add,
            )


def _body(tc, ins, out_dram):
    nc = tc.nc
    ctx = ExitStack()
    with ctx:
        dram = ctx.enter_context(tc.tile_pool(name="dram", bufs=1, space="DRAM"))
        ada_dr = dram.tile([6 * D], F32)
        w1f8_dr = dram.tile([18, 128, KT, 256], FP8)

        pers = ctx.enter_context(tc.tile_pool(name="pers", bufs=1))
        ident = pers.tile([128, 128], F32)
        make_identity(nc, ident[:, :])
        ones_col = pers.tile([128, 1], BF16)
        nc.vector.memset(ones_col[:, :], 1.0)
        ones_row = pers.tile([1, 128], BF16)
        nc.vector.memset(ones_row[:, :], 1.0)

        t_pp = pers.tile([128, KT], F32)
        nc.sync.dma_start(t_pp[:, :], ins["t_emb"].rearrange("(k p) -> p k", p=128))
        t_pr = pers.tile([128, KT], F32R)
        nc.scalar.activation(t_pr[:, :], t_pp[:, :], AF.Silu)

        bq_s = pers.tile([72, H], F32)
        bk_s = pers.tile([72, H], F32)
        bv_row = pers.tile([1, D], F32)
        bv_b = pers.tile([1, D], BF16)
        bproj_pp = pers.tile([128, KT], F32)
        bfc1_pp = pers.tile([128, MH], F32)
        bfc2_pp = pers.tile([128, KT], F32)
        bada_pp = pers.tile([128, 6, KT], F32)
        ada_pp = pers.tile([128, 6, KT], F32)

        def emit_bias_loads():
            nc.sync.dma_start(
                bq_s[:, :], ins["b_qkv"][0:D].rearrange("(h p) -> p h", p=72))
            nc.sync.dma_start(
                bk_s[:, :], ins["b_qkv"][D:2 * D].rearrange("(h p) -> p h", p=72))
            nc.sync.dma_start(
                bv_row[:, :],
                ins["b_qkv"][2 * D:3 * D].rearrange("(a b) -> a b", a=1))
            # bv enters the v accumulation in (AS*WS)-scaled psum units
            nc.vector.tensor_scalar_mul(bv_b[:, :], bv_row[:, :], AS * WS)
            nc.sync.dma_start(
                bproj_pp[:, :], ins["b_proj"].rearrange("(m p) -> p m", p=128))
            nc.sync.dma_start(
                bfc1_pp[:, :], ins["b_fc1"].rearrange("(m p) -> p m", p=128))
            nc.sync.dma_start(
                bfc2_pp[:, :], ins["b_fc2"].rearrange("(m p) -> p m", p=128))
            nc.sync.dma_start(
                bada_pp[:, :, :],
                ins["b_ada"].rearrange("(c k p) -> p c k", k=KT, p=128))
            # pre-scale q/k biases by QS (folded into the psum->fp8 copies)
            nc.vector.tensor_scalar_mul(bq_s[:, :], bq_s[:, :], QS)
            nc.vector.tensor_scalar_mul(bk_s[:, :], bk_s[:, :], QS)

        xT = pers.tile([128, KT, NT], F32)      # becomes x2T after residual 1
        mod12T = pers.tile([128, KTP, NT], FP8)  # mod1T, later reused as mod2T
        nc.gpsimd.memset(mod12T[:, KT, :], 0.0)  # DoubleRow pad k-tile

        # ================= phase A: x load/transpose, ada, LN1 ==============

        def emit_ada_chunk(c, p1w, ps_pro, ps_bufs=2):
            """chunk c covers w_ada cols [c*384, (c+1)*384); param p=c//3."""
            wst = p1w.tile([128, KT, 384], F32R, tag="adast", bufs=2, name="wst")
            nc.sync.dma_start(
                wst[:, :, :],
                ins["w_ada"][:, c * 384:(c + 1) * 384]
                .rearrange("(k p) m -> p k m", p=128).bitcast(F32R),
            )
            pa = ps_pro.tile([1, 384], F32, tag="psada", bufs=ps_bufs,
                             name="pa")
            for k in range(KT):
                nc.tensor.matmul(
                    pa[:, :], t_pr[:, k:k + 1], wst[:, k, :],
                    start=(k == 0), stop=(k == KT - 1),
                )
            asb = p1w.tile([1, 384], F32, tag="asb", bufs=3, name="asb")
            nc.vector.tensor_copy(asb[:, :], pa[:, :])
            nc.scalar.dma_start(
                ada_dr[c * 384:(c + 1) * 384].rearrange("(a b) -> a b", a=1),
                asb[0:1, :],
            )

        def emit_ada_pp_load(cs):
            """Load+finalize ada params cs (list) into ada_pp; params 0/1
            (shift_a/scale_a) and 3/4 are pre-scaled by AS; 1/4 get +1."""
            for c in cs:
                nc.scalar.dma_start(
                    ada_pp[:, c, :],
                    ada_dr[c * D:(c + 1) * D].rearrange("(k p) -> p k", p=128),
                )
            lo, hi = min(cs), max(cs) + 1
            nc.vector.tensor_add(ada_pp[:, lo:hi, :], ada_pp[:, lo:hi, :],
                                 bada_pp[:, lo:hi, :])
            for c in cs:
                if c in (1, 4):
                    nc.vector.tensor_scalar_add(ada_pp[:, c, :],
                                                ada_pp[:, c, :], 1.0)
                if c in (0, 1, 3, 4):
                    nc.vector.tensor_scalar_mul(ada_pp[:, c, :],
                                                ada_pp[:, c, :], AS)

        with tc.tile_pool(name="p1w", bufs=1) as p1w, \
             tc.tile_pool(name="pxin", bufs=3) as pxin, \
             tc.tile_pool(name="ps_pro", bufs=2, space="PSUM") as ps_pro, \
             tc.tile_pool(name="ps_tr", bufs=2, space="PSUM") as ps_tr:

            def emit_transpose_block(tt):
                xin = pxin.tile([128, D], F32, tag="xin", name="xin")
                nc.sync.dma_start(
                    xin[:, :], ins["x"][tt * 128:(tt + 1) * 128, :])
                for kd in range(KT):
                    pt = ps_tr.tile([128, 128], F32, tag="ptr", name="pt")
                    nc.tensor.transpose(
                        pt[:, :], xin[:, kd * 128:(kd + 1) * 128], ident[:, :])
                    tsl = slice(tt * 128, (tt + 1) * 128)
                    if kd % 2 == 0:
                        nc.vector.tensor_copy(xT[:, kd, tsl], pt[:, :])
                    else:
                        nc.scalar.copy(xT[:, kd, tsl], pt[:, :])

            for i in range(8):
                emit_transpose_block(i)
                if i == 0:
                    emit_bias_loads()
                if i < 6:
                    emit_ada_chunk(i, p1w, ps_pro)
            emit_ada_pp_load([0, 1])

        # ====== phase B part 1: qkv weight loads + converts (emitted before
        # LN1 so SP streams the loads while ada finishes / LN runs) =========
        es_qk = ExitStack()
        pqk8 = es_qk.enter_context(tc.tile_pool(name="pqk8", bufs=1))
        wq8 = pqk8.tile([128, KTP, D], FP8, name="wq8")
        wk8 = pqk8.tile([128, KTP, D], FP8, name="wk8")
        nc.gpsimd.memset(wq8[:, KT, :], 0.0)
        nc.gpsimd.memset(wk8[:, KT, :], 0.0)

        es_att = ExitStack()
        patt = es_att.enter_context(tc.tile_pool(name="patt", bufs=1, side="right"))
        attn_hs = patt.tile([72, H, NT], FP8, name="attn_hs")
        es_wp = ExitStack()
        pwp8 = es_wp.enter_context(
            tc.tile_pool(name="pwp8", bufs=1, side="right"))
        wp8 = pwp8.tile([72, H, D], FP8, name="wp8")
        es_va = ExitStack()
        pva = es_va.enter_context(tc.tile_pool(name="pva", bufs=1, side="right"))
        v_aug = pva.tile([128, NT // 128, H, 97], FP8, name="v_aug")
        nc.gpsimd.memset(v_aug[:, :, :, HD:97], 0.0)
        nc.gpsimd.memset(v_aug[:, :, :, 96:97], 1.0)

        es_b = ExitStack()
        pwst = es_b.enter_context(tc.tile_pool(name="pwst", bufs=1))
        wv8 = pwst.tile([128, KTP, D], FP8, tag="wv8", bufs=1, name="wv8")
        nc.gpsimd.memset(wv8[:, KT, :], 0.0)
        engs = ["act", "dve", "act", "dve", "act", "dve"]
        for j, (dst8, c0) in enumerate(((wq8, 0), (wk8, D), (wv8, 2 * D))):
            for half in range(2):
                msl = slice(half * 576, (half + 1) * 576)
                wst = pwst.tile([128, KT, 576], F32, tag="wst", bufs=2,
                                name="wst")
                nc.sync.dma_start(
                    wst[:, :, :],
                    ins["w_qkv"][:, c0 + half * 576:c0 + (half + 1) * 576]
                    .rearrange("(k p) m -> p k m", p=128),
                )
                eng = engs[j * 2 + half]
                for kk in range(3):
                    ksl = slice(kk * 3, kk * 3 + 3)
                    if eng == "act":
                        nc.scalar.activation(
                            dst8[:, ksl, msl], wst[:, ksl, :],
                            AF.Identity, scale=WS)
                    elif eng == "dve":
                        nc.vector.tensor_scalar_mul(
                            dst8[:, ksl, msl], wst[:, ksl, :], WS)
                    else:
                        nc.gpsimd.tensor_scalar_mul(
                            dst8[:, ksl, msl], wst[:, ksl, :], WS)

        # ====== LN1 (per-half, interleaved with v matmuls) ==================
        with tc.tile_pool(name="pst", bufs=1) as pst, \
             tc.tile_pool(name="pln", bufs=1) as pln, \
             tc.tile_pool(name="ps_st", bufs=4, space="PSUM") as ps_st, \
             tc.tile_pool(name="ps_v", bufs=3, space="PSUM") as ps_v:

            def v_block(tts):
                for tt in tts:
                    tsl = slice(tt * 128, (tt + 1) * 128)
                    for si, (c0, c1, h0, h1) in enumerate(V_SLICES):
                        pmv = ps_v.tile([128, 512], F32, tag="mv", name="pmv")
                        for i in range(KTP // 2):
                            nc.tensor.matmul(
                                pmv[:, 0:c1 - c0],
                                mod12T[:, 2 * i:2 * i + 2, tsl],
                                wv8[:, 2 * i:2 * i + 2, c0:c1],
                                start=(i == 0), stop=False, perf_mode=DR,
                                skip_group_check=True,
                            )
                        nc.tensor.matmul(
                            pmv[:, 0:c1 - c0], ones_row[:, :],
                            bv_b[:, c0:c1],
                            start=False, stop=True, skip_group_check=True,
                        )
                        vsrc = pmv[:, 0:c1 - c0].rearrange(
                            "p (h d) -> p h d", d=HD)
                        nc.vector.tensor_scalar_mul(
                            v_aug[:, tt, h0:h1, 0:HD], vsrc, 1.0 / (AS * WS))

            st1 = {}
            _ln_stats(tc, nc, xT, ones_col, pst, pln, ps_st, halves=(0,),
                      st=st1)
            _ln_apply(tc, nc, xT, mod12T, st1, ada_pp, 0, 1, pln, halves=(0,))
            _ln_stats(tc, nc, xT, ones_col, pst, pln, ps_st, halves=(1,),
                      st=st1)
            v_block(range(0, 4))
            _ln_apply(tc, nc, xT, mod12T, st1, ada_pp, 0, 1, pln, halves=(1,))
            v_block(range(4, 8))
        es_b.close()

        # ================= phase C: attention ===============================
        with tc.tile_pool(name="p3w", bufs=1) as p3w, \
             tc.tile_pool(name="pexp", bufs=1) as pexp, \
             tc.tile_pool(name="pat3", bufs=1) as pat3, \
             tc.tile_pool(name="ps_qk", bufs=2, space="PSUM") as ps_qk, \
             tc.tile_pool(name="ps_s", bufs=2, space="PSUM") as ps_s, \
             tc.tile_pool(name="ps_av", bufs=1, space="PSUM") as ps_av, \
             tc.tile_pool(name="ps_pa", bufs=1, space="PSUM") as ps_pa:

            def emit_fc1_stream(j):
                f1st = p3w.tile([128, KT, 256], F32, tag="f1st",
                                bufs=2, name="f1st")
                nc.sync.dma_start(
                    f1st[:, :, :],
                    ins["w_fc1"][:, j * 256:(j + 1) * 256]
                    .rearrange("(k p) m -> p k m", p=128),
                )
                f18o = p3w.tile([128, KT, 256], FP8, tag="f18o",
                                bufs=2, name="f18o")
                nc.gpsimd.tensor_scalar_mul(
                    f18o[:, :, :], f1st[:, :, :], WS)
                nc.scalar.dma_start(w1f8_dr[j, :, :, :], f18o[:, :, :])

            def emit_wp_stream(c):
                # reuse the f1st staging tag: [128, KT*256] bytes == 16*144
                wpt = p3w.tile([128, KT, 256], F32, tag="f1st", bufs=2,
                               name="wpt")
                wpv = (wpt[:, :, :].rearrange("p k m -> p (k m)")[0:72, :]
                       .rearrange("p (h m) -> p h m", h=H))
                msl = slice(c * 144, (c + 1) * 144)
                nc.sync.dma_start(
                    wpv[:, :, :],
                    ins["w_proj"][:, msl].rearrange("(h p) m -> p h m", p=72),
                )
                nc.vector.tensor_scalar_mul(
                    wp8[:, :, msl], wpv[:, :, :], WS)

            def emit_wp_stream(c):
                # reuse the f1st staging tag: KT*256 f32 bytes == 16*144
                wpt = p3w.tile([128, KT, 256], F32, tag="f1st", bufs=2,
                               name="wpt")
                wpv = (wpt[:, :, :].rearrange("p k m -> p (k m)")[0:72, :]
                       .rearrange("p (h m) -> p h m", h=H))
                msl = slice(c * 144, (c + 1) * 144)
                nc.sync.dma_start(
                    wpv[:, :, :],
                    ins["w_proj"][:, msl].rearrange("(h p) m -> p h m", p=72),
                )
                nc.vector.tensor_scalar_mul(
                    wp8[:, :, msl], wpv[:, :, :], WS)

            def emit_filler(h):
                # late ada chunks; fc1 fp8 stream-convert to DRAM
                if h % 4 != 3:
                    emit_ada_chunk(6 + h - h // 4, p3w, ps_pa, ps_bufs=1)
                if h == 15:
                    emit_ada_pp_load([2, 3])
                    emit_ada_pp_load([4, 5])
                if 2 <= h:
                    js = ([2 * h - 4, 2 * h - 3] if h < 6
                          else [h + 2])
                    for j in js:
                        emit_fc1_stream(j)
                if h >= 12:
                    emit_wp_stream(h - 12)

            for h in range(H):
                emit_filler(h)
                q_h = pat3.tile([72, NT], FP8, tag="qh", bufs=2, name="q_h")
                k_h = pat3.tile([72, NT], FP8, tag="kh", bufs=2, name="k_h")
                for n in range(2):
                    nsl = slice(n * 512, (n + 1) * 512)
                    pq = ps_qk.tile([72, 512], F32, tag="qk", name="pq")
                    for i in range(KTP // 2):
                        nc.tensor.matmul(
                            pq[:, :],
                            wq8[:, 2 * i:2 * i + 2, h * HD:(h + 1) * HD],
                            mod12T[:, 2 * i:2 * i + 2, nsl],
                            start=(i == 0), stop=(i == KTP // 2 - 1),
                            perf_mode=DR,
                        )
                    nc.vector.tensor_scalar(
                        q_h[:, nsl], pq[:, :], QS / (AS * WS),
                        bq_s[:, h:h + 1], ALU.mult, ALU.add,
                    )
                for n in range(2):
                    nsl = slice(n * 512, (n + 1) * 512)
                    pk = ps_qk.tile([72, 512], F32, tag="qk", name="pk")
                    for i in range(KTP // 2):
                        nc.tensor.matmul(
                            pk[:, :],
                            wk8[:, 2 * i:2 * i + 2, h * HD:(h + 1) * HD],
                            mod12T[:, 2 * i:2 * i + 2, nsl],
                            start=(i == 0), stop=(i == KTP // 2 - 1),
                            perf_mode=DR,
                        )
                    nc.vector.tensor_scalar(
                        k_h[:, nsl], pk[:, :], QS / (AS * WS),
                        bk_s[:, h:h + 1], ALU.mult, ALU.add,
                    )
                for n in range(2):
                    nsl = slice(n * 512, (n + 1) * 512)
                    exp_hn = pexp.tile([128, NT // 128, 512], FP8, tag="exp",
                                       bufs=3, name="exp_hn")
                    for kp in range(NT // 256):
                        pss = ps_s.tile([128, 2, 512], F32, tag="s",
                                        name="pss")
                        for j in range(2):
                            kt_i = 2 * kp + j
                            nc.tensor.matmul(
                                pss[:, j, :],
                                k_h[:, kt_i * 128:(kt_i + 1) * 128],
                                q_h[:, nsl], start=True, stop=True,
                            )
                        nc.scalar.activation(
                            exp_hn[:, 2 * kp:2 * kp + 2, :],
                            pss[:, :, :], AF.Exp, scale=ES)
                    pav = ps_av.tile([97, 512], F32, tag="av", name="pav")
                    for i in range(NT // 256):
                        nc.tensor.matmul(
                            pav[:, :],
                            v_aug[:, 2 * i:2 * i + 2, h, :],
                            exp_hn[:, 2 * i:2 * i + 2, :],
                            start=(i == 0), stop=(i == NT // 256 - 1),
                            perf_mode=DR,
                        )
                    den = pat3.tile([1, 512], F32, tag="den", bufs=2,
                                    name="den")
                    nc.vector.tensor_scalar_mul(den[:, :], pav[96:97, :],
                                                1.0 / PS)
                    nc.vector.reciprocal(den[:, :], den[:, :])
                    denB = pat3.tile([72, 512], F32, tag="denB", bufs=2,
                                     name="denB")
                    nc.gpsimd.partition_broadcast(denB[:, :], den[:, :])
                    nc.vector.tensor_mul(
                        attn_hs[:, h, nsl], pav[0:HD, :], denB[:, :])
        es_qk.close()  # wq8/wk8 no longer needed
        es_va.close()

        # ================= phase D: proj + residual + LN2 ===================
        es_w2 = ExitStack()
        pw2 = es_w2.enter_context(
            tc.tile_pool(name="pw2", bufs=1, side="right"))
        w2f8 = pw2.tile([128, KT, MH, 128], FP8, name="w2f8")

        with tc.tile_pool(name="p4", bufs=1) as p4, \
             tc.tile_pool(name="pst4", bufs=1) as pst4, \
             tc.tile_pool(name="pln4", bufs=1) as pln4:

            for i in range(4, 8):
                msl = slice(i * 144, (i + 1) * 144)
                wpst = p4.tile([72, H, 144], F32, tag="wpst", bufs=2,
                               name="wpst")
                nc.sync.dma_start(
                    wpst[:, :, :],
                    ins["w_proj"][:, msl].rearrange("(h p) m -> p h m", p=72),
                )
                for kk in range(2):
                    hsl = slice(kk * 8, kk * 8 + 8)
                    nc.vector.tensor_scalar_mul(
                        wp8[:, hsl, msl], wpst[:, hsl, :], WS)

            def emit_fc2_chunk(ch, eng, pool):
                f2s = pool.tile([128, MH, 64], F32, tag="f2s", bufs=2,
                                name="f2s")
                nc.sync.dma_start(
                    f2s[:, :, :],
                    ins["w_fc2"][:, ch * 64:(ch + 1) * 64]
                    .rearrange("(k p) m -> p k m", p=128),
                )
                eng.tensor_scalar_mul(
                    w2f8[:, ch // 2, :, (ch % 2) * 64:(ch % 2 + 1) * 64],
                    f2s[:, :, :], WS)

            st2 = {}
            with tc.tile_pool(name="ps_mm2", bufs=3, space="PSUM") as ps_mm2, \
                 tc.tile_pool(name="ps_st2", bufs=4, space="PSUM") as ps_st2:
                for n in range(2):
                    nsl = slice(n * 512, (n + 1) * 512)
                    for mo in range(KT):
                        if mo < 4:
                            ch = n * 4 + mo
                            eng = nc.vector if ch % 2 else nc.gpsimd
                            emit_fc2_chunk(ch, eng, p4)
                        pm2 = ps_mm2.tile([128, 512], F32, tag="mm2",
                                          name="pm2")
                        for i in range(H // 2):
                            nc.tensor.matmul(
                                pm2[:, :],
                                wp8[:, 2 * i:2 * i + 2,
                                    mo * 128:(mo + 1) * 128],
                                attn_hs[:, 2 * i:2 * i + 2, nsl],
                                start=(i == 0), stop=(i == H // 2 - 1),
                                perf_mode=DR,
                            )
                        t_sb = p4.tile([128, 512], F32, tag="tsb", bufs=2,
                                       name="t_sb")
                        nc.scalar.activation(
                            t_sb[:, :], pm2[:, :], AF.Identity,
                            bias=bproj_pp[:, mo:mo + 1], scale=1.0 / (PS * WS),
                        )
                        nc.vector.scalar_tensor_tensor(
                            xT[:, mo, nsl], t_sb[:, :],
                            ada_pp[:, 2, mo:mo + 1], xT[:, mo, nsl],
                            ALU.mult, ALU.add,
                        )
                    _ln_stats(tc, nc, xT, ones_col, pst4, pln4, ps_st2,
                              halves=(n,), st=st2)
                    _ln_apply(tc, nc, xT, mod12T, st2, ada_pp, 3, 4, pln4,
                              halves=(n,))

        # ================= phase E: FFN =====================================
        es_e = ExitStack()
        ph = es_e.enter_context(tc.tile_pool(name="ph", bufs=1))
        hT = ph.tile([128, MH, NT], FP8, name="hT")
        po = es_e.enter_context(tc.tile_pool(name="po", bufs=1))

        with tc.tile_pool(name="ps_f1", bufs=3, space="PSUM") as ps_f1, \
             tc.tile_pool(name="ps_f2", bufs=2, space="PSUM") as ps_f2, \
             tc.tile_pool(name="ps_tro", bufs=2, space="PSUM") as ps_tro:
            # fc1 in 18 chunks of 256 columns (2 m-tiles each), weights
            # already converted to fp8 in DRAM during the attention window
            with tc.tile_pool(name="p5a", bufs=1) as p5a:
                for ch in range(18):
                    f18 = p5a.tile([128, KTP, 256], FP8, tag="f18", bufs=3,
                                   name="f18")
                    nc.sync.dma_start(f18[:, 0:KT, :], w1f8_dr[ch, :, :, :])
                    nc.gpsimd.memset(f18[:, KT, :], 0.0)
                    if ch < 10:
                        emit_fc2_chunk(8 + ch,
                                       nc.vector if ch % 2 else nc.gpsimd,
                                       p5a)
                    for m in range(2):
                        mo = ch * 2 + m
                        for n in range(2):
                            nsl = slice(n * 512, (n + 1) * 512)
                            pf1 = ps_f1.tile([128, 512], F32, tag="f1",
                                             name="pf1")
                            for i in range(KTP // 2):
                                nc.tensor.matmul(
                                    pf1[:, :],
                                    f18[:, 2 * i:2 * i + 2,
                                        m * 128:(m + 1) * 128],
                                    mod12T[:, 2 * i:2 * i + 2, nsl],
                                    start=(i == 0), stop=(i == KTP // 2 - 1),
                                    perf_mode=DR,
                                )
                            nc.scalar.activation(
                                hT[:, mo, nsl], pf1[:, :], AF.Gelu_apprx_tanh,
                                bias=bfc1_pp[:, mo:mo + 1],
                                scale=1.0 / (AS * WS),
                            )
            # fc2: weights already fp8-resident in SBUF (w2f8)
            with tc.tile_pool(name="p5b", bufs=1) as p5b:
                for mo in range(KT):
                    for n in range(2):
                        nsl = slice(n * 512, (n + 1) * 512)
                        pf2 = ps_f2.tile([128, 512], F32, tag="f2", name="pf2")
                        for i in range(MH // 2):
                            nc.tensor.matmul(
                                pf2[:, :], w2f8[:, mo, 2 * i:2 * i + 2, :],
                                hT[:, 2 * i:2 * i + 2, nsl],
                                start=(i == 0), stop=(i == MH // 2 - 1),
                                perf_mode=DR,
                            )
                        t2 = p5b.tile([128, 512], F32, tag="t2", bufs=3,
                                      name="t2")
                        nc.scalar.activation(
                            t2[:, :], pf2[:, :], AF.Identity,
                            bias=bfc2_pp[:, mo:mo + 1], scale=1.0 / WS,
                        )
                        nc.vector.scalar_tensor_tensor(
                            xT[:, mo, nsl], t2[:, :], ada_pp[:, 5, mo:mo + 1],
                            xT[:, mo, nsl], ALU.mult, ALU.add,
                        )
                    o_slab = po.tile([128, NT // 128, 128], F32, tag="osl",
                                     bufs=2, name="o_slab")
                    for tt in range(NT // 128):
                        pt = ps_tro.tile([128, 128], F32, tag="tro",
                                         name="pt6")
                        nc.tensor.transpose(
                            pt[:, :], xT[:, mo, tt * 128:(tt + 1) * 128],
                            ident[:, :],
                        )
                        dst = o_slab[:, tt, :]
                        nc.vector.tensor_copy(dst, pt[:, :])
                    nc.scalar.dma_start(
                        out_dram[:, mo * 128:(mo + 1) * 128]
                        .rearrange("(t p) m -> p t m", p=128),
                        o_slab[:, :, :])
        es_w2.close()
        es_wp.close()
        es_att.close()
        es_e.close()


_LOCK = threading.Lock()
_PROG = None


def _get_program():
    global _PROG
    with _LOCK:
        if _PROG is None:
            _PROG = _build_program()
    return _PROG


def _make_in_maps(inputs):
    arrs = {k: np.ascontiguousarray(np.asarray(v, dtype=np.float32))
            for k, v in inputs.items()}
    in_maps = []
    for c in range(NCORES):
        m = {k: v for k, v in arrs.items() if k not in ("x", "t_emb")}
        m["x"] = np.ascontiguousarray(arrs["x"][c])
        m["t_emb"] = np.ascontiguousarray(arrs["t_emb"][c])
        in_maps.append(m)
    return in_maps


def kernel(**inputs):
    nc = _get_program()
    res = run_bass_kernel_spmd(nc, _make_in_maps(inputs),
                               core_ids=list(range(NCORES)))
    return np.stack([r["out"] for r in res.results], axis=0)


def kernel_traced(inputs, **kw):
    """test-harness helper: returns full BassKernelResults with trace."""
    nc = _get_program()
    return run_bass_kernel_spmd(
        nc, _make_in_maps(inputs), core_ids=list(range(NCORES)), trace=True,
        **kw
    )


# revision 89
# speedup vs baseline: 1.5982x; 1.0006x over previous
"""DiT block kernel for Trainium2 (Bass/Tile), 8-core data parallel.

Shapes (hardcoded from the problem spec):
  x: (8, 1024, 1152), t_emb: (8, 1152)
  w_qkv (1152, 3456), w_proj (1152, 1152), w_fc1 (1152, 4608),
  w_fc2 (4608, 1152), w_ada (1152, 6912) + biases.

Strategy: batch-parallel across 8 cores (one batch element each, no
collectives). Activations live transposed [D on partitions, tokens free].
The large matmuls (qkv, attention AV, proj, fc1, fc2) run in fp8e4 with
DoubleRow perf mode (two 128-row k-tiles contracted per instruction);
scale factors for fp8 range are folded into the existing activation
bias/scale stages so no extra elementwise work is added.  LayerNorm
statistics reduce over the partition axis via ones-vector f32r matmuls;
softmax runs transposed (keys on partitions) with denominators collected
through a ones-column appended to V and a fused divide.  q/k are produced
per-head directly (M=72 matmuls cost the same per column as M=128), so
attention needs no partition-crossing gather DMAs.  Weights stream
through big staged f32 DMA loads (few, large transfers) and are
converted on-chip; ada (error-sensitive) stays f32r.
"""

import threading
from contextlib import ExitStack

import numpy as np

import concourse.bass as bass
import concourse.mybir as mybir
import concourse.tile as tile
from concourse import bacc
from concourse.bass_utils import run_bass_kernel_spmd
from concourse.masks import make_identity

F32 = mybir.dt.float32
F32R = mybir.dt.float32r
BF16 = mybir.dt.bfloat16
FP8 = mybir.dt.float8e4
AF = mybir.ActivationFunctionType
ALU = mybir.AluOpType
DR = mybir.MatmulPerfMode.DoubleRow

NCORES = 8
D = 1152
NT = 1024
KT = D // 128       # 9
KTP = KT + 1        # padded to even for DoubleRow pairs
H = 16
HD = 72
HID = 4 * D
MH = HID // 128     # 36
EPS = 1e-6
ISC = 1.0 / float(np.sqrt(HD))

# fp8 scale factors
WS = 64.0           # weights
AS = 8.0            # modulated activations (mod1/mod2)
QS = 2.0            # q/k
PS = 4.0            # attention output
ES = ISC / (QS * QS)  # exp() input scale applied to the scores psum

# v output column slices aligned to head boundaries
V_SLICES = [(0, 432, 0, 6), (432, 864, 6, 12), (864, 1152, 12, 16)]


def _r(ap):
    return ap.bitcast(F32R)


def _build_program():
    nc = bacc.Bacc(
        "TRN2", target_bir_lowering=False, debug=False, enable_asserts=False
    )
    ins = {}
    ins["x"] = nc.dram_tensor("x", [NT, D], F32, kind="ExternalInput").ap()
    ins["t_emb"] = nc.dram_tensor("t_emb", [D], F32, kind="ExternalInput").ap()
    for name, shape in [
        ("w_qkv", [D, 3 * D]), ("b_qkv", [3 * D]),
        ("w_proj", [D, D]), ("b_proj", [D]),
        ("w_fc1", [D, HID]), ("b_fc1", [HID]),
        ("w_fc2", [HID, D]), ("b_fc2", [D]),
        ("w_ada", [D, 6 * D]), ("b_ada", [6 * D]),
    ]:
        ins[name] = nc.dram_tensor(name, shape, F32, kind="ExternalInput").ap()
    out_dram = nc.dram_tensor("out", [NT, D], F32, kind="ExternalOutput").ap()

    with tile.TileContext(nc) as tc:
        _body(tc, ins, out_dram)
    nc.compile()
    return nc


def _ln_stats(tc, nc, src, ones_col, pst, pln, ps_st, halves=(0, 1),
              st=None):
    """Return st[n] = [mean; rstd] rows [1, 2, 512] per 512-token half,
    reducing over the partition (D) axis of src [128, KT, NT] f32."""
    ps_x, ps_q = {}, {}
    if st is None:
        st = {}
    for n in halves:
        nsl = slice(n * 512, (n + 1) * 512)
        ps_x[n] = ps_st.tile([1, 512], F32, tag="st", name=f"psx{n}")
        ps_q[n] = ps_st.tile([1, 512], F32, tag="st", name=f"psq{n}")
        for k in range(KT):
            xb = pln.tile([128, 512], BF16, tag="xb", bufs=2, name="xb")
            nc.scalar.copy(xb[:, :], src[:, k, nsl])
            sq = pln.tile([128, 512], BF16, tag="sq", bufs=2, name="sq")
            nc.vector.tensor_mul(sq[:, :], src[:, k, nsl], src[:, k, nsl])
            nc.tensor.matmul(
                ps_x[n][:, :], ones_col[:, :], xb[:, :],
                start=(k == 0), stop=(k == KT - 1), skip_group_check=True,
            )
            nc.tensor.matmul(
                ps_q[n][:, :], ones_col[:, :], sq[:, :],
                start=(k == 0), stop=(k == KT - 1), skip_group_check=True,
            )
    eps_sb = pst.tile([1, 1], F32, tag="eps", bufs=1, name="eps_sb")
    nc.vector.memset(eps_sb[:, :], EPS)
    for n in halves:
        st[n] = pst.tile([1, 2, 512], F32, tag="lnst", bufs=2, name=f"st{n}")
        nc.vector.tensor_scalar_mul(st[n][:, 0, :], ps_x[n][:, :], 1.0 / D)
        work = pst.tile([1, 512], F32, tag="lnwork", bufs=2, name="work")
        nc.vector.tensor_mul(work[:, :], st[n][:, 0, :], st[n][:, 0, :])
        nc.vector.scalar_tensor_tensor(
            st[n][:, 1, :], ps_q[n][:, :], 1.0 / D, work[:, :],
            ALU.mult, ALU.subtract,
        )
        nc.scalar.activation(st[n][:, 1, :], st[n][:, 1, :], AF.Sqrt,
                             bias=eps_sb[:, :], scale=1.0)
        nc.vector.reciprocal(st[n][:, 1, :], st[n][:, 1, :])
    return st


def _ln_apply(tc, nc, src, dst, st, ada_pp, sh_c, sc_c, pln,
              halves=(0, 1)):
    """dst[:,k,nsl] (fp8) = ((src-mean)*rstd) * ada[sc_c] + ada[sh_c]
    (ada params pre-scaled by AS)."""
    for n in halves:
        nsl = slice(n * 512, (n + 1) * 512)
        meanB = pln.tile([128, 512], F32, tag="meanB", bufs=2, name="meanB")
        rstdB = pln.tile([128, 512], F32, tag="rstdB", bufs=2, name="rstdB")
        nc.gpsimd.partition_broadcast(meanB[:, :], st[n][:, 0, :])
        nc.gpsimd.partition_broadcast(rstdB[:, :], st[n][:, 1, :])
        for k in range(KT):
            t1 = pln.tile([128, 512], F32, tag="lnt1", bufs=3, name="t1")
            nc.vector.tensor_sub(t1[:, :], src[:, k, nsl], meanB[:, :])
            nc.vector.tensor_mul(t1[:, :], t1[:, :], rstdB[:, :])
            nc.gpsimd.tensor_scalar(
                dst[:, k, nsl], t1[:, :],
                ada_pp[:, sc_c, k:k + 1], ada_pp[:, sh_c, k:k + 1],
                ALU.mult, ALU.add,
            )


def _body(tc, ins, out_dram):
    nc = tc.nc
    ctx = ExitStack()
    with ctx:
        dram = ctx.enter_context(tc.tile_pool(name="dram", bufs=1, space="DRAM"))
        ada_dr = dram.tile([6 * D], F32)
        w1f8_dr = dram.tile([18, 128, KT, 256], FP8)

        pers = ctx.enter_context(tc.tile_pool(name="pers", bufs=1))
        ident = pers.tile([128, 128], F32)
        make_identity(nc, ident[:, :])
        ones_col = pers.tile([128, 1], BF16)
        nc.vector.memset(ones_col[:, :], 1.0)
        ones_row = pers.tile([1, 128], BF16)
        nc.vector.memset(ones_row[:, :], 1.0)

        t_pp = pers.tile([128, KT], F32)
        nc.sync.dma_start(t_pp[:, :], ins["t_emb"].rearrange("(k p) -> p k", p=128))
        t_pr = pers.tile([128, KT], F32R)
        nc.scalar.activation(t_pr[:, :], t_pp[:, :], AF.Silu)

        bq_s = pers.tile([72, H], F32)
        bk_s = pers.tile([72, H], F32)
        bv_row = pers.tile([1, D], F32)
        bv_b = pers.tile([1, D], BF16)
        bproj_pp = pers.tile([128, KT], F32)
        bfc1_pp = pers.tile([128, MH], F32)
        bfc2_pp = pers.tile([128, KT], F32)
        bada_pp = pers.tile([128, 6, KT], F32)
        ada_pp = pers.tile([128, 6, KT], F32)

        def emit_bias_loads():
            nc.sync.dma_start(
                bq_s[:, :], ins["b_qkv"][0:D].rearrange("(h p) -> p h", p=72))
            nc.sync.dma_start(
                bk_s[:, :], ins["b_qkv"][D:2 * D].rearrange("(h p) -> p h", p=72))
            nc.sync.dma_start(
                bv_row[:, :],
                ins["b_qkv"][2 * D:3 * D].rearrange("(a b) -> a b", a=1))
            # bv enters the v accumulation in (AS*WS)-scaled psum units
            nc.vector.tensor_scalar_mul(bv_b[:, :], bv_row[:, :], AS * WS)
            nc.sync.dma_start(
                bproj_pp[:, :], ins["b_proj"].rearrange("(m p) -> p m", p=128))
            nc.sync.dma_start(
                bfc1_pp[:, :], ins["b_fc1"].rearrange("(m p) -> p m", p=128))
            nc.sync.dma_start(
                bfc2_pp[:, :], ins["b_fc2"].rearrange("(m p) -> p m", p=128))
            nc.sync.dma_start(
                bada_pp[:, :, :],
                ins["b_ada"].rearrange("(c k p) -> p c k", k=KT, p=128))
            # pre-scale q/k biases by QS (folded into the psum->fp8 copies)
            nc.vector.tensor_scalar_mul(bq_s[:, :], bq_s[:, :], QS)
            nc.vector.tensor_scalar_mul(bk_s[:, :], bk_s[:, :], QS)

        xT = pers.tile([128, KT, NT], F32)      # becomes x2T after residual 1
        mod12T = pers.tile([128, KTP, NT], FP8)  # mod1T, later reused as mod2T
        nc.gpsimd.memset(mod12T[:, KT, :], 0.0)  # DoubleRow pad k-tile

        # ================= phase A: x load/transpose, ada, LN1 ==============

        def emit_ada_chunk(c, p1w, ps_pro, ps_bufs=2):
            """chunk c covers w_ada cols [c*384, (c+1)*384); param p=c//3."""
            wst = p1w.tile([128, KT, 384], F32R, tag="adast", bufs=2, name="wst")
            nc.sync.dma_start(
                wst[:, :, :],
                ins["w_ada"][:, c * 384:(c + 1) * 384]
                .rearrange("(k p) m -> p k m", p=128).bitcast(F32R),
            )
            pa = ps_pro.tile([1, 384], F32, tag="psada", bufs=ps_bufs,
                             name="pa")
            for k in range(KT):
                nc.tensor.matmul(
                    pa[:, :], t_pr[:, k:k + 1], wst[:, k, :],
                    start=(k == 0), stop=(k == KT - 1),
                )
            asb = p1w.tile([1, 384], F32, tag="asb", bufs=3, name="asb")
            nc.vector.tensor_copy(asb[:, :], pa[:, :])
            nc.scalar.dma_start(
                ada_dr[c * 384:(c + 1) * 384].rearrange("(a b) -> a b", a=1),
                asb[0:1, :],
            )

        def emit_ada_pp_load(cs):
            """Load+finalize ada params cs (list) into ada_pp; params 0/1
            (shift_a/scale_a) and 3/4 are pre-scaled by AS; 1/4 get +1."""
            for c in cs:
                nc.scalar.dma_start(
                    ada_pp[:, c, :],
                    ada_dr[c * D:(c + 1) * D].rearrange("(k p) -> p k", p=128),
                )
            lo, hi = min(cs), max(cs) + 1
            nc.vector.tensor_add(ada_pp[:, lo:hi, :], ada_pp[:, lo:hi, :],
                                 bada_pp[:, lo:hi, :])
            for c in cs:
                if c in (1, 4):
                    nc.vector.tensor_scalar_add(ada_pp[:, c, :],
                                                ada_pp[:, c, :], 1.0)
                if c in (0, 1, 3, 4):
                    nc.vector.tensor_scalar_mul(ada_pp[:, c, :],
                                                ada_pp[:, c, :], AS)

        with tc.tile_pool(name="p1w", bufs=1) as p1w, \
             tc.tile_pool(name="pxin", bufs=3) as pxin, \
             tc.tile_pool(name="ps_pro", bufs=2, space="PSUM") as ps_pro, \
             tc.tile_pool(name="ps_tr", bufs=2, space="PSUM") as ps_tr:

            def emit_transpose_block(tt):
                xin = pxin.tile([128, D], F32, tag="xin", name="xin")
                nc.sync.dma_start(
                    xin[:, :], ins["x"][tt * 128:(tt + 1) * 128, :])
                for kd in range(KT):
                    pt = ps_tr.tile([128, 128], F32, tag="ptr", name="pt")
                    nc.tensor.transpose(
                        pt[:, :], xin[:, kd * 128:(kd + 1) * 128], ident[:, :])
                    tsl = slice(tt * 128, (tt + 1) * 128)
                    if kd % 2 == 0:
                        nc.vector.tensor_copy(xT[:, kd, tsl], pt[:, :])
                    else:
                        nc.scalar.copy(xT[:, kd, tsl], pt[:, :])

            for i in range(8):
                emit_transpose_block(i)
                if i == 0:
                    emit_bias_loads()
                if i < 6:
                    emit_ada_chunk(i, p1w, ps_pro)
            emit_ada_pp_load([0, 1])

        # ====== phase B part 1: qkv weight loads + converts (emitted before
        # LN1 so SP streams the loads while ada finishes / LN runs) =========
        es_qk = ExitStack()
        pqk8 = es_qk.enter_context(tc.tile_pool(name="pqk8", bufs=1))
        wq8 = pqk8.tile([128, KTP, D], FP8, name="wq8")
        wk8 = pqk8.tile([128, KTP, D], FP8, name="wk8")
        nc.gpsimd.memset(wq8[:, KT, :], 0.0)
        nc.gpsimd.memset(wk8[:, KT, :], 0.0)

        es_att = ExitStack()
        patt = es_att.enter_context(tc.tile_pool(name="patt", bufs=1, side="right"))
        attn_hs = patt.tile([72, H, NT], FP8, name="attn_hs")
        es_wp = ExitStack()
        pwp8 = es_wp.enter_context(
            tc.tile_pool(name="pwp8", bufs=1, side="right"))
        wp8 = pwp8.tile([72, H, D], FP8, name="wp8")
        es_va = ExitStack()
        pva = es_va.enter_context(tc.tile_pool(name="pva", bufs=1, side="right"))
        v_aug = pva.tile([128, NT // 128, H, 97], FP8, name="v_aug")
        nc.gpsimd.memset(v_aug[:, :, :, HD:97], 0.0)
        nc.gpsimd.memset(v_aug[:, :, :, 96:97], 1.0)

        es_b = ExitStack()
        pwst = es_b.enter_context(tc.tile_pool(name="pwst", bufs=1))
        wv8 = pwst.tile([128, KTP, D], FP8, tag="wv8", bufs=1, name="wv8")
        nc.gpsimd.memset(wv8[:, KT, :], 0.0)
        engs = ["act", "dve", "act", "dve", "act", "dve"]
        for j, (dst8, c0) in enumerate(((wq8, 0), (wk8, D), (wv8, 2 * D))):
            for half in range(2):
                msl = slice(half * 576, (half + 1) * 576)
                wst = pwst.tile([128, KT, 576], F32, tag="wst", bufs=2,
                                name="wst")
                nc.sync.dma_start(
                    wst[:, :, :],
                    ins["w_qkv"][:, c0 + half * 576:c0 + (half + 1) * 576]
                    .rearrange("(k p) m -> p k m", p=128),
                )
                eng = engs[j * 2 + half]
                for kk in range(3):
                    ksl = slice(kk * 3, kk * 3 + 3)
                    if eng == "act":
                        nc.scalar.activation(
                            dst8[:, ksl, msl], wst[:, ksl, :],
                            AF.Identity, scale=WS)
                    elif eng == "dve":
                        nc.vector.tensor_scalar_mul(
                            dst8[:, ksl, msl], wst[:, ksl, :], WS)
                    else:
                        nc.gpsimd.tensor_scalar_mul(
                            dst8[:, ksl, msl], wst[:, ksl, :], WS)

        # ====== LN1 (per-half, interleaved with v matmuls) ==================
        with tc.tile_pool(name="pst", bufs=1) as pst, \
             tc.tile_pool(name="pln", bufs=1) as pln, \
             tc.tile_pool(name="ps_st", bufs=4, space="PSUM") as ps_st, \
             tc.tile_pool(name="ps_v", bufs=3, space="PSUM") as ps_v:

            def v_block(tts):
                for tt in tts:
                    tsl = slice(tt * 128, (tt + 1) * 128)
                    for si, (c0, c1, h0, h1) in enumerate(V_SLICES):
                        pmv = ps_v.tile([128, 512], F32, tag="mv", name="pmv")
                        for i in range(KTP // 2):
                            nc.tensor.matmul(
                                pmv[:, 0:c1 - c0],
                                mod12T[:, 2 * i:2 * i + 2, tsl],
                                wv8[:, 2 * i:2 * i + 2, c0:c1],
                                start=(i == 0), stop=False, perf_mode=DR,
                                skip_group_check=True,
                            )
                        nc.tensor.matmul(
                            pmv[:, 0:c1 - c0], ones_row[:, :],
                            bv_b[:, c0:c1],
                            start=False, stop=True, skip_group_check=True,
                        )
                        vsrc = pmv[:, 0:c1 - c0].rearrange(
                            "p (h d) -> p h d", d=HD)
                        nc.vector.tensor_scalar_mul(
                            v_aug[:, tt, h0:h1, 0:HD], vsrc, 1.0 / (AS * WS))

            st1 = {}
            _ln_stats(tc, nc, xT, ones_col, pst, pln, ps_st, halves=(0,),
                      st=st1)
            _ln_apply(tc, nc, xT, mod12T, st1, ada_pp, 0, 1, pln, halves=(0,))
            _ln_stats(tc, nc, xT, ones_col, pst, pln, ps_st, halves=(1,),
                      st=st1)
            v_block(range(0, 4))
            _ln_apply(tc, nc, xT, mod12T, st1, ada_pp, 0, 1, pln, halves=(1,))
            v_block(range(4, 8))
        es_b.close()

        # ================= phase C: attention ===============================
        with tc.tile_pool(name="p3w", bufs=1) as p3w, \
             tc.tile_pool(name="pexp", bufs=1) as pexp, \
             tc.tile_pool(name="pat3", bufs=1) as pat3, \
             tc.tile_pool(name="ps_qk", bufs=2, space="PSUM") as ps_qk, \
             tc.tile_pool(name="ps_s", bufs=2, space="PSUM") as ps_s, \
             tc.tile_pool(name="ps_av", bufs=1, space="PSUM") as ps_av, \
             tc.tile_pool(name="ps_pa", bufs=1, space="PSUM") as ps_pa:

            def emit_fc1_stream(j):
                f1st = p3w.tile([128, KT, 256], F32, tag="f1st",
                                bufs=2, name="f1st")
                nc.sync.dma_start(
                    f1st[:, :, :],
                    ins["w_fc1"][:, j * 256:(j + 1) * 256]
                    .rearrange("(k p) m -> p k m", p=128),
                )
                f18o = p3w.tile([128, KT, 256], FP8, tag="f18o",
                                bufs=2, name="f18o")
                nc.gpsimd.tensor_scalar_mul(
                    f18o[:, :, :], f1st[:, :, :], WS)
                nc.scalar.dma_start(w1f8_dr[j, :, :, :], f18o[:, :, :])

            def emit_wp_stream(c):
                # reuse the f1st staging tag: [128, KT*256] bytes == 16*144
                wpt = p3w.tile([128, KT, 256], F32, tag="f1st", bufs=2,
                               name="wpt")
                wpv = (wpt[:, :, :].rearrange("p k m -> p (k m)")[0:72, :]
                       .rearrange("p (h m) -> p h m", h=H))
                msl = slice(c * 144, (c + 1) * 144)
                nc.sync.dma_start(
                    wpv[:, :, :],
                    ins["w_proj"][:, msl].rearrange("(h p) m -> p h m", p=72),
                )
                nc.vector.tensor_scalar_mul(
                    wp8[:, :, msl], wpv[:, :, :], WS)

            def emit_wp_stream(c):
                # reuse the f1st staging tag: KT*256 f32 bytes == 16*144
                wpt = p3w.tile([128, KT, 256], F32, tag="f1st", bufs=2,
                               name="wpt")
                wpv = (wpt[:, :, :].rearrange("p k m -> p (k m)")[0:72, :]
                       .rearrange("p (h m) -> p h m", h=H))
                msl = slice(c * 144, (c + 1) * 144)
                nc.sync.dma_start(
                    wpv[:, :, :],
                    ins["w_proj"][:, msl].rearrange("(h p) m -> p h m", p=72),
                )
                nc.vector.tensor_scalar_mul(
                    wp8[:, :, msl], wpv[:, :, :], WS)

            def emit_filler(h):
                # late ada chunks; fc1 fp8 stream-convert to DRAM
                if h % 4 != 3:
                    emit_ada_chunk(6 + h - h // 4, p3w, ps_pa, ps_bufs=1)
                if h == 15:
                    emit_ada_pp_load([2, 3])
                    emit_ada_pp_load([4, 5])
                if 2 <= h:
                    js = ([2 * h - 4, 2 * h - 3] if h < 6
                          else [h + 2])
                    for j in js:
                        emit_fc1_stream(j)
                if h >= 12:
                    emit_wp_stream(h - 12)

            for h in range(H):
                emit_filler(h)
                q_h = pat3.tile([72, NT], FP8, tag="qh", bufs=2, name="q_h")
                k_h = pat3.tile([72, NT], FP8, tag="kh", bufs=2, name="k_h")
                for n in range(2):
                    nsl = slice(n * 512, (n + 1) * 512)
                    pq = ps_qk.tile([72, 512], F32, tag="qk", name="pq")
                    for i in range(KTP // 2):
                        nc.tensor.matmul(
                            pq[:, :],
                            wq8[:, 2 * i:2 * i + 2, h * HD:(h + 1) * HD],
                            mod12T[:, 2 * i:2 * i + 2, nsl],
                            start=(i == 0), stop=(i == KTP // 2 - 1),
                            perf_mode=DR,
                        )
                    nc.vector.tensor_scalar(
                        q_h[:, nsl], pq[:, :], QS / (AS * WS),
                        bq_s[:, h:h + 1], ALU.mult, ALU.add,
                    )
                for n in range(2):
                    nsl = slice(n * 512, (n + 1) * 512)
                    pk = ps_qk.tile([72, 512], F32, tag="qk", name="pk")
                    for i in range(KTP // 2):
                        nc.tensor.matmul(
                            pk[:, :],
                            wk8[:, 2 * i:2 * i + 2, h * HD:(h + 1) * HD],
                            mod12T[:, 2 * i:2 * i + 2, nsl],
                            start=(i == 0), stop=(i == KTP // 2 - 1),
                            perf_mode=DR,
                        )
                    nc.vector.tensor_scalar(
                        k_h[:, nsl], pk[:, :], QS / (AS * WS),
                        bk_s[:, h:h + 1], ALU.mult, ALU.add,
                    )
                for n in range(2):
                    nsl = slice(n * 512, (n + 1) * 512)
                    exp_hn = pexp.tile([128, NT // 128, 512], FP8, tag="exp",
                                       bufs=3, name="exp_hn")
                    for kp in range(NT // 256):
                        pss = ps_s.tile([128, 2, 512], F32, tag="s",
                                        name="pss")
                        for j in range(2):
                            kt_i = 2 * kp + j
                            nc.tensor.matmul(
                                pss[:, j, :],
                                k_h[:, kt_i * 128:(kt_i + 1) * 128],
                                q_h[:, nsl], start=True, stop=True,
                            )
                        nc.scalar.activation(
                            exp_hn[:, 2 * kp:2 * kp + 2, :],
                            pss[:, :, :], AF.Exp, scale=ES)
                    pav = ps_av.tile([97, 512], F32, tag="av", name="pav")
                    for i in range(NT // 256):
                        nc.tensor.matmul(
                            pav[:, :],
                            v_aug[:, 2 * i:2 * i + 2, h, :],
                            exp_hn[:, 2 * i:2 * i + 2, :],
                            start=(i == 0), stop=(i == NT // 256 - 1),
                            perf_mode=DR,
                        )
                    den = pat3.tile([1, 512], F32, tag="den", bufs=3,
                                    name="den")
                    nc.vector.tensor_scalar_mul(den[:, :], pav[96:97, :],
                                                1.0 / PS)
                    nc.vector.reciprocal(den[:, :], den[:, :])
                    denB = pat3.tile([72, 512], F32, tag="denB", bufs=3,
                                     name="denB")
                    nc.gpsimd.partition_broadcast(denB[:, :], den[:, :])
                    nc.vector.tensor_mul(
                        attn_hs[:, h, nsl], pav[0:HD, :], denB[:, :])
        es_qk.close()  # wq8/wk8 no longer needed
        es_va.close()

        # ================= phase D: proj + residual + LN2 ===================
        es_w2 = ExitStack()
        pw2 = es_w2.enter_context(
            tc.tile_pool(name="pw2", bufs=1, side="right"))
        w2f8 = pw2.tile([128, KT, MH, 128], FP8, name="w2f8")

        with tc.tile_pool(name="p4", bufs=1) as p4, \
             tc.tile_pool(name="pst4", bufs=1) as pst4, \
             tc.tile_pool(name="pln4", bufs=1) as pln4:

            for i in range(4, 8):
                msl = slice(i * 144, (i + 1) * 144)
                wpst = p4.tile([72, H, 144], F32, tag="wpst", bufs=2,
                               name="wpst")
                nc.sync.dma_start(
                    wpst[:, :, :],
                    ins["w_proj"][:, msl].rearrange("(h p) m -> p h m", p=72),
                )
                for kk in range(2):
                    hsl = slice(kk * 8, kk * 8 + 8)
                    nc.vector.tensor_scalar_mul(
                        wp8[:, hsl, msl], wpst[:, hsl, :], WS)

            def emit_fc2_chunk(ch, eng, pool):
                f2s = pool.tile([128, MH, 64], F32, tag="f2s", bufs=2,
                                name="f2s")
                nc.sync.dma_start(
                    f2s[:, :, :],
                    ins["w_fc2"][:, ch * 64:(ch + 1) * 64]
                    .rearrange("(k p) m -> p k m", p=128),
                )
                eng.tensor_scalar_mul(
                    w2f8[:, ch // 2, :, (ch % 2) * 64:(ch % 2 + 1) * 64],
                    f2s[:, :, :], WS)

            st2 = {}
            with tc.tile_pool(name="ps_mm2", bufs=3, space="PSUM") as ps_mm2, \
                 tc.tile_pool(name="ps_st2", bufs=4, space="PSUM") as ps_st2:
                for n in range(2):
                    nsl = slice(n * 512, (n + 1) * 512)
                    for mo in range(KT):
                        if mo < 4:
                            ch = n * 4 + mo
                            eng = nc.vector if ch % 2 else nc.gpsimd
                            emit_fc2_chunk(ch, eng, p4)
                        pm2 = ps_mm2.tile([128, 512], F32, tag="mm2",
                                          name="pm2")
                        for i in range(H // 2):
                            nc.tensor.matmul(
                                pm2[:, :],
                                wp8[:, 2 * i:2 * i + 2,
                                    mo * 128:(mo + 1) * 128],
                                attn_hs[:, 2 * i:2 * i + 2, nsl],
                                start=(i == 0), stop=(i == H // 2 - 1),
                                perf_mode=DR,
                            )
                        t_sb = p4.tile([128, 512], F32, tag="tsb", bufs=2,
                                       name="t_sb")
                        nc.scalar.activation(
                            t_sb[:, :], pm2[:, :], AF.Identity,
                            bias=bproj_pp[:, mo:mo + 1], scale=1.0 / (PS * WS),
                        )
                        nc.vector.scalar_tensor_tensor(
                            xT[:, mo, nsl], t_sb[:, :],
                            ada_pp[:, 2, mo:mo + 1], xT[:, mo, nsl],
                            ALU.mult, ALU.add,
                        )
                    _ln_stats(tc, nc, xT, ones_col, pst4, pln4, ps_st2,
                              halves=(n,), st=st2)
                    _ln_apply(tc, nc, xT, mod12T, st2, ada_pp, 3, 4, pln4,
                              halves=(n,))

        # ================= phase E: FFN =====================================
        es_e = ExitStack()
        ph = es_e.enter_context(tc.tile_pool(name="ph", bufs=1))
        hT = ph.tile([128, MH, NT], FP8, name="hT")
        po = es_e.enter_context(tc.tile_pool(name="po", bufs=1))

        with tc.tile_pool(name="ps_f1", bufs=3, space="PSUM") as ps_f1, \
             tc.tile_pool(name="ps_f2", bufs=2, space="PSUM") as ps_f2, \
             tc.tile_pool(name="ps_tro", bufs=2, space="PSUM") as ps_tro:
            # fc1 in 18 chunks of 256 columns (2 m-tiles each), weights
            # already converted to fp8 in DRAM during the attention window
            with tc.tile_pool(name="p5a", bufs=1) as p5a:
                for ch in range(18):
                    f18 = p5a.tile([128, KTP, 256], FP8, tag="f18", bufs=3,
                                   name="f18")
                    nc.sync.dma_start(f18[:, 0:KT, :], w1f8_dr[ch, :, :, :])
                    nc.gpsimd.memset(f18[:, KT, :], 0.0)
                    if ch < 10:
                        emit_fc2_chunk(8 + ch,
                                       nc.vector if ch % 2 else nc.gpsimd,
                                       p5a)
                    for m in range(2):
                        mo = ch * 2 + m
                        for n in range(2):
                            nsl = slice(n * 512, (n + 1) * 512)
                            pf1 = ps_f1.tile([128, 512], F32, tag="f1",
                                             name="pf1")
                            for i in range(KTP // 2):
                                nc.tensor.matmul(
                                    pf1[:, :],
                                    f18[:, 2 * i:2 * i + 2,
                                        m * 128:(m + 1) * 128],
                                    mod12T[:, 2 * i:2 * i + 2, nsl],
                                    start=(i == 0), stop=(i == KTP // 2 - 1),
                                    perf_mode=DR,
                                )
                            nc.scalar.activation(
                                hT[:, mo, nsl], pf1[:, :], AF.Gelu_apprx_tanh,
                                bias=bfc1_pp[:, mo:mo + 1],
                                scale=1.0 / (AS * WS),
                            )
            # fc2: weights already fp8-resident in SBUF (w2f8)
            with tc.tile_pool(name="p5b", bufs=1) as p5b:
                for mo in range(KT):
                    for n in range(2):
                        nsl = slice(n * 512, (n + 1) * 512)
                        pf2 = ps_f2.tile([128, 512], F32, tag="f2", name="pf2")
                        for i in range(MH // 2):
                            nc.tensor.matmul(
                                pf2[:, :], w2f8[:, mo, 2 * i:2 * i + 2, :],
                                hT[:, 2 * i:2 * i + 2, nsl],
                                start=(i == 0), stop=(i == MH // 2 - 1),
                                perf_mode=DR,
                            )
                        t2 = p5b.tile([128, 512], F32, tag="t2", bufs=3,
                                      name="t2")
                        nc.scalar.activation(
                            t2[:, :], pf2[:, :], AF.Identity,
                            bias=bfc2_pp[:, mo:mo + 1], scale=1.0 / WS,
                        )
                        nc.vector.scalar_tensor_tensor(
                            xT[:, mo, nsl], t2[:, :], ada_pp[:, 5, mo:mo + 1],
                            xT[:, mo, nsl], ALU.mult, ALU.add,
                        )
                    o_slab = po.tile([128, NT // 128, 128], F32, tag="osl",
                                     bufs=2, name="o_slab")
                    for tt in range(NT // 128):
                        pt = ps_tro.tile([128, 128], F32, tag="tro",
                                         name="pt6")
                        nc.tensor.transpose(
                            pt[:, :], xT[:, mo, tt * 128:(tt + 1) * 128],
                            ident[:, :],
                        )
                        dst = o_slab[:, tt, :]
                        nc.vector.tensor_copy(dst, pt[:, :])
                    nc.scalar.dma_start(
                        out_dram[:, mo * 128:(mo + 1) * 128]
                        .rearrange("(t p) m -> p t m", p=128),
                        o_slab[:, :, :])
        es_w2.close()
        es_wp.close()
        es_att.close()
        es_e.close()


_LOCK = threading.Lock()
_PROG = None


def _get_program():
    global _PROG
    with _LOCK:
        if _PROG is None:
            _PROG = _build_program()
    return _PROG


def _make_in_maps(inputs):
    arrs = {k: np.ascontiguousarray(np.asarray(v, dtype=np.float32))
            for k, v in inputs.items()}
    in_maps = []
    for c in range(NCORES):
        m = {k: v for k, v in arrs.items() if k not in ("x", "t_emb")}
        m["x"] = np.ascontiguousarray(arrs["x"][c])
        m["t_emb"] = np.ascontiguousarray(arrs["t_emb"][c])
        in_maps.append(m)
    return in_maps


def kernel(**inputs):
    nc = _get_program()
    res = run_bass_kernel_spmd(nc, _make_in_maps(inputs),
                               core_ids=list(range(NCORES)))
    return np.stack([r["out"] for r in res.results], axis=0)


def kernel_traced(inputs, **kw):
    """test-harness helper: returns full BassKernelResults with trace."""
    nc = _get_program()
    return run_bass_kernel_spmd(
        nc, _make_in_maps(inputs), core_ids=list(range(NCORES)), trace=True,
        **kw
    )


# revision 96
# speedup vs baseline: 1.6023x; 1.0026x over previous
"""DiT block kernel for Trainium2 (Bass/Tile), 8-core data parallel.

Shapes (hardcoded from the problem spec):
  x: (8, 1024, 1152), t_emb: (8, 1152)
  w_qkv (1152, 3456), w_proj (1152, 1152), w_fc1 (1152, 4608),
  w_fc2 (4608, 1152), w_ada (1152, 6912) + biases.

Strategy: batch-parallel across 8 cores (one batch element each, no
collectives). Activations live transposed [D on partitions, tokens free].
The large matmuls (qkv, attention AV, proj, fc1, fc2) run in fp8e4 with
DoubleRow perf mode (two 128-row k-tiles contracted per instruction);
scale factors for fp8 range are folded into the existing activation
bias/scale stages so no extra elementwise work is added.  LayerNorm
statistics reduce over the partition axis via ones-vector f32r matmuls;
softmax runs transposed (keys on partitions) with denominators collected
through a ones-column appended to V and a fused divide.  q/k are produced
per-head directly (M=72 matmuls cost the same per column as M=128), so
attention needs no partition-crossing gather DMAs.  Weights stream
through big staged f32 DMA loads (few, large transfers) and are
converted on-chip; ada (error-sensitive) stays f32r.
"""

import threading
from contextlib import ExitStack

import numpy as np

import concourse.bass as bass
import concourse.mybir as mybir
import concourse.tile as tile
from concourse import bacc
from concourse.bass_utils import run_bass_kernel_spmd
from concourse.masks import make_identity

F32 = mybir.dt.float32
F32R = mybir.dt.float32r
BF16 = mybir.dt.bfloat16
FP8 = mybir.dt.float8e4
AF = mybir.ActivationFunctionType
ALU = mybir.AluOpType
DR = mybir.MatmulPerfMode.DoubleRow

NCORES = 8
D = 1152
NT = 1024
KT = D // 128       # 9
KTP = KT + 1        # padded to even for DoubleRow pairs
H = 16
HD = 72
HID = 4 * D
MH = HID // 128     # 36
EPS = 1e-6
ISC = 1.0 / float(np.sqrt(HD))

# fp8 scale factors
WS = 64.0           # weights
AS = 8.0            # modulated activations (mod1/mod2)
QS = 2.0            # q/k
PS = 4.0            # attention output
ES = ISC / (QS * QS)  # exp() input scale applied to the scores psum

# v output column slices aligned to head boundaries
V_SLICES = [(0, 432, 0, 6), (432, 864, 6, 12), (864, 1152, 12, 16)]


def _r(ap):
    return ap.bitcast(F32R)


def _build_program():
    nc = bacc.Bacc(
        "TRN2", target_bir_lowering=False, debug=False, enable_asserts=False
    )
    ins = {}
    ins["x"] = nc.dram_tensor("x", [NT, D], F32, kind="ExternalInput").ap()
    ins["t_emb"] = nc.dram_tensor("t_emb", [D], F32, kind="ExternalInput").ap()
    for name, shape in [
        ("w_qkv", [D, 3 * D]), ("b_qkv", [3 * D]),
        ("w_proj", [D, D]), ("b_proj", [D]),
        ("w_fc1", [D, HID]), ("b_fc1", [HID]),
        ("w_fc2", [HID, D]), ("b_fc2", [D]),
        ("w_ada", [D, 6 * D]), ("b_ada", [6 * D]),
    ]:
        ins[name] = nc.dram_tensor(name, shape, F32, kind="ExternalInput").ap()
    out_dram = nc.dram_tensor("out", [NT, D], F32, kind="ExternalOutput").ap()

    with tile.TileContext(nc) as tc:
        _body(tc, ins, out_dram)
    nc.compile()
    return nc


def _ln_stats(tc, nc, src, ones_col, pst, pln, ps_st, halves=(0, 1),
              st=None):
    """Return st[n] = [mean; rstd] rows [1, 2, 512] per 512-token half,
    reducing over the partition (D) axis of src [128, KT, NT] f32."""
    ps_x, ps_q = {}, {}
    if st is None:
        st = {}
    for n in halves:
        nsl = slice(n * 512, (n + 1) * 512)
        ps_x[n] = ps_st.tile([1, 512], F32, tag="st", name=f"psx{n}")
        ps_q[n] = ps_st.tile([1, 512], F32, tag="st", name=f"psq{n}")
        for k in range(KT):
            xb = pln.tile([128, 512], BF16, tag="xb", bufs=2, name="xb")
            nc.scalar.copy(xb[:, :], src[:, k, nsl])
            sq = pln.tile([128, 512], BF16, tag="sq", bufs=2, name="sq")
            nc.vector.tensor_mul(sq[:, :], src[:, k, nsl], src[:, k, nsl])
            nc.tensor.matmul(
                ps_x[n][:, :], ones_col[:, :], xb[:, :],
                start=(k == 0), stop=(k == KT - 1), skip_group_check=True,
            )
            nc.tensor.matmul(
                ps_q[n][:, :], ones_col[:, :], sq[:, :],
                start=(k == 0), stop=(k == KT - 1), skip_group_check=True,
            )
    eps_sb = pst.tile([1, 1], F32, tag="eps", bufs=1, name="eps_sb")
    nc.vector.memset(eps_sb[:, :], EPS)
    for n in halves:
        st[n] = pst.tile([1, 2, 512], F32, tag="lnst", bufs=2, name=f"st{n}")
        nc.vector.tensor_scalar_mul(st[n][:, 0, :], ps_x[n][:, :], 1.0 / D)
        work = pst.tile([1, 512], F32, tag="lnwork", bufs=2, name="work")
        nc.vector.tensor_mul(work[:, :], st[n][:, 0, :], st[n][:, 0, :])
        nc.vector.scalar_tensor_tensor(
            st[n][:, 1, :], ps_q[n][:, :], 1.0 / D, work[:, :],
            ALU.mult, ALU.subtract,
        )
        nc.scalar.activation(st[n][:, 1, :], st[n][:, 1, :], AF.Sqrt,
                             bias=eps_sb[:, :], scale=1.0)
        nc.vector.reciprocal(st[n][:, 1, :], st[n][:, 1, :])
    return st


def _ln_apply(tc, nc, src, dst, st, ada_pp, sh_c, sc_c, pln,
              halves=(0, 1)):
    """dst[:,k,nsl] (fp8) = ((src-mean)*rstd) * ada[sc_c] + ada[sh_c]
    (ada params pre-scaled by AS)."""
    for n in halves:
        nsl = slice(n * 512, (n + 1) * 512)
        meanB = pln.tile([128, 512], F32, tag="meanB", bufs=2, name="meanB")
        rstdB = pln.tile([128, 512], F32, tag="rstdB", bufs=2, name="rstdB")
        nc.gpsimd.partition_broadcast(meanB[:, :], st[n][:, 0, :])
        nc.gpsimd.partition_broadcast(rstdB[:, :], st[n][:, 1, :])
        for k in range(KT):
            t1 = pln.tile([128, 512], F32, tag="lnt1", bufs=3, name="t1")
            nc.vector.tensor_sub(t1[:, :], src[:, k, nsl], meanB[:, :])
            nc.vector.tensor_mul(t1[:, :], t1[:, :], rstdB[:, :])
            nc.gpsimd.tensor_scalar(
                dst[:, k, nsl], t1[:, :],
                ada_pp[:, sc_c, k:k + 1], ada_pp[:, sh_c, k:k + 1],
                ALU.mult, ALU.add,
            )


def _body(tc, ins, out_dram):
    nc = tc.nc
    ctx = ExitStack()
    with ctx:
        dram = ctx.enter_context(tc.tile_pool(name="dram", bufs=1, space="DRAM"))
        ada_dr = dram.tile([6 * D], F32)
        w1f8_dr = dram.tile([18, 128, KT, 256], FP8)

        pers = ctx.enter_context(tc.tile_pool(name="pers", bufs=1))
        ident = pers.tile([128, 128], F32)
        make_identity(nc, ident[:, :])
        ones_col = pers.tile([128, 1], BF16)
        nc.vector.memset(ones_col[:, :], 1.0)
        ones_row = pers.tile([1, 128], BF16)
        nc.vector.memset(ones_row[:, :], 1.0)

        t_pp = pers.tile([128, KT], F32)
        nc.sync.dma_start(t_pp[:, :], ins["t_emb"].rearrange("(k p) -> p k", p=128))
        t_pr = pers.tile([128, KT], F32R)
        nc.scalar.activation(t_pr[:, :], t_pp[:, :], AF.Silu)

        bq_s = pers.tile([72, H], F32)
        bk_s = pers.tile([72, H], F32)
        bv_row = pers.tile([1, D], F32)
        bv_b = pers.tile([1, D], BF16)
        bproj_pp = pers.tile([128, KT], F32)
        bfc1_pp = pers.tile([128, MH], F32)
        bfc2_pp = pers.tile([128, KT], F32)
        bada_pp = pers.tile([128, 6, KT], F32)
        ada_pp = pers.tile([128, 6, KT], F32)

        def emit_bias_loads():
            nc.sync.dma_start(
                bq_s[:, :], ins["b_qkv"][0:D].rearrange("(h p) -> p h", p=72))
            nc.sync.dma_start(
                bk_s[:, :], ins["b_qkv"][D:2 * D].rearrange("(h p) -> p h", p=72))
            nc.sync.dma_start(
                bv_row[:, :],
                ins["b_qkv"][2 * D:3 * D].rearrange("(a b) -> a b", a=1))
            # bv enters the v accumulation in (AS*WS)-scaled psum units
            nc.vector.tensor_scalar_mul(bv_b[:, :], bv_row[:, :], AS * WS)
            nc.sync.dma_start(
                bproj_pp[:, :], ins["b_proj"].rearrange("(m p) -> p m", p=128))
            nc.sync.dma_start(
                bfc1_pp[:, :], ins["b_fc1"].rearrange("(m p) -> p m", p=128))
            nc.sync.dma_start(
                bfc2_pp[:, :], ins["b_fc2"].rearrange("(m p) -> p m", p=128))
            nc.sync.dma_start(
                bada_pp[:, :, :],
                ins["b_ada"].rearrange("(c k p) -> p c k", k=KT, p=128))
            # pre-scale q/k biases by QS (folded into the psum->fp8 copies)
            nc.vector.tensor_scalar_mul(bq_s[:, :], bq_s[:, :], QS)
            nc.vector.tensor_scalar_mul(bk_s[:, :], bk_s[:, :], QS)

        xT = pers.tile([128, KT, NT], F32)      # becomes x2T after residual 1
        mod12T = pers.tile([128, KTP, NT], FP8)  # mod1T, later reused as mod2T
        nc.gpsimd.memset(mod12T[:, KT, :], 0.0)  # DoubleRow pad k-tile

        # ================= phase A: x load/transpose, ada, LN1 ==============

        def emit_ada_chunk(c, p1w, ps_pro, ps_bufs=2):
            """chunk c covers w_ada cols [c*384, (c+1)*384); param p=c//3."""
            wst = p1w.tile([128, KT, 384], F32R, tag="adast", bufs=2, name="wst")
            nc.sync.dma_start(
                wst[:, :, :],
                ins["w_ada"][:, c * 384:(c + 1) * 384]
                .rearrange("(k p) m -> p k m", p=128).bitcast(F32R),
            )
            pa = ps_pro.tile([1, 384], F32, tag="psada", bufs=ps_bufs,
                             name="pa")
            for k in range(KT):
                nc.tensor.matmul(
                    pa[:, :], t_pr[:, k:k + 1], wst[:, k, :],
                    start=(k == 0), stop=(k == KT - 1),
                )
            asb = p1w.tile([1, 384], F32, tag="asb", bufs=3, name="asb")
            nc.vector.tensor_copy(asb[:, :], pa[:, :])
            nc.scalar.dma_start(
                ada_dr[c * 384:(c + 1) * 384].rearrange("(a b) -> a b", a=1),
                asb[0:1, :],
            )

        def emit_ada_pp_load(cs):
            """Load+finalize ada params cs (list) into ada_pp; params 0/1
            (shift_a/scale_a) and 3/4 are pre-scaled by AS; 1/4 get +1."""
            for c in cs:
                nc.scalar.dma_start(
                    ada_pp[:, c, :],
                    ada_dr[c * D:(c + 1) * D].rearrange("(k p) -> p k", p=128),
                )
            lo, hi = min(cs), max(cs) + 1
            nc.vector.tensor_add(ada_pp[:, lo:hi, :], ada_pp[:, lo:hi, :],
                                 bada_pp[:, lo:hi, :])
            for c in cs:
                if c in (1, 4):
                    nc.vector.tensor_scalar_add(ada_pp[:, c, :],
                                                ada_pp[:, c, :], 1.0)
                if c in (0, 1, 3, 4):
                    nc.vector.tensor_scalar_mul(ada_pp[:, c, :],
                                                ada_pp[:, c, :], AS)

        with tc.tile_pool(name="p1w", bufs=1) as p1w, \
             tc.tile_pool(name="pxin", bufs=3) as pxin, \
             tc.tile_pool(name="ps_pro", bufs=2, space="PSUM") as ps_pro, \
             tc.tile_pool(name="ps_tr", bufs=2, space="PSUM") as ps_tr:

            def emit_transpose_block(tt):
                xin = pxin.tile([128, D], F32, tag="xin", name="xin")
                nc.sync.dma_start(
                    xin[:, :], ins["x"][tt * 128:(tt + 1) * 128, :])
                for kd in range(KT):
                    pt = ps_tr.tile([128, 128], F32, tag="ptr", name="pt")
                    nc.tensor.transpose(
                        pt[:, :], xin[:, kd * 128:(kd + 1) * 128], ident[:, :])
                    tsl = slice(tt * 128, (tt + 1) * 128)
                    if kd % 2 == 0:
                        nc.vector.tensor_copy(xT[:, kd, tsl], pt[:, :])
                    else:
                        nc.scalar.copy(xT[:, kd, tsl], pt[:, :])

            for i in range(8):
                emit_transpose_block(i)
                if i == 0:
                    emit_bias_loads()
                if i < 6:
                    emit_ada_chunk(i, p1w, ps_pro)
            emit_ada_pp_load([0, 1])

        # ====== phase B part 1: qkv weight loads + converts (emitted before
        # LN1 so SP streams the loads while ada finishes / LN runs) =========
        es_qk = ExitStack()
        pqk8 = es_qk.enter_context(tc.tile_pool(name="pqk8", bufs=1))
        wq8 = pqk8.tile([128, KTP, D], FP8, name="wq8")
        wk8 = pqk8.tile([128, KTP, D], FP8, name="wk8")
        nc.gpsimd.memset(wq8[:, KT, :], 0.0)
        nc.gpsimd.memset(wk8[:, KT, :], 0.0)

        es_att = ExitStack()
        patt = es_att.enter_context(tc.tile_pool(name="patt", bufs=1, side="right"))
        attn_hs = patt.tile([72, H, NT], FP8, name="attn_hs")
        es_wp = ExitStack()
        pwp8 = es_wp.enter_context(
            tc.tile_pool(name="pwp8", bufs=1, side="right"))
        wp8 = pwp8.tile([72, H, D], FP8, name="wp8")
        es_va = ExitStack()
        pva = es_va.enter_context(tc.tile_pool(name="pva", bufs=1, side="right"))
        v_aug = pva.tile([128, NT // 128, H, 97], FP8, name="v_aug")
        nc.gpsimd.memset(v_aug[:, :, :, HD:97], 0.0)
        nc.gpsimd.memset(v_aug[:, :, :, 96:97], 1.0)

        es_b = ExitStack()
        pwst = es_b.enter_context(tc.tile_pool(name="pwst", bufs=1))
        wv8 = pwst.tile([128, KTP, D], FP8, tag="wv8", bufs=1, name="wv8")
        nc.gpsimd.memset(wv8[:, KT, :], 0.0)
        engs = ["act", "dve", "act", "dve", "act", "dve"]
        for j, (dst8, c0) in enumerate(((wq8, 0), (wk8, D), (wv8, 2 * D))):
            for half in range(2):
                msl = slice(half * 576, (half + 1) * 576)
                wst = pwst.tile([128, KT, 576], F32, tag="wst", bufs=2,
                                name="wst")
                nc.sync.dma_start(
                    wst[:, :, :],
                    ins["w_qkv"][:, c0 + half * 576:c0 + (half + 1) * 576]
                    .rearrange("(k p) m -> p k m", p=128),
                )
                eng = engs[j * 2 + half]
                for kk in range(3):
                    ksl = slice(kk * 3, kk * 3 + 3)
                    if eng == "act":
                        nc.scalar.activation(
                            dst8[:, ksl, msl], wst[:, ksl, :],
                            AF.Identity, scale=WS)
                    elif eng == "dve":
                        nc.vector.tensor_scalar_mul(
                            dst8[:, ksl, msl], wst[:, ksl, :], WS)
                    else:
                        nc.gpsimd.tensor_scalar_mul(
                            dst8[:, ksl, msl], wst[:, ksl, :], WS)

        # ====== LN1 (per-half, interleaved with v matmuls) ==================
        with tc.tile_pool(name="pst", bufs=1) as pst, \
             tc.tile_pool(name="pln", bufs=1) as pln, \
             tc.tile_pool(name="ps_st", bufs=4, space="PSUM") as ps_st, \
             tc.tile_pool(name="ps_v", bufs=3, space="PSUM") as ps_v:

            def v_block(tts):
                for tt in tts:
                    tsl = slice(tt * 128, (tt + 1) * 128)
                    for si, (c0, c1, h0, h1) in enumerate(V_SLICES):
                        pmv = ps_v.tile([128, 512], F32, tag="mv", name="pmv")
                        for i in range(KTP // 2):
                            nc.tensor.matmul(
                                pmv[:, 0:c1 - c0],
                                mod12T[:, 2 * i:2 * i + 2, tsl],
                                wv8[:, 2 * i:2 * i + 2, c0:c1],
                                start=(i == 0), stop=False, perf_mode=DR,
                                skip_group_check=True,
                            )
                        nc.tensor.matmul(
                            pmv[:, 0:c1 - c0], ones_row[:, :],
                            bv_b[:, c0:c1],
                            start=False, stop=True, skip_group_check=True,
                        )
                        vsrc = pmv[:, 0:c1 - c0].rearrange(
                            "p (h d) -> p h d", d=HD)
                        nc.vector.tensor_scalar_mul(
                            v_aug[:, tt, h0:h1, 0:HD], vsrc, 1.0 / (AS * WS))

            st1 = {}
            _ln_stats(tc, nc, xT, ones_col, pst, pln, ps_st, halves=(0,),
                      st=st1)
            _ln_apply(tc, nc, xT, mod12T, st1, ada_pp, 0, 1, pln, halves=(0,))
            _ln_stats(tc, nc, xT, ones_col, pst, pln, ps_st, halves=(1,),
                      st=st1)
            v_block(range(0, 4))
            _ln_apply(tc, nc, xT, mod12T, st1, ada_pp, 0, 1, pln, halves=(1,))
            v_block(range(4, 8))
        es_b.close()

        # ================= phase C: attention ===============================
        with tc.tile_pool(name="p3w", bufs=1) as p3w, \
             tc.tile_pool(name="pexp", bufs=1) as pexp, \
             tc.tile_pool(name="pat3", bufs=1) as pat3, \
             tc.tile_pool(name="ps_qk", bufs=2, space="PSUM") as ps_qk, \
             tc.tile_pool(name="ps_s", bufs=2, space="PSUM") as ps_s, \
             tc.tile_pool(name="ps_av", bufs=1, space="PSUM") as ps_av, \
             tc.tile_pool(name="ps_pa", bufs=1, space="PSUM") as ps_pa:

            def emit_fc1_stream(j):
                f1st = p3w.tile([128, KT, 256], F32, tag="f1st",
                                bufs=2, name="f1st")
                nc.sync.dma_start(
                    f1st[:, :, :],
                    ins["w_fc1"][:, j * 256:(j + 1) * 256]
                    .rearrange("(k p) m -> p k m", p=128),
                )
                f18o = p3w.tile([128, KT, 256], FP8, tag="f18o",
                                bufs=2, name="f18o")
                nc.gpsimd.tensor_scalar_mul(
                    f18o[:, :, :], f1st[:, :, :], WS)
                nc.scalar.dma_start(w1f8_dr[j, :, :, :], f18o[:, :, :])

            def emit_wp_stream(c):
                # reuse the f1st staging tag: [128, KT*256] bytes == 16*144
                wpt = p3w.tile([128, KT, 256], F32, tag="f1st", bufs=2,
                               name="wpt")
                wpv = (wpt[:, :, :].rearrange("p k m -> p (k m)")[0:72, :]
                       .rearrange("p (h m) -> p h m", h=H))
                msl = slice(c * 144, (c + 1) * 144)
                nc.sync.dma_start(
                    wpv[:, :, :],
                    ins["w_proj"][:, msl].rearrange("(h p) m -> p h m", p=72),
                )
                nc.vector.tensor_scalar_mul(
                    wp8[:, :, msl], wpv[:, :, :], WS)

            def emit_wp_stream(c):
                # reuse the f1st staging tag: KT*256 f32 bytes == 16*144
                wpt = p3w.tile([128, KT, 256], F32, tag="f1st", bufs=2,
                               name="wpt")
                wpv = (wpt[:, :, :].rearrange("p k m -> p (k m)")[0:72, :]
                       .rearrange("p (h m) -> p h m", h=H))
                msl = slice(c * 144, (c + 1) * 144)
                nc.sync.dma_start(
                    wpv[:, :, :],
                    ins["w_proj"][:, msl].rearrange("(h p) m -> p h m", p=72),
                )
                nc.vector.tensor_scalar_mul(
                    wp8[:, :, msl], wpv[:, :, :], WS)

            def emit_filler(h):
                # late ada chunks; fc1 fp8 stream-convert to DRAM
                if h % 4 != 3:
                    emit_ada_chunk(6 + h - h // 4, p3w, ps_pa, ps_bufs=1)
                if h == 15:
                    emit_ada_pp_load([2, 3])
                    emit_ada_pp_load([4, 5])
                if 2 <= h:
                    js = ([2 * h - 4, 2 * h - 3] if h < 6
                          else [h + 2])
                    for j in js:
                        emit_fc1_stream(j)
                if h >= 12:
                    emit_wp_stream(h - 12)

            for h in range(H):
                emit_filler(h)
                q_h = pat3.tile([72, NT], FP8, tag="qh", bufs=2, name="q_h")
                k_h = pat3.tile([72, NT], FP8, tag="kh", bufs=2, name="k_h")
                for n in range(2):
                    nsl = slice(n * 512, (n + 1) * 512)
                    pq = ps_qk.tile([72, 512], F32, tag="qk", name="pq")
                    for i in range(KTP // 2):
                        nc.tensor.matmul(
                            pq[:, :],
                            wq8[:, 2 * i:2 * i + 2, h * HD:(h + 1) * HD],
                            mod12T[:, 2 * i:2 * i + 2, nsl],
                            start=(i == 0), stop=(i == KTP // 2 - 1),
                            perf_mode=DR,
                        )
                    nc.vector.tensor_scalar(
                        q_h[:, nsl], pq[:, :], QS / (AS * WS),
                        bq_s[:, h:h + 1], ALU.mult, ALU.add,
                    )
                for n in range(2):
                    nsl = slice(n * 512, (n + 1) * 512)
                    pk = ps_qk.tile([72, 512], F32, tag="qk", name="pk")
                    for i in range(KTP // 2):
                        nc.tensor.matmul(
                            pk[:, :],
                            wk8[:, 2 * i:2 * i + 2, h * HD:(h + 1) * HD],
                            mod12T[:, 2 * i:2 * i + 2, nsl],
                            start=(i == 0), stop=(i == KTP // 2 - 1),
                            perf_mode=DR,
                        )
                    nc.vector.tensor_scalar(
                        k_h[:, nsl], pk[:, :], QS / (AS * WS),
                        bk_s[:, h:h + 1], ALU.mult, ALU.add,
                    )
                for n in range(2):
                    nsl = slice(n * 512, (n + 1) * 512)
                    exp_hn = pexp.tile([128, NT // 128, 512], FP8, tag="exp",
                                       bufs=3, name="exp_hn")
                    for kp in range(NT // 256):
                        pss = ps_s.tile([128, 2, 512], F32, tag="s",
                                        name="pss")
                        for j in range(2):
                            kt_i = 2 * kp + j
                            nc.tensor.matmul(
                                pss[:, j, :],
                                k_h[:, kt_i * 128:(kt_i + 1) * 128],
                                q_h[:, nsl], start=True, stop=True,
                            )
                        nc.scalar.activation(
                            exp_hn[:, 2 * kp:2 * kp + 2, :],
                            pss[:, :, :], AF.Exp, scale=ES)
                    pav = ps_av.tile([97, 512], F32, tag="av", name="pav")
                    for i in range(NT // 256):
                        nc.tensor.matmul(
                            pav[:, :],
                            v_aug[:, 2 * i:2 * i + 2, h, :],
                            exp_hn[:, 2 * i:2 * i + 2, :],
                            start=(i == 0), stop=(i == NT // 256 - 1),
                            perf_mode=DR,
                        )
                    den = pat3.tile([1, 512], F32, tag="den", bufs=3,
                                    name="den")
                    nc.vector.tensor_scalar_mul(den[:, :], pav[96:97, :],
                                                1.0 / PS)
                    nc.vector.reciprocal(den[:, :], den[:, :])
                    denB = pat3.tile([72, 512], F32, tag="denB", bufs=3,
                                     name="denB")
                    nc.gpsimd.partition_broadcast(denB[:, :], den[:, :])
                    nc.vector.tensor_mul(
                        attn_hs[:, h, nsl], pav[0:HD, :], denB[:, :])
        es_qk.close()  # wq8/wk8 no longer needed
        es_va.close()

        # ================= phase D: proj + residual + LN2 ===================
        es_w2 = ExitStack()
        pw2 = es_w2.enter_context(
            tc.tile_pool(name="pw2", bufs=1, side="right"))
        w2f8 = pw2.tile([128, KT, MH, 128], FP8, name="w2f8")

        with tc.tile_pool(name="p4", bufs=1) as p4, \
             tc.tile_pool(name="pst4", bufs=1) as pst4, \
             tc.tile_pool(name="pln4", bufs=1) as pln4:

            for i in range(4, 8):
                msl = slice(i * 144, (i + 1) * 144)
                wpst = p4.tile([72, H, 144], F32, tag="wpst", bufs=2,
                               name="wpst")
                nc.sync.dma_start(
                    wpst[:, :, :],
                    ins["w_proj"][:, msl].rearrange("(h p) m -> p h m", p=72),
                )
                for kk in range(2):
                    hsl = slice(kk * 8, kk * 8 + 8)
                    nc.vector.tensor_scalar_mul(
                        wp8[:, hsl, msl], wpst[:, hsl, :], WS)

            def emit_fc2_chunk(ch, eng, pool):
                f2s = pool.tile([128, MH, 64], F32, tag="f2s", bufs=2,
                                name="f2s")
                nc.sync.dma_start(
                    f2s[:, :, :],
                    ins["w_fc2"][:, ch * 64:(ch + 1) * 64]
                    .rearrange("(k p) m -> p k m", p=128),
                )
                eng.tensor_scalar_mul(
                    w2f8[:, ch // 2, :, (ch % 2) * 64:(ch % 2 + 1) * 64],
                    f2s[:, :, :], WS)

            st2 = {}
            with tc.tile_pool(name="ps_mm2", bufs=3, space="PSUM") as ps_mm2, \
                 tc.tile_pool(name="ps_st2", bufs=4, space="PSUM") as ps_st2:
                for n in range(2):
                    nsl = slice(n * 512, (n + 1) * 512)
                    for mo in range(KT):
                        if mo < 5:
                            ch = n * 5 + mo
                            eng = nc.vector if ch % 2 else nc.gpsimd
                            emit_fc2_chunk(ch, eng, p4)
                        pm2 = ps_mm2.tile([128, 512], F32, tag="mm2",
                                          name="pm2")
                        for i in range(H // 2):
                            nc.tensor.matmul(
                                pm2[:, :],
                                wp8[:, 2 * i:2 * i + 2,
                                    mo * 128:(mo + 1) * 128],
                                attn_hs[:, 2 * i:2 * i + 2, nsl],
                                start=(i == 0), stop=(i == H // 2 - 1),
                                perf_mode=DR,
                            )
                        t_sb = p4.tile([128, 512], F32, tag="tsb", bufs=2,
                                       name="t_sb")
                        nc.scalar.activation(
                            t_sb[:, :], pm2[:, :], AF.Identity,
                            bias=bproj_pp[:, mo:mo + 1], scale=1.0 / (PS * WS),
                        )
                        nc.vector.scalar_tensor_tensor(
                            xT[:, mo, nsl], t_sb[:, :],
                            ada_pp[:, 2, mo:mo + 1], xT[:, mo, nsl],
                            ALU.mult, ALU.add,
                        )
                    _ln_stats(tc, nc, xT, ones_col, pst4, pln4, ps_st2,
                              halves=(n,), st=st2)
                    _ln_apply(tc, nc, xT, mod12T, st2, ada_pp, 3, 4, pln4,
                              halves=(n,))

        # ================= phase E: FFN =====================================
        es_e = ExitStack()
        ph = es_e.enter_context(tc.tile_pool(name="ph", bufs=1))
        hT = ph.tile([128, MH, NT], FP8, name="hT")
        po = es_e.enter_context(tc.tile_pool(name="po", bufs=1))

        with tc.tile_pool(name="ps_f1", bufs=3, space="PSUM") as ps_f1, \
             tc.tile_pool(name="ps_f2", bufs=2, space="PSUM") as ps_f2, \
             tc.tile_pool(name="ps_tro", bufs=2, space="PSUM") as ps_tro:
            # fc1 in 18 chunks of 256 columns (2 m-tiles each), weights
            # already converted to fp8 in DRAM during the attention window
            with tc.tile_pool(name="p5a", bufs=1) as p5a:
                for ch in range(18):
                    f18 = p5a.tile([128, KTP, 256], FP8, tag="f18", bufs=3,
                                   name="f18")
                    nc.sync.dma_start(f18[:, 0:KT, :], w1f8_dr[ch, :, :, :])
                    nc.gpsimd.memset(f18[:, KT, :], 0.0)
                    if ch < 8:
                        emit_fc2_chunk(10 + ch,
                                       nc.vector if ch % 2 else nc.gpsimd,
                                       p5a)
                    for m in range(2):
                        mo = ch * 2 + m
                        for n in range(2):
                            nsl = slice(n * 512, (n + 1) * 512)
                            pf1 = ps_f1.tile([128, 512], F32, tag="f1",
                                             name="pf1")
                            for i in range(KTP // 2):
                                nc.tensor.matmul(
                                    pf1[:, :],
                                    f18[:, 2 * i:2 * i + 2,
                                        m * 128:(m + 1) * 128],
                                    mod12T[:, 2 * i:2 * i + 2, nsl],
                                    start=(i == 0), stop=(i == KTP // 2 - 1),
                                    perf_mode=DR,
                                )
                            nc.scalar.activation(
                                hT[:, mo, nsl], pf1[:, :], AF.Gelu_apprx_tanh,
                                bias=bfc1_pp[:, mo:mo + 1],
                                scale=1.0 / (AS * WS),
                            )
            # fc2: weights already fp8-resident in SBUF (w2f8)
            with tc.tile_pool(name="p5b", bufs=1) as p5b:
                for mo in range(KT):
                    for n in range(2):
                        nsl = slice(n * 512, (n + 1) * 512)
                        pf2 = ps_f2.tile([128, 512], F32, tag="f2", name="pf2")
                        for i in range(MH // 2):
                            nc.tensor.matmul(
                                pf2[:, :], w2f8[:, mo, 2 * i:2 * i + 2, :],
                                hT[:, 2 * i:2 * i + 2, nsl],
                                start=(i == 0), stop=(i == MH // 2 - 1),
                                perf_mode=DR,
                            )
                        t2 = p5b.tile([128, 512], F32, tag="t2", bufs=3,
                                      name="t2")
                        nc.scalar.activation(
                            t2[:, :], pf2[:, :], AF.Identity,
                            bias=bfc2_pp[:, mo:mo + 1], scale=1.0 / WS,
                        )
                        nc.vector.scalar_tensor_tensor(
                            xT[:, mo, nsl], t2[:, :], ada_pp[:, 5, mo:mo + 1],
                            xT[:, mo, nsl], ALU.mult, ALU.add,
                        )
                    o_slab = po.tile([128, NT // 128, 128], F32, tag="osl",
                                     bufs=2, name="o_slab")
                    for tt in range(NT // 128):
                        pt = ps_tro.tile([128, 128], F32, tag="tro",
                                         name="pt6")
                        nc.tensor.transpose(
                            pt[:, :], xT[:, mo, tt * 128:(tt + 1) * 128],
                            ident[:, :],
                        )
                        dst = o_slab[:, tt, :]
                        nc.vector.tensor_copy(dst, pt[:, :])
                    nc.scalar.dma_start(
                        out_dram[:, mo * 128:(mo + 1) * 128]
                        .rearrange("(t p) m -> p t m", p=128),
                        o_slab[:, :, :])
        es_w2.close()
        es_wp.close()
        es_att.close()
        es_e.close()


_LOCK = threading.Lock()
_PROG = None


def _get_program():
    global _PROG
    with _LOCK:
        if _PROG is None:
            _PROG = _build_program()
    return _PROG


def _make_in_maps(inputs):
    arrs = {k: np.ascontiguousarray(np.asarray(v, dtype=np.float32))
            for k, v in inputs.items()}
    in_maps = []
    for c in range(NCORES):
        m = {k: v for k, v in arrs.items() if k not in ("x", "t_emb")}
        m["x"] = np.ascontiguousarray(arrs["x"][c])
        m["t_emb"] = np.ascontiguousarray(arrs["t_emb"][c])
        in_maps.append(m)
    return in_maps


def kernel(**inputs):
    nc = _get_program()
    res = run_bass_kernel_spmd(nc, _make_in_maps(inputs),
                               core_ids=list(range(NCORES)))
    return np.stack([r["out"] for r in res.results], axis=0)


def kernel_traced(inputs, **kw):
    """test-harness helper: returns full BassKernelResults with trace."""
    nc = _get_program()
    return run_bass_kernel_spmd(
        nc, _make_in_maps(inputs), core_ids=list(range(NCORES)), trace=True,
        **kw
    )


# revision 103
# speedup vs baseline: 1.6065x; 1.0027x over previous
"""DiT block kernel for Trainium2 (Bass/Tile), 8-core data parallel.

Shapes (hardcoded from the problem spec):
  x: (8, 1024, 1152), t_emb: (8, 1152)
  w_qkv (1152, 3456), w_proj (1152, 1152), w_fc1 (1152, 4608),
  w_fc2 (4608, 1152), w_ada (1152, 6912) + biases.

Strategy: batch-parallel across 8 cores (one batch element each, no
collectives). Activations live transposed [D on partitions, tokens free].
The large matmuls (qkv, attention AV, proj, fc1, fc2) run in fp8e4 with
DoubleRow perf mode (two 128-row k-tiles contracted per instruction);
scale factors for fp8 range are folded into the existing activation
bias/scale stages so no extra elementwise work is added.  LayerNorm
statistics reduce over the partition axis via ones-vector f32r matmuls;
softmax runs transposed (keys on partitions) with denominators collected
through a ones-column appended to V and a fused divide.  q/k are produced
per-head directly (M=72 matmuls cost the same per column as M=128), so
attention needs no partition-crossing gather DMAs.  Weights stream
through big staged f32 DMA loads (few, large transfers) and are
converted on-chip; ada (error-sensitive) stays f32r.
"""

import threading
from contextlib import ExitStack

import numpy as np

import concourse.bass as bass
import concourse.mybir as mybir
import concourse.tile as tile
from concourse import bacc
from concourse.bass_utils import run_bass_kernel_spmd
from concourse.masks import make_identity

F32 = mybir.dt.float32
F32R = mybir.dt.float32r
BF16 = mybir.dt.bfloat16
FP8 = mybir.dt.float8e4
AF = mybir.ActivationFunctionType
ALU = mybir.AluOpType
DR = mybir.MatmulPerfMode.DoubleRow

NCORES = 8
D = 1152
NT = 1024
KT = D // 128       # 9
KTP = KT + 1        # padded to even for DoubleRow pairs
H = 16
HD = 72
HID = 4 * D
MH = HID // 128     # 36
EPS = 1e-6
ISC = 1.0 / float(np.sqrt(HD))

# fp8 scale factors
WS = 64.0           # weights
AS = 8.0            # modulated activations (mod1/mod2)
QS = 2.0            # q/k
PS = 4.0            # attention output
ES = ISC / (QS * QS)  # exp() input scale applied to the scores psum

# v output column slices aligned to head boundaries
V_SLICES = [(0, 432, 0, 6), (432, 864, 6, 12), (864, 1152, 12, 16)]


def _r(ap):
    return ap.bitcast(F32R)


def _build_program():
    nc = bacc.Bacc(
        "TRN2", target_bir_lowering=False, debug=False, enable_asserts=False
    )
    ins = {}
    ins["x"] = nc.dram_tensor("x", [NT, D], F32, kind="ExternalInput").ap()
    ins["t_emb"] = nc.dram_tensor("t_emb", [D], F32, kind="ExternalInput").ap()
    for name, shape in [
        ("w_qkv", [D, 3 * D]), ("b_qkv", [3 * D]),
        ("w_proj", [D, D]), ("b_proj", [D]),
        ("w_fc1", [D, HID]), ("b_fc1", [HID]),
        ("w_fc2", [HID, D]), ("b_fc2", [D]),
        ("w_ada", [D, 6 * D]), ("b_ada", [6 * D]),
    ]:
        ins[name] = nc.dram_tensor(name, shape, F32, kind="ExternalInput").ap()
    out_dram = nc.dram_tensor("out", [NT, D], F32, kind="ExternalOutput").ap()

    with tile.TileContext(nc) as tc:
        _body(tc, ins, out_dram)
    nc.compile()
    return nc


def _ln_stats(tc, nc, src, ones_col, pst, pln, ps_st, halves=(0, 1),
              st=None):
    """Return st[n] = [mean; rstd] rows [1, 2, 512] per 512-token half,
    reducing over the partition (D) axis of src [128, KT, NT] f32."""
    ps_x, ps_q = {}, {}
    if st is None:
        st = {}
    for n in halves:
        nsl = slice(n * 512, (n + 1) * 512)
        ps_x[n] = ps_st.tile([1, 512], F32, tag="st", name=f"psx{n}")
        ps_q[n] = ps_st.tile([1, 512], F32, tag="st", name=f"psq{n}")
        for k in range(KT):
            xb = pln.tile([128, 512], BF16, tag="xb", bufs=2, name="xb")
            nc.scalar.copy(xb[:, :], src[:, k, nsl])
            sq = pln.tile([128, 512], BF16, tag="sq", bufs=2, name="sq")
            nc.vector.tensor_mul(sq[:, :], src[:, k, nsl], src[:, k, nsl])
            nc.tensor.matmul(
                ps_x[n][:, :], ones_col[:, :], xb[:, :],
                start=(k == 0), stop=(k == KT - 1), skip_group_check=True,
            )
            nc.tensor.matmul(
                ps_q[n][:, :], ones_col[:, :], sq[:, :],
                start=(k == 0), stop=(k == KT - 1), skip_group_check=True,
            )
    eps_sb = pst.tile([1, 1], F32, tag="eps", bufs=1, name="eps_sb")
    nc.vector.memset(eps_sb[:, :], EPS)
    for n in halves:
        st[n] = pst.tile([1, 2, 512], F32, tag="lnst", bufs=2, name=f"st{n}")
        nc.vector.tensor_scalar_mul(st[n][:, 0, :], ps_x[n][:, :], 1.0 / D)
        work = pst.tile([1, 512], F32, tag="lnwork", bufs=2, name="work")
        nc.vector.tensor_mul(work[:, :], st[n][:, 0, :], st[n][:, 0, :])
        nc.vector.scalar_tensor_tensor(
            st[n][:, 1, :], ps_q[n][:, :], 1.0 / D, work[:, :],
            ALU.mult, ALU.subtract,
        )
        nc.scalar.activation(st[n][:, 1, :], st[n][:, 1, :], AF.Sqrt,
                             bias=eps_sb[:, :], scale=1.0)
        nc.vector.reciprocal(st[n][:, 1, :], st[n][:, 1, :])
    return st


def _ln_apply(tc, nc, src, dst, st, ada_pp, sh_c, sc_c, pln,
              halves=(0, 1)):
    """dst[:,k,nsl] (fp8) = ((src-mean)*rstd) * ada[sc_c] + ada[sh_c]
    (ada params pre-scaled by AS)."""
    for n in halves:
        nsl = slice(n * 512, (n + 1) * 512)
        meanB = pln.tile([128, 512], F32, tag="meanB", bufs=2, name="meanB")
        rstdB = pln.tile([128, 512], F32, tag="rstdB", bufs=2, name="rstdB")
        nc.gpsimd.partition_broadcast(meanB[:, :], st[n][:, 0, :])
        nc.gpsimd.partition_broadcast(rstdB[:, :], st[n][:, 1, :])
        for k in range(KT):
            t1 = pln.tile([128, 512], F32, tag="lnt1", bufs=3, name="t1")
            nc.vector.tensor_sub(t1[:, :], src[:, k, nsl], meanB[:, :])
            nc.vector.tensor_mul(t1[:, :], t1[:, :], rstdB[:, :])
            nc.gpsimd.tensor_scalar(
                dst[:, k, nsl], t1[:, :],
                ada_pp[:, sc_c, k:k + 1], ada_pp[:, sh_c, k:k + 1],
                ALU.mult, ALU.add,
            )


def _body(tc, ins, out_dram):
    nc = tc.nc
    ctx = ExitStack()
    with ctx:
        dram = ctx.enter_context(tc.tile_pool(name="dram", bufs=1, space="DRAM"))
        ada_dr = dram.tile([6 * D], F32)
        w1f8_dr = dram.tile([18, 128, KT, 256], FP8)

        pers = ctx.enter_context(tc.tile_pool(name="pers", bufs=1))
        ident = pers.tile([128, 128], F32)
        make_identity(nc, ident[:, :])
        ones_col = pers.tile([128, 1], BF16)
        nc.vector.memset(ones_col[:, :], 1.0)
        ones_row = pers.tile([1, 128], BF16)
        nc.vector.memset(ones_row[:, :], 1.0)

        t_pp = pers.tile([128, KT], F32)
        nc.sync.dma_start(t_pp[:, :], ins["t_emb"].rearrange("(k p) -> p k", p=128))
        t_pr = pers.tile([128, KT], F32R)
        nc.scalar.activation(t_pr[:, :], t_pp[:, :], AF.Silu)

        bq_s = pers.tile([72, H], F32)
        bk_s = pers.tile([72, H], F32)
        bv_row = pers.tile([1, D], F32)
        bv_b = pers.tile([1, D], BF16)
        bproj_pp = pers.tile([128, KT], F32)
        bfc1_pp = pers.tile([128, MH], F32)
        bfc2_pp = pers.tile([128, KT], F32)
        bada_pp = pers.tile([128, 6, KT], F32)
        ada_pp = pers.tile([128, 6, KT], F32)

        def emit_bias_loads():
            nc.sync.dma_start(
                bq_s[:, :], ins["b_qkv"][0:D].rearrange("(h p) -> p h", p=72))
            nc.sync.dma_start(
                bk_s[:, :], ins["b_qkv"][D:2 * D].rearrange("(h p) -> p h", p=72))
            nc.sync.dma_start(
                bv_row[:, :],
                ins["b_qkv"][2 * D:3 * D].rearrange("(a b) -> a b", a=1))
            # bv enters the v accumulation in (AS*WS)-scaled psum units
            nc.vector.tensor_scalar_mul(bv_b[:, :], bv_row[:, :], AS * WS)
            nc.sync.dma_start(
                bproj_pp[:, :], ins["b_proj"].rearrange("(m p) -> p m", p=128))
            nc.sync.dma_start(
                bfc1_pp[:, :], ins["b_fc1"].rearrange("(m p) -> p m", p=128))
            nc.sync.dma_start(
                bfc2_pp[:, :], ins["b_fc2"].rearrange("(m p) -> p m", p=128))
            nc.sync.dma_start(
                bada_pp[:, :, :],
                ins["b_ada"].rearrange("(c k p) -> p c k", k=KT, p=128))
            # pre-scale q/k biases by QS (folded into the psum->fp8 copies)
            nc.vector.tensor_scalar_mul(bq_s[:, :], bq_s[:, :], QS)
            nc.vector.tensor_scalar_mul(bk_s[:, :], bk_s[:, :], QS)

        xT = pers.tile([128, KT, NT], F32)      # becomes x2T after residual 1
        mod12T = pers.tile([128, KTP, NT], FP8)  # mod1T, later reused as mod2T
        nc.gpsimd.memset(mod12T[:, KT, :], 0.0)  # DoubleRow pad k-tile

        # ================= phase A: x load/transpose, ada, LN1 ==============

        def emit_ada_chunk(c, p1w, ps_pro, ps_bufs=2):
            """chunk c covers w_ada cols [c*384, (c+1)*384); param p=c//3."""
            wst = p1w.tile([128, KT, 384], F32R, tag="adast", bufs=2, name="wst")
            nc.sync.dma_start(
                wst[:, :, :],
                ins["w_ada"][:, c * 384:(c + 1) * 384]
                .rearrange("(k p) m -> p k m", p=128).bitcast(F32R),
            )
            pa = ps_pro.tile([1, 384], F32, tag="psada", bufs=ps_bufs,
                             name="pa")
            for k in range(KT):
                nc.tensor.matmul(
                    pa[:, :], t_pr[:, k:k + 1], wst[:, k, :],
                    start=(k == 0), stop=(k == KT - 1),
                )
            asb = p1w.tile([1, 384], F32, tag="asb", bufs=3, name="asb")
            nc.vector.tensor_copy(asb[:, :], pa[:, :])
            nc.scalar.dma_start(
                ada_dr[c * 384:(c + 1) * 384].rearrange("(a b) -> a b", a=1),
                asb[0:1, :],
            )

        def emit_ada_pp_load(cs):
            """Load+finalize ada params cs (list) into ada_pp; params 0/1
            (shift_a/scale_a) and 3/4 are pre-scaled by AS; 1/4 get +1."""
            for c in cs:
                nc.scalar.dma_start(
                    ada_pp[:, c, :],
                    ada_dr[c * D:(c + 1) * D].rearrange("(k p) -> p k", p=128),
                )
            lo, hi = min(cs), max(cs) + 1
            nc.vector.tensor_add(ada_pp[:, lo:hi, :], ada_pp[:, lo:hi, :],
                                 bada_pp[:, lo:hi, :])
            for c in cs:
                if c in (1, 4):
                    nc.vector.tensor_scalar_add(ada_pp[:, c, :],
                                                ada_pp[:, c, :], 1.0)
                if c in (0, 1, 3, 4):
                    nc.vector.tensor_scalar_mul(ada_pp[:, c, :],
                                                ada_pp[:, c, :], AS)

        with tc.tile_pool(name="p1w", bufs=1) as p1w, \
             tc.tile_pool(name="pxin", bufs=3) as pxin, \
             tc.tile_pool(name="ps_pro", bufs=2, space="PSUM") as ps_pro, \
             tc.tile_pool(name="ps_tr", bufs=2, space="PSUM") as ps_tr:

            def emit_transpose_block(tt):
                xin = pxin.tile([128, D], F32, tag="xin", name="xin")
                nc.sync.dma_start(
                    xin[:, :], ins["x"][tt * 128:(tt + 1) * 128, :])
                for kd in range(KT):
                    pt = ps_tr.tile([128, 128], F32, tag="ptr", name="pt")
                    nc.tensor.transpose(
                        pt[:, :], xin[:, kd * 128:(kd + 1) * 128], ident[:, :])
                    tsl = slice(tt * 128, (tt + 1) * 128)
                    if kd % 2 == 0:
                        nc.vector.tensor_copy(xT[:, kd, tsl], pt[:, :])
                    else:
                        nc.scalar.copy(xT[:, kd, tsl], pt[:, :])

            for i in range(8):
                emit_transpose_block(i)
                if i == 0:
                    emit_bias_loads()
                if i < 6:
                    emit_ada_chunk(i, p1w, ps_pro)
            emit_ada_pp_load([0, 1])

        # ====== phase B part 1: qkv weight loads + converts (emitted before
        # LN1 so SP streams the loads while ada finishes / LN runs) =========
        es_qk = ExitStack()
        pqk8 = es_qk.enter_context(tc.tile_pool(name="pqk8", bufs=1))
        wq8 = pqk8.tile([128, KTP, D], FP8, name="wq8")
        wk8 = pqk8.tile([128, KTP, D], FP8, name="wk8")
        nc.gpsimd.memset(wq8[:, KT, :], 0.0)
        nc.gpsimd.memset(wk8[:, KT, :], 0.0)

        es_att = ExitStack()
        patt = es_att.enter_context(tc.tile_pool(name="patt", bufs=1, side="right"))
        attn_hs = patt.tile([72, H, NT], FP8, name="attn_hs")
        es_wp = ExitStack()
        pwp8 = es_wp.enter_context(
            tc.tile_pool(name="pwp8", bufs=1, side="right"))
        wp8 = pwp8.tile([72, H, D], FP8, name="wp8")
        es_va = ExitStack()
        pva = es_va.enter_context(tc.tile_pool(name="pva", bufs=1, side="right"))
        v_aug = pva.tile([128, NT // 128, H, 97], FP8, name="v_aug")
        nc.gpsimd.memset(v_aug[:, :, :, HD:97], 0.0)
        nc.gpsimd.memset(v_aug[:, :, :, 96:97], 1.0)

        es_b = ExitStack()
        pwst = es_b.enter_context(tc.tile_pool(name="pwst", bufs=1))
        wv8 = pwst.tile([128, KTP, D], FP8, tag="wv8", bufs=1, name="wv8")
        nc.gpsimd.memset(wv8[:, KT, :], 0.0)
        engs = ["act", "dve", "act", "dve", "act", "dve"]
        for j, (dst8, c0) in enumerate(((wq8, 0), (wk8, D), (wv8, 2 * D))):
            for half in range(2):
                msl = slice(half * 576, (half + 1) * 576)
                wst = pwst.tile([128, KT, 576], F32, tag="wst", bufs=2,
                                name="wst")
                nc.sync.dma_start(
                    wst[:, :, :],
                    ins["w_qkv"][:, c0 + half * 576:c0 + (half + 1) * 576]
                    .rearrange("(k p) m -> p k m", p=128),
                )
                eng = engs[j * 2 + half]
                for kk in range(3):
                    ksl = slice(kk * 3, kk * 3 + 3)
                    if eng == "act":
                        nc.scalar.activation(
                            dst8[:, ksl, msl], wst[:, ksl, :],
                            AF.Identity, scale=WS)
                    elif eng == "dve":
                        nc.vector.tensor_scalar_mul(
                            dst8[:, ksl, msl], wst[:, ksl, :], WS)
                    else:
                        nc.gpsimd.tensor_scalar_mul(
                            dst8[:, ksl, msl], wst[:, ksl, :], WS)

        # ====== LN1 (per-half, interleaved with v matmuls) ==================
        with tc.tile_pool(name="pst", bufs=1) as pst, \
             tc.tile_pool(name="pln", bufs=1) as pln, \
             tc.tile_pool(name="ps_st", bufs=4, space="PSUM") as ps_st, \
             tc.tile_pool(name="ps_v", bufs=3, space="PSUM") as ps_v:

            def v_block(tts):
                for tt in tts:
                    tsl = slice(tt * 128, (tt + 1) * 128)
                    for si, (c0, c1, h0, h1) in enumerate(V_SLICES):
                        pmv = ps_v.tile([128, 512], F32, tag="mv", name="pmv")
                        for i in range(KTP // 2):
                            nc.tensor.matmul(
                                pmv[:, 0:c1 - c0],
                                mod12T[:, 2 * i:2 * i + 2, tsl],
                                wv8[:, 2 * i:2 * i + 2, c0:c1],
                                start=(i == 0), stop=False, perf_mode=DR,
                                skip_group_check=True,
                            )
                        nc.tensor.matmul(
                            pmv[:, 0:c1 - c0], ones_row[:, :],
                            bv_b[:, c0:c1],
                            start=False, stop=True, skip_group_check=True,
                        )
                        vsrc = pmv[:, 0:c1 - c0].rearrange(
                            "p (h d) -> p h d", d=HD)
                        nc.vector.tensor_scalar_mul(
                            v_aug[:, tt, h0:h1, 0:HD], vsrc, 1.0 / (AS * WS))

            st1 = {}
            _ln_stats(tc, nc, xT, ones_col, pst, pln, ps_st, halves=(0,),
                      st=st1)
            _ln_apply(tc, nc, xT, mod12T, st1, ada_pp, 0, 1, pln, halves=(0,))
            _ln_stats(tc, nc, xT, ones_col, pst, pln, ps_st, halves=(1,),
                      st=st1)
            v_block(range(0, 4))
            _ln_apply(tc, nc, xT, mod12T, st1, ada_pp, 0, 1, pln, halves=(1,))
            v_block(range(4, 8))
        es_b.close()

        # ================= phase C: attention ===============================
        with tc.tile_pool(name="p3w", bufs=1) as p3w, \
             tc.tile_pool(name="pexp", bufs=1) as pexp, \
             tc.tile_pool(name="pat3", bufs=1) as pat3, \
             tc.tile_pool(name="ps_qk", bufs=2, space="PSUM") as ps_qk, \
             tc.tile_pool(name="ps_s", bufs=2, space="PSUM") as ps_s, \
             tc.tile_pool(name="ps_av", bufs=1, space="PSUM") as ps_av, \
             tc.tile_pool(name="ps_pa", bufs=1, space="PSUM") as ps_pa:

            def emit_fc1_stream(j):
                f1st = p3w.tile([128, KT, 256], F32, tag="f1st",
                                bufs=2, name="f1st")
                nc.sync.dma_start(
                    f1st[:, :, :],
                    ins["w_fc1"][:, j * 256:(j + 1) * 256]
                    .rearrange("(k p) m -> p k m", p=128),
                )
                f18o = p3w.tile([128, KT, 256], FP8, tag="f18o",
                                bufs=2, name="f18o")
                nc.gpsimd.tensor_scalar_mul(
                    f18o[:, :, :], f1st[:, :, :], WS)
                nc.scalar.dma_start(w1f8_dr[j, :, :, :], f18o[:, :, :])

            def emit_wp_stream(c):
                # reuse the f1st staging tag: [128, KT*256] bytes == 16*144
                wpt = p3w.tile([128, KT, 256], F32, tag="f1st", bufs=2,
                               name="wpt")
                wpv = (wpt[:, :, :].rearrange("p k m -> p (k m)")[0:72, :]
                       .rearrange("p (h m) -> p h m", h=H))
                msl = slice(c * 144, (c + 1) * 144)
                nc.sync.dma_start(
                    wpv[:, :, :],
                    ins["w_proj"][:, msl].rearrange("(h p) m -> p h m", p=72),
                )
                nc.vector.tensor_scalar_mul(
                    wp8[:, :, msl], wpv[:, :, :], WS)

            def emit_wp_stream(c):
                # reuse the f1st staging tag: KT*256 f32 bytes == 16*144
                wpt = p3w.tile([128, KT, 256], F32, tag="f1st", bufs=2,
                               name="wpt")
                wpv = (wpt[:, :, :].rearrange("p k m -> p (k m)")[0:72, :]
                       .rearrange("p (h m) -> p h m", h=H))
                msl = slice(c * 144, (c + 1) * 144)
                nc.sync.dma_start(
                    wpv[:, :, :],
                    ins["w_proj"][:, msl].rearrange("(h p) m -> p h m", p=72),
                )
                nc.vector.tensor_scalar_mul(
                    wp8[:, :, msl], wpv[:, :, :], WS)

            def emit_filler(h):
                # late ada chunks; fc1 fp8 stream-convert to DRAM
                if h % 4 != 3:
                    emit_ada_chunk(6 + h - h // 4, p3w, ps_pa, ps_bufs=1)
                if h == 15:
                    emit_ada_pp_load([2, 3])
                    emit_ada_pp_load([4, 5])
                if 2 <= h:
                    js = ([2 * h - 4, 2 * h - 3] if h < 6
                          else [h + 2])
                    for j in js:
                        emit_fc1_stream(j)
                if h >= 12:
                    emit_wp_stream(h - 12)

            for h in range(H):
                emit_filler(h)
                q_h = pat3.tile([72, NT], FP8, tag="qh", bufs=2, name="q_h")
                k_h = pat3.tile([72, NT], FP8, tag="kh", bufs=2, name="k_h")
                for n in range(2):
                    nsl = slice(n * 512, (n + 1) * 512)
                    pq = ps_qk.tile([72, 512], F32, tag="qk", name="pq")
                    for i in range(KTP // 2):
                        nc.tensor.matmul(
                            pq[:, :],
                            wq8[:, 2 * i:2 * i + 2, h * HD:(h + 1) * HD],
                            mod12T[:, 2 * i:2 * i + 2, nsl],
                            start=(i == 0), stop=(i == KTP // 2 - 1),
                            perf_mode=DR,
                        )
                    nc.vector.tensor_scalar(
                        q_h[:, nsl], pq[:, :], QS / (AS * WS),
                        bq_s[:, h:h + 1], ALU.mult, ALU.add,
                    )
                for n in range(2):
                    nsl = slice(n * 512, (n + 1) * 512)
                    pk = ps_qk.tile([72, 512], F32, tag="qk", name="pk")
                    for i in range(KTP // 2):
                        nc.tensor.matmul(
                            pk[:, :],
                            wk8[:, 2 * i:2 * i + 2, h * HD:(h + 1) * HD],
                            mod12T[:, 2 * i:2 * i + 2, nsl],
                            start=(i == 0), stop=(i == KTP // 2 - 1),
                            perf_mode=DR,
                        )
                    nc.vector.tensor_scalar(
                        k_h[:, nsl], pk[:, :], QS / (AS * WS),
                        bk_s[:, h:h + 1], ALU.mult, ALU.add,
                    )
                for n in range(2):
                    nsl = slice(n * 512, (n + 1) * 512)
                    exp_hn = pexp.tile([128, NT // 128, 512], FP8, tag="exp",
                                       bufs=3, name="exp_hn")
                    for kp in range(NT // 256):
                        pss = ps_s.tile([128, 2, 512], F32, tag="s",
                                        name="pss")
                        for j in range(2):
                            kt_i = 2 * kp + j
                            nc.tensor.matmul(
                                pss[:, j, :],
                                k_h[:, kt_i * 128:(kt_i + 1) * 128],
                                q_h[:, nsl], start=True, stop=True,
                            )
                        nc.scalar.activation(
                            exp_hn[:, 2 * kp:2 * kp + 2, :],
                            pss[:, :, :], AF.Exp, scale=ES)
                    pav = ps_av.tile([97, 512], F32, tag="av", name="pav")
                    for i in range(NT // 256):
                        nc.tensor.matmul(
                            pav[:, :],
                            v_aug[:, 2 * i:2 * i + 2, h, :],
                            exp_hn[:, 2 * i:2 * i + 2, :],
                            start=(i == 0), stop=(i == NT // 256 - 1),
                            perf_mode=DR,
                        )
                    den = pat3.tile([1, 512], F32, tag="den", bufs=3,
                                    name="den")
                    nc.vector.tensor_scalar_mul(den[:, :], pav[96:97, :],
                                                1.0 / PS)
                    nc.vector.reciprocal(den[:, :], den[:, :])
                    denB = pat3.tile([72, 512], F32, tag="denB", bufs=3,
                                     name="denB")
                    nc.gpsimd.partition_broadcast(denB[:, :], den[:, :])
                    nc.vector.tensor_mul(
                        attn_hs[:, h, nsl], pav[0:HD, :], denB[:, :])
        es_qk.close()  # wq8/wk8 no longer needed
        es_va.close()

        # ================= phase D: proj + residual + LN2 ===================
        es_w2 = ExitStack()
        pw2 = es_w2.enter_context(
            tc.tile_pool(name="pw2", bufs=1, side="right"))
        w2f8 = pw2.tile([128, KT, MH, 128], FP8, name="w2f8")

        with tc.tile_pool(name="p4", bufs=1) as p4, \
             tc.tile_pool(name="pst4", bufs=1) as pst4, \
             tc.tile_pool(name="pln4", bufs=1) as pln4:

            for i in range(4, 8):
                msl = slice(i * 144, (i + 1) * 144)
                wpst = p4.tile([72, H, 144], F32, tag="wpst", bufs=2,
                               name="wpst")
                nc.sync.dma_start(
                    wpst[:, :, :],
                    ins["w_proj"][:, msl].rearrange("(h p) m -> p h m", p=72),
                )
                for kk in range(2):
                    hsl = slice(kk * 8, kk * 8 + 8)
                    nc.vector.tensor_scalar_mul(
                        wp8[:, hsl, msl], wpst[:, hsl, :], WS)

            def emit_fc2_chunk(ch, eng, pool):
                f2s = pool.tile([128, MH, 64], F32, tag="f2s", bufs=2,
                                name="f2s")
                nc.sync.dma_start(
                    f2s[:, :, :],
                    ins["w_fc2"][:, ch * 64:(ch + 1) * 64]
                    .rearrange("(k p) m -> p k m", p=128),
                )
                eng.tensor_scalar_mul(
                    w2f8[:, ch // 2, :, (ch % 2) * 64:(ch % 2 + 1) * 64],
                    f2s[:, :, :], WS)

            st2 = {}
            with tc.tile_pool(name="ps_mm2", bufs=4, space="PSUM") as ps_mm2, \
                 tc.tile_pool(name="ps_st2", bufs=4, space="PSUM") as ps_st2:
                for n in range(2):
                    nsl = slice(n * 512, (n + 1) * 512)
                    for mo in range(KT):
                        if mo < 5:
                            ch = n * 5 + mo
                            eng = nc.gpsimd if ch % 2 else nc.vector
                            emit_fc2_chunk(ch, eng, p4)
                        pm2 = ps_mm2.tile([128, 512], F32, tag="mm2",
                                          name="pm2")
                        for i in range(H // 2):
                            nc.tensor.matmul(
                                pm2[:, :],
                                wp8[:, 2 * i:2 * i + 2,
                                    mo * 128:(mo + 1) * 128],
                                attn_hs[:, 2 * i:2 * i + 2, nsl],
                                start=(i == 0), stop=(i == H // 2 - 1),
                                perf_mode=DR,
                            )
                        t_sb = p4.tile([128, 512], F32, tag="tsb", bufs=2,
                                       name="t_sb")
                        nc.scalar.activation(
                            t_sb[:, :], pm2[:, :], AF.Identity,
                            bias=bproj_pp[:, mo:mo + 1], scale=1.0 / (PS * WS),
                        )
                        nc.vector.scalar_tensor_tensor(
                            xT[:, mo, nsl], t_sb[:, :],
                            ada_pp[:, 2, mo:mo + 1], xT[:, mo, nsl],
                            ALU.mult, ALU.add,
                        )
                    _ln_stats(tc, nc, xT, ones_col, pst4, pln4, ps_st2,
                              halves=(n,), st=st2)
                    _ln_apply(tc, nc, xT, mod12T, st2, ada_pp, 3, 4, pln4,
                              halves=(n,))

        # ================= phase E: FFN =====================================
        es_e = ExitStack()
        ph = es_e.enter_context(tc.tile_pool(name="ph", bufs=1))
        hT = ph.tile([128, MH, NT], FP8, name="hT")
        po = es_e.enter_context(tc.tile_pool(name="po", bufs=1))

        with tc.tile_pool(name="ps_f1", bufs=3, space="PSUM") as ps_f1, \
             tc.tile_pool(name="ps_f2", bufs=3, space="PSUM") as ps_f2, \
             tc.tile_pool(name="ps_tro", bufs=2, space="PSUM") as ps_tro:
            # fc1 in 18 chunks of 256 columns (2 m-tiles each), weights
            # already converted to fp8 in DRAM during the attention window
            with tc.tile_pool(name="p5a", bufs=1) as p5a:
                for ch in range(18):
                    f18 = p5a.tile([128, KTP, 256], FP8, tag="f18", bufs=3,
                                   name="f18")
                    nc.sync.dma_start(f18[:, 0:KT, :], w1f8_dr[ch, :, :, :])
                    nc.gpsimd.memset(f18[:, KT, :], 0.0)
                    if ch < 8:
                        emit_fc2_chunk(10 + ch,
                                       nc.gpsimd if ch % 2 else nc.vector,
                                       p5a)
                    for m in range(2):
                        mo = ch * 2 + m
                        for n in range(2):
                            nsl = slice(n * 512, (n + 1) * 512)
                            pf1 = ps_f1.tile([128, 512], F32, tag="f1",
                                             name="pf1")
                            for i in range(KTP // 2):
                                nc.tensor.matmul(
                                    pf1[:, :],
                                    f18[:, 2 * i:2 * i + 2,
                                        m * 128:(m + 1) * 128],
                                    mod12T[:, 2 * i:2 * i + 2, nsl],
                                    start=(i == 0), stop=(i == KTP // 2 - 1),
                                    perf_mode=DR,
                                )
                            nc.scalar.activation(
                                hT[:, mo, nsl], pf1[:, :], AF.Gelu_apprx_tanh,
                                bias=bfc1_pp[:, mo:mo + 1],
                                scale=1.0 / (AS * WS),
                            )
            # fc2: weights already fp8-resident in SBUF (w2f8)
            with tc.tile_pool(name="p5b", bufs=1) as p5b:
                for mo in range(KT):
                    for n in range(2):
                        nsl = slice(n * 512, (n + 1) * 512)
                        pf2 = ps_f2.tile([128, 512], F32, tag="f2", name="pf2")
                        for i in range(MH // 2):
                            nc.tensor.matmul(
                                pf2[:, :], w2f8[:, mo, 2 * i:2 * i + 2, :],
                                hT[:, 2 * i:2 * i + 2, nsl],
                                start=(i == 0), stop=(i == MH // 2 - 1),
                                perf_mode=DR,
                            )
                        t2 = p5b.tile([128, 512], F32, tag="t2", bufs=3,
                                      name="t2")
                        nc.scalar.activation(
                            t2[:, :], pf2[:, :], AF.Identity,
                            bias=bfc2_pp[:, mo:mo + 1], scale=1.0 / WS,
                        )
                        nc.vector.scalar_tensor_tensor(
                            xT[:, mo, nsl], t2[:, :], ada_pp[:, 5, mo:mo + 1],
                            xT[:, mo, nsl], ALU.mult, ALU.add,
                        )
                    o_slab = po.tile([128, NT // 128, 128], F32, tag="osl",
                                     bufs=2, name="o_slab")
                    for tt in range(NT // 128):
                        pt = ps_tro.tile([128, 128], F32, tag="tro",
                                         name="pt6")
                        nc.tensor.transpose(
                            pt[:, :], xT[:, mo, tt * 128:(tt + 1) * 128],
                            ident[:, :],
                        )
                        dst = o_slab[:, tt, :]
                        nc.vector.tensor_copy(dst, pt[:, :])
                    nc.scalar.dma_start(
                        out_dram[:, mo * 128:(mo + 1) * 128]
                        .rearrange("(t p) m -> p t m", p=128),
                        o_slab[:, :, :])
        es_w2.close()
        es_wp.close()
        es_att.close()
        es_e.close()


_LOCK = threading.Lock()
_PROG = None


def _get_program():
    global _PROG
    with _LOCK:
        if _PROG is None:
            _PROG = _build_program()
    return _PROG


def _make_in_maps(inputs):
    arrs = {k: np.ascontiguousarray(np.asarray(v, dtype=np.float32))
            for k, v in inputs.items()}
    in_maps = []
    for c in range(NCORES):
        m = {k: v for k, v in arrs.items() if k not in ("x", "t_emb")}
        m["x"] = np.ascontiguousarray(arrs["x"][c])
        m["t_emb"] = np.ascontiguousarray(arrs["t_emb"][c])
        in_maps.append(m)
    return in_maps


def kernel(**inputs):
    nc = _get_program()
    res = run_bass_kernel_spmd(nc, _make_in_maps(inputs),
                               core_ids=list(range(NCORES)))
    return np.stack([r["out"] for r in res.results], axis=0)


def kernel_traced(inputs, **kw):
    """test-harness helper: returns full BassKernelResults with trace."""
    nc = _get_program()
    return run_bass_kernel_spmd(
        nc, _make_in_maps(inputs), core_ids=list(range(NCORES)), trace=True,
        **kw
    )


# revision 109
# speedup vs baseline: 1.6113x; 1.0030x over previous
"""DiT block kernel for Trainium2 (Bass/Tile), 8-core data parallel.

Shapes (hardcoded from the problem spec):
  x: (8, 1024, 1152), t_emb: (8, 1152)
  w_qkv (1152, 3456), w_proj (1152, 1152), w_fc1 (1152, 4608),
  w_fc2 (4608, 1152), w_ada (1152, 6912) + biases.

Strategy: batch-parallel across 8 cores (one batch element each, no
collectives). Activations live transposed [D on partitions, tokens free].
The large matmuls (qkv, attention AV, proj, fc1, fc2) run in fp8e4 with
DoubleRow perf mode (two 128-row k-tiles contracted per instruction);
scale factors for fp8 range are folded into the existing activation
bias/scale stages so no extra elementwise work is added.  LayerNorm
statistics reduce over the partition axis via ones-vector f32r matmuls;
softmax runs transposed (keys on partitions) with denominators collected
through a ones-column appended to V and a fused divide.  q/k are produced
per-head directly (M=72 matmuls cost the same per column as M=128), so
attention needs no partition-crossing gather DMAs.  Weights stream
through big staged f32 DMA loads (few, large transfers) and are
converted on-chip; ada (error-sensitive) stays f32r.
"""

import threading
from contextlib import ExitStack

import numpy as np

import concourse.bass as bass
import concourse.mybir as mybir
import concourse.tile as tile
from concourse import bacc
from concourse.bass_utils import run_bass_kernel_spmd
from concourse.masks import make_identity

F32 = mybir.dt.float32
F32R = mybir.dt.float32r
BF16 = mybir.dt.bfloat16
FP8 = mybir.dt.float8e4
AF = mybir.ActivationFunctionType
ALU = mybir.AluOpType
DR = mybir.MatmulPerfMode.DoubleRow

NCORES = 8
D = 1152
NT = 1024
KT = D // 128       # 9
KTP = KT + 1        # padded to even for DoubleRow pairs
H = 16
HD = 72
HID = 4 * D
MH = HID // 128     # 36
EPS = 1e-6
ISC = 1.0 / float(np.sqrt(HD))

# fp8 scale factors
WS = 64.0           # weights
AS = 8.0            # modulated activations (mod1/mod2)
QS = 2.0            # q/k
PS = 4.0            # attention output
ES = ISC / (QS * QS)  # exp() input scale applied to the scores psum

# v output column slices aligned to head boundaries
V_SLICES = [(0, 432, 0, 6), (432, 864, 6, 12), (864, 1152, 12, 16)]


def _r(ap):
    return ap.bitcast(F32R)


def _build_program():
    nc = bacc.Bacc(
        "TRN2", target_bir_lowering=False, debug=False, enable_asserts=False
    )
    ins = {}
    ins["x"] = nc.dram_tensor("x", [NT, D], F32, kind="ExternalInput").ap()
    ins["t_emb"] = nc.dram_tensor("t_emb", [D], F32, kind="ExternalInput").ap()
    for name, shape in [
        ("w_qkv", [D, 3 * D]), ("b_qkv", [3 * D]),
        ("w_proj", [D, D]), ("b_proj", [D]),
        ("w_fc1", [D, HID]), ("b_fc1", [HID]),
        ("w_fc2", [HID, D]), ("b_fc2", [D]),
        ("w_ada", [D, 6 * D]), ("b_ada", [6 * D]),
    ]:
        ins[name] = nc.dram_tensor(name, shape, F32, kind="ExternalInput").ap()
    out_dram = nc.dram_tensor("out", [NT, D], F32, kind="ExternalOutput").ap()

    with tile.TileContext(nc) as tc:
        _body(tc, ins, out_dram)
    nc.compile()
    return nc


def _ln_stats(tc, nc, src, ones_col, pst, pln, ps_st, halves=(0, 1),
              st=None):
    """Return st[n] = [mean; rstd] rows [1, 2, 512] per 512-token half,
    reducing over the partition (D) axis of src [128, KT, NT] f32."""
    ps_x, ps_q = {}, {}
    if st is None:
        st = {}
    for n in halves:
        nsl = slice(n * 512, (n + 1) * 512)
        ps_x[n] = ps_st.tile([1, 512], F32, tag="st", name=f"psx{n}")
        ps_q[n] = ps_st.tile([1, 512], F32, tag="st", name=f"psq{n}")
        for k in range(KT):
            xb = pln.tile([128, 512], BF16, tag="xb", bufs=2, name="xb")
            nc.scalar.copy(xb[:, :], src[:, k, nsl])
            sq = pln.tile([128, 512], BF16, tag="sq", bufs=2, name="sq")
            nc.vector.tensor_mul(sq[:, :], src[:, k, nsl], src[:, k, nsl])
            nc.tensor.matmul(
                ps_x[n][:, :], ones_col[:, :], xb[:, :],
                start=(k == 0), stop=(k == KT - 1), skip_group_check=True,
            )
            nc.tensor.matmul(
                ps_q[n][:, :], ones_col[:, :], sq[:, :],
                start=(k == 0), stop=(k == KT - 1), skip_group_check=True,
            )
    eps_sb = pst.tile([1, 1], F32, tag="eps", bufs=1, name="eps_sb")
    nc.vector.memset(eps_sb[:, :], EPS)
    for n in halves:
        st[n] = pst.tile([1, 2, 512], F32, tag="lnst", bufs=2, name=f"st{n}")
        nc.vector.tensor_scalar_mul(st[n][:, 0, :], ps_x[n][:, :], 1.0 / D)
        work = pst.tile([1, 512], F32, tag="lnwork", bufs=2, name="work")
        nc.vector.tensor_mul(work[:, :], st[n][:, 0, :], st[n][:, 0, :])
        nc.vector.scalar_tensor_tensor(
            st[n][:, 1, :], ps_q[n][:, :], 1.0 / D, work[:, :],
            ALU.mult, ALU.subtract,
        )
        nc.scalar.activation(st[n][:, 1, :], st[n][:, 1, :], AF.Sqrt,
                             bias=eps_sb[:, :], scale=1.0)
        nc.vector.reciprocal(st[n][:, 1, :], st[n][:, 1, :])
    return st


def _ln_apply(tc, nc, src, dst, st, ada_pp, sh_c, sc_c, pln,
              halves=(0, 1)):
    """dst[:,k,nsl] (fp8) = ((src-mean)*rstd) * ada[sc_c] + ada[sh_c]
    (ada params pre-scaled by AS)."""
    for n in halves:
        nsl = slice(n * 512, (n + 1) * 512)
        meanB = pln.tile([128, 512], F32, tag="meanB", bufs=2, name="meanB")
        rstdB = pln.tile([128, 512], F32, tag="rstdB", bufs=2, name="rstdB")
        nc.gpsimd.partition_broadcast(meanB[:, :], st[n][:, 0, :])
        nc.gpsimd.partition_broadcast(rstdB[:, :], st[n][:, 1, :])
        for k in range(KT):
            t1 = pln.tile([128, 512], F32, tag="lnt1", bufs=3, name="t1")
            nc.vector.tensor_sub(t1[:, :], src[:, k, nsl], meanB[:, :])
            nc.vector.tensor_mul(t1[:, :], t1[:, :], rstdB[:, :])
            nc.gpsimd.tensor_scalar(
                dst[:, k, nsl], t1[:, :],
                ada_pp[:, sc_c, k:k + 1], ada_pp[:, sh_c, k:k + 1],
                ALU.mult, ALU.add,
            )


def _body(tc, ins, out_dram):
    nc = tc.nc
    ctx = ExitStack()
    with ctx:
        dram = ctx.enter_context(tc.tile_pool(name="dram", bufs=1, space="DRAM"))
        ada_dr = dram.tile([6 * D], F32)
        w1f8_dr = dram.tile([18, 128, KT, 256], FP8)

        pers = ctx.enter_context(tc.tile_pool(name="pers", bufs=1))
        ident = pers.tile([128, 128], F32)
        make_identity(nc, ident[:, :])
        ones_col = pers.tile([128, 1], BF16)
        nc.vector.memset(ones_col[:, :], 1.0)
        ones_row = pers.tile([1, 128], BF16)
        nc.vector.memset(ones_row[:, :], 1.0)

        t_pp = pers.tile([128, KT], F32)
        nc.sync.dma_start(t_pp[:, :], ins["t_emb"].rearrange("(k p) -> p k", p=128))
        t_pr = pers.tile([128, KT], F32R)
        nc.scalar.activation(t_pr[:, :], t_pp[:, :], AF.Silu)

        bq_s = pers.tile([72, H], F32)
        bk_s = pers.tile([72, H], F32)
        bv_row = pers.tile([1, D], F32)
        bv_b = pers.tile([1, D], BF16)
        bproj_pp = pers.tile([128, KT], F32)
        bfc1_pp = pers.tile([128, MH], F32)
        bfc2_pp = pers.tile([128, KT], F32)
        bada_pp = pers.tile([128, 6, KT], F32)
        ada_pp = pers.tile([128, 6, KT], F32)

        def emit_bias_loads():
            nc.sync.dma_start(
                bq_s[:, :], ins["b_qkv"][0:D].rearrange("(h p) -> p h", p=72))
            nc.sync.dma_start(
                bk_s[:, :], ins["b_qkv"][D:2 * D].rearrange("(h p) -> p h", p=72))
            nc.sync.dma_start(
                bv_row[:, :],
                ins["b_qkv"][2 * D:3 * D].rearrange("(a b) -> a b", a=1))
            # bv enters the v accumulation in (AS*WS)-scaled psum units
            nc.vector.tensor_scalar_mul(bv_b[:, :], bv_row[:, :], AS * WS)
            nc.sync.dma_start(
                bproj_pp[:, :], ins["b_proj"].rearrange("(m p) -> p m", p=128))
            nc.sync.dma_start(
                bfc1_pp[:, :], ins["b_fc1"].rearrange("(m p) -> p m", p=128))
            nc.sync.dma_start(
                bfc2_pp[:, :], ins["b_fc2"].rearrange("(m p) -> p m", p=128))
            nc.sync.dma_start(
                bada_pp[:, :, :],
                ins["b_ada"].rearrange("(c k p) -> p c k", k=KT, p=128))
            # pre-scale q/k biases by QS (folded into the psum->fp8 copies)
            nc.vector.tensor_scalar_mul(bq_s[:, :], bq_s[:, :], QS)
            nc.vector.tensor_scalar_mul(bk_s[:, :], bk_s[:, :], QS)

        xT = pers.tile([128, KT, NT], F32)      # becomes x2T after residual 1
        mod12T = pers.tile([128, KTP, NT], FP8)  # mod1T, later reused as mod2T
        nc.gpsimd.memset(mod12T[:, KT, :], 0.0)  # DoubleRow pad k-tile

        # ================= phase A: x load/transpose, ada, LN1 ==============

        def emit_ada_chunk(c, p1w, ps_pro, ps_bufs=2):
            """chunk c covers w_ada cols [c*384, (c+1)*384); param p=c//3."""
            wst = p1w.tile([128, KT, 384], F32R, tag="adast", bufs=2, name="wst")
            nc.sync.dma_start(
                wst[:, :, :],
                ins["w_ada"][:, c * 384:(c + 1) * 384]
                .rearrange("(k p) m -> p k m", p=128).bitcast(F32R),
            )
            pa = ps_pro.tile([1, 384], F32, tag="psada", bufs=ps_bufs,
                             name="pa")
            for k in range(KT):
                nc.tensor.matmul(
                    pa[:, :], t_pr[:, k:k + 1], wst[:, k, :],
                    start=(k == 0), stop=(k == KT - 1),
                )
            asb = p1w.tile([1, 384], F32, tag="asb", bufs=3, name="asb")
            nc.vector.tensor_copy(asb[:, :], pa[:, :])
            nc.scalar.dma_start(
                ada_dr[c * 384:(c + 1) * 384].rearrange("(a b) -> a b", a=1),
                asb[0:1, :],
            )

        def emit_ada_pp_load(cs):
            """Load+finalize ada params cs (list) into ada_pp; params 0/1
            (shift_a/scale_a) and 3/4 are pre-scaled by AS; 1/4 get +1."""
            for c in cs:
                nc.scalar.dma_start(
                    ada_pp[:, c, :],
                    ada_dr[c * D:(c + 1) * D].rearrange("(k p) -> p k", p=128),
                )
            lo, hi = min(cs), max(cs) + 1
            nc.vector.tensor_add(ada_pp[:, lo:hi, :], ada_pp[:, lo:hi, :],
                                 bada_pp[:, lo:hi, :])
            for c in cs:
                if c in (1, 4):
                    nc.vector.tensor_scalar_add(ada_pp[:, c, :],
                                                ada_pp[:, c, :], 1.0)
                if c in (0, 1, 3, 4):
                    nc.vector.tensor_scalar_mul(ada_pp[:, c, :],
                                                ada_pp[:, c, :], AS)

        with tc.tile_pool(name="p1w", bufs=1) as p1w, \
             tc.tile_pool(name="pxin", bufs=3) as pxin, \
             tc.tile_pool(name="ps_pro", bufs=2, space="PSUM") as ps_pro, \
             tc.tile_pool(name="ps_tr", bufs=2, space="PSUM") as ps_tr:

            def emit_transpose_block(tt):
                xin = pxin.tile([128, D], F32, tag="xin", name="xin")
                nc.sync.dma_start(
                    xin[:, :], ins["x"][tt * 128:(tt + 1) * 128, :])
                for kd in range(KT):
                    pt = ps_tr.tile([128, 128], F32, tag="ptr", name="pt")
                    nc.tensor.transpose(
                        pt[:, :], xin[:, kd * 128:(kd + 1) * 128], ident[:, :])
                    tsl = slice(tt * 128, (tt + 1) * 128)
                    if kd % 2 == 0:
                        nc.vector.tensor_copy(xT[:, kd, tsl], pt[:, :])
                    else:
                        nc.scalar.copy(xT[:, kd, tsl], pt[:, :])

            for i in range(8):
                emit_transpose_block(i)
                if i == 0:
                    emit_bias_loads()
                if i < 6:
                    emit_ada_chunk(i, p1w, ps_pro)
            emit_ada_pp_load([0, 1])

        # ====== phase B part 1: qkv weight loads + converts (emitted before
        # LN1 so SP streams the loads while ada finishes / LN runs) =========
        es_qk = ExitStack()
        pqk8 = es_qk.enter_context(tc.tile_pool(name="pqk8", bufs=1))
        wq8 = pqk8.tile([128, KTP, D], FP8, name="wq8")
        wk8 = pqk8.tile([128, KTP, D], FP8, name="wk8")
        nc.gpsimd.memset(wq8[:, KT, :], 0.0)
        nc.gpsimd.memset(wk8[:, KT, :], 0.0)

        es_att = ExitStack()
        patt = es_att.enter_context(tc.tile_pool(name="patt", bufs=1, side="right"))
        attn_hs = patt.tile([72, H, NT], FP8, name="attn_hs")
        es_wp = ExitStack()
        pwp8 = es_wp.enter_context(
            tc.tile_pool(name="pwp8", bufs=1, side="right"))
        wp8 = pwp8.tile([72, H, D], FP8, name="wp8")
        es_va = ExitStack()
        pva = es_va.enter_context(tc.tile_pool(name="pva", bufs=1, side="right"))
        v_aug = pva.tile([128, NT // 128, H, 97], FP8, name="v_aug")
        nc.gpsimd.memset(v_aug[:, :, :, HD:97], 0.0)
        nc.gpsimd.memset(v_aug[:, :, :, 96:97], 1.0)

        es_b = ExitStack()
        pwst = es_b.enter_context(tc.tile_pool(name="pwst", bufs=1))
        wv8 = pwst.tile([128, KTP, D], FP8, tag="wv8", bufs=1, name="wv8")
        nc.gpsimd.memset(wv8[:, KT, :], 0.0)
        engs = ["act", "dve", "act", "dve", "act", "dve"]
        for j, (dst8, c0) in enumerate(((wq8, 0), (wk8, D), (wv8, 2 * D))):
            for half in range(2):
                msl = slice(half * 576, (half + 1) * 576)
                wst = pwst.tile([128, KT, 576], F32, tag="wst", bufs=2,
                                name="wst")
                nc.sync.dma_start(
                    wst[:, :, :],
                    ins["w_qkv"][:, c0 + half * 576:c0 + (half + 1) * 576]
                    .rearrange("(k p) m -> p k m", p=128),
                )
                eng = engs[j * 2 + half]
                for kk in range(3):
                    ksl = slice(kk * 3, kk * 3 + 3)
                    if eng == "act":
                        nc.scalar.activation(
                            dst8[:, ksl, msl], wst[:, ksl, :],
                            AF.Identity, scale=WS)
                    elif eng == "dve":
                        nc.vector.tensor_scalar_mul(
                            dst8[:, ksl, msl], wst[:, ksl, :], WS)
                    else:
                        nc.gpsimd.tensor_scalar_mul(
                            dst8[:, ksl, msl], wst[:, ksl, :], WS)

        # ====== LN1 (per-half, interleaved with v matmuls) ==================
        with tc.tile_pool(name="pst", bufs=1) as pst, \
             tc.tile_pool(name="pln", bufs=1) as pln, \
             tc.tile_pool(name="ps_st", bufs=4, space="PSUM") as ps_st, \
             tc.tile_pool(name="ps_v", bufs=3, space="PSUM") as ps_v:

            def v_block(tts):
                for tt in tts:
                    tsl = slice(tt * 128, (tt + 1) * 128)
                    for si, (c0, c1, h0, h1) in enumerate(V_SLICES):
                        pmv = ps_v.tile([128, 512], F32, tag="mv", name="pmv")
                        for i in range(KTP // 2):
                            nc.tensor.matmul(
                                pmv[:, 0:c1 - c0],
                                mod12T[:, 2 * i:2 * i + 2, tsl],
                                wv8[:, 2 * i:2 * i + 2, c0:c1],
                                start=(i == 0), stop=False, perf_mode=DR,
                                skip_group_check=True,
                            )
                        nc.tensor.matmul(
                            pmv[:, 0:c1 - c0], ones_row[:, :],
                            bv_b[:, c0:c1],
                            start=False, stop=True, skip_group_check=True,
                        )
                        vsrc = pmv[:, 0:c1 - c0].rearrange(
                            "p (h d) -> p h d", d=HD)
                        nc.vector.tensor_scalar_mul(
                            v_aug[:, tt, h0:h1, 0:HD], vsrc, 1.0 / (AS * WS))

            st1 = {}
            _ln_stats(tc, nc, xT, ones_col, pst, pln, ps_st, halves=(0,),
                      st=st1)
            _ln_apply(tc, nc, xT, mod12T, st1, ada_pp, 0, 1, pln, halves=(0,))
            _ln_stats(tc, nc, xT, ones_col, pst, pln, ps_st, halves=(1,),
                      st=st1)
            v_block(range(0, 4))
            _ln_apply(tc, nc, xT, mod12T, st1, ada_pp, 0, 1, pln, halves=(1,))
            v_block(range(4, 8))
        es_b.close()

        # ================= phase C: attention ===============================
        with tc.tile_pool(name="p3w", bufs=1) as p3w, \
             tc.tile_pool(name="pexp", bufs=1) as pexp, \
             tc.tile_pool(name="pat3", bufs=1) as pat3, \
             tc.tile_pool(name="ps_qk", bufs=2, space="PSUM") as ps_qk, \
             tc.tile_pool(name="ps_s", bufs=2, space="PSUM") as ps_s, \
             tc.tile_pool(name="ps_av", bufs=1, space="PSUM") as ps_av, \
             tc.tile_pool(name="ps_pa", bufs=1, space="PSUM") as ps_pa:

            def emit_fc1_stream(j):
                f1st = p3w.tile([128, KT, 256], F32, tag="f1st",
                                bufs=2, name="f1st")
                nc.sync.dma_start(
                    f1st[:, :, :],
                    ins["w_fc1"][:, j * 256:(j + 1) * 256]
                    .rearrange("(k p) m -> p k m", p=128),
                )
                f18o = p3w.tile([128, KT, 256], FP8, tag="f18o",
                                bufs=2, name="f18o")
                nc.gpsimd.tensor_scalar_mul(
                    f18o[:, :, :], f1st[:, :, :], WS)
                nc.scalar.dma_start(w1f8_dr[j, :, :, :], f18o[:, :, :])

            def emit_wp_stream(c):
                # reuse the f1st staging tag: [128, KT*256] bytes == 16*144
                wpt = p3w.tile([128, KT, 256], F32, tag="f1st", bufs=2,
                               name="wpt")
                wpv = (wpt[:, :, :].rearrange("p k m -> p (k m)")[0:72, :]
                       .rearrange("p (h m) -> p h m", h=H))
                msl = slice(c * 144, (c + 1) * 144)
                nc.sync.dma_start(
                    wpv[:, :, :],
                    ins["w_proj"][:, msl].rearrange("(h p) m -> p h m", p=72),
                )
                nc.vector.tensor_scalar_mul(
                    wp8[:, :, msl], wpv[:, :, :], WS)

            def emit_wp_stream(c):
                # reuse the f1st staging tag: KT*256 f32 bytes == 16*144
                wpt = p3w.tile([128, KT, 256], F32, tag="f1st", bufs=2,
                               name="wpt")
                wpv = (wpt[:, :, :].rearrange("p k m -> p (k m)")[0:72, :]
                       .rearrange("p (h m) -> p h m", h=H))
                msl = slice(c * 144, (c + 1) * 144)
                nc.sync.dma_start(
                    wpv[:, :, :],
                    ins["w_proj"][:, msl].rearrange("(h p) m -> p h m", p=72),
                )
                nc.vector.tensor_scalar_mul(
                    wp8[:, :, msl], wpv[:, :, :], WS)

            def emit_filler(h):
                # late ada chunks; fc1 fp8 stream-convert to DRAM
                if h % 4 != 3:
                    emit_ada_chunk(6 + h - h // 4, p3w, ps_pa, ps_bufs=1)
                if h == 15:
                    emit_ada_pp_load([2, 3])
                    emit_ada_pp_load([4, 5])
                if 2 <= h:
                    js = ([2 * h - 4, 2 * h - 3] if h < 6
                          else [h + 2])
                    for j in js:
                        emit_fc1_stream(j)
                if h >= 12:
                    emit_wp_stream(h - 12)

            for h in range(H):
                emit_filler(h)
                q_h = pat3.tile([72, NT], FP8, tag="qh", bufs=2, name="q_h")
                k_h = pat3.tile([72, NT], FP8, tag="kh", bufs=2, name="k_h")
                for n in range(2):
                    nsl = slice(n * 512, (n + 1) * 512)
                    pq = ps_qk.tile([72, 512], F32, tag="qk", name="pq")
                    for i in range(KTP // 2):
                        nc.tensor.matmul(
                            pq[:, :],
                            wq8[:, 2 * i:2 * i + 2, h * HD:(h + 1) * HD],
                            mod12T[:, 2 * i:2 * i + 2, nsl],
                            start=(i == 0), stop=(i == KTP // 2 - 1),
                            perf_mode=DR,
                        )
                    nc.vector.tensor_scalar(
                        q_h[:, nsl], pq[:, :], QS / (AS * WS),
                        bq_s[:, h:h + 1], ALU.mult, ALU.add,
                    )
                for n in range(2):
                    nsl = slice(n * 512, (n + 1) * 512)
                    pk = ps_qk.tile([72, 512], F32, tag="qk", name="pk")
                    for i in range(KTP // 2):
                        nc.tensor.matmul(
                            pk[:, :],
                            wk8[:, 2 * i:2 * i + 2, h * HD:(h + 1) * HD],
                            mod12T[:, 2 * i:2 * i + 2, nsl],
                            start=(i == 0), stop=(i == KTP // 2 - 1),
                            perf_mode=DR,
                        )
                    nc.vector.tensor_scalar(
                        k_h[:, nsl], pk[:, :], QS / (AS * WS),
                        bk_s[:, h:h + 1], ALU.mult, ALU.add,
                    )
                for n in range(2):
                    nsl = slice(n * 512, (n + 1) * 512)
                    exp_hn = pexp.tile([128, NT // 128, 512], FP8, tag="exp",
                                       bufs=3, name="exp_hn")
                    for kp in range(NT // 256):
                        pss = ps_s.tile([128, 2, 512], F32, tag="s",
                                        name="pss")
                        for j in range(2):
                            kt_i = 2 * kp + j
                            nc.tensor.matmul(
                                pss[:, j, :],
                                k_h[:, kt_i * 128:(kt_i + 1) * 128],
                                q_h[:, nsl], start=True, stop=True,
                            )
                        nc.scalar.activation(
                            exp_hn[:, 2 * kp:2 * kp + 2, :],
                            pss[:, :, :], AF.Exp, scale=ES)
                    pav = ps_av.tile([97, 512], F32, tag="av", name="pav")
                    for i in range(NT // 256):
                        nc.tensor.matmul(
                            pav[:, :],
                            v_aug[:, 2 * i:2 * i + 2, h, :],
                            exp_hn[:, 2 * i:2 * i + 2, :],
                            start=(i == 0), stop=(i == NT // 256 - 1),
                            perf_mode=DR,
                        )
                    den = pat3.tile([1, 512], F32, tag="den", bufs=3,
                                    name="den")
                    nc.vector.tensor_scalar_mul(den[:, :], pav[96:97, :],
                                                1.0 / PS)
                    nc.vector.reciprocal(den[:, :], den[:, :])
                    denB = pat3.tile([72, 512], F32, tag="denB", bufs=3,
                                     name="denB")
                    nc.gpsimd.partition_broadcast(denB[:, :], den[:, :])
                    nc.vector.tensor_mul(
                        attn_hs[:, h, nsl], pav[0:HD, :], denB[:, :])
        es_qk.close()  # wq8/wk8 no longer needed
        es_va.close()

        # ================= phase D: proj + residual + LN2 ===================
        es_w2 = ExitStack()
        pw2 = es_w2.enter_context(
            tc.tile_pool(name="pw2", bufs=1, side="right"))
        w2f8 = pw2.tile([128, KT, MH, 128], FP8, name="w2f8")

        with tc.tile_pool(name="p4", bufs=1) as p4, \
             tc.tile_pool(name="pst4", bufs=1) as pst4, \
             tc.tile_pool(name="pln4", bufs=1) as pln4:

            for i in range(4, 8):
                msl = slice(i * 144, (i + 1) * 144)
                wpst = p4.tile([72, H, 144], F32, tag="wpst", bufs=2,
                               name="wpst")
                nc.sync.dma_start(
                    wpst[:, :, :],
                    ins["w_proj"][:, msl].rearrange("(h p) m -> p h m", p=72),
                )
                for kk in range(2):
                    hsl = slice(kk * 8, kk * 8 + 8)
                    nc.vector.tensor_scalar_mul(
                        wp8[:, hsl, msl], wpst[:, hsl, :], WS)

            def emit_fc2_chunk(ch, eng, pool):
                f2s = pool.tile([128, MH, 64], F32, tag="f2s", bufs=2,
                                name="f2s")
                nc.sync.dma_start(
                    f2s[:, :, :],
                    ins["w_fc2"][:, ch * 64:(ch + 1) * 64]
                    .rearrange("(k p) m -> p k m", p=128),
                )
                eng.tensor_scalar_mul(
                    w2f8[:, ch // 2, :, (ch % 2) * 64:(ch % 2 + 1) * 64],
                    f2s[:, :, :], WS)

            st2 = {}
            with tc.tile_pool(name="ps_mm2", bufs=4, space="PSUM") as ps_mm2, \
                 tc.tile_pool(name="ps_st2", bufs=4, space="PSUM") as ps_st2:
                for n in range(2):
                    nsl = slice(n * 512, (n + 1) * 512)
                    for mo in range(KT):
                        if mo < 4:
                            ch = n * 4 + mo
                            eng = nc.gpsimd if ch % 2 else nc.vector
                            emit_fc2_chunk(ch, eng, p4)
                        pm2 = ps_mm2.tile([128, 512], F32, tag="mm2",
                                          name="pm2")
                        for i in range(H // 2):
                            nc.tensor.matmul(
                                pm2[:, :],
                                wp8[:, 2 * i:2 * i + 2,
                                    mo * 128:(mo + 1) * 128],
                                attn_hs[:, 2 * i:2 * i + 2, nsl],
                                start=(i == 0), stop=(i == H // 2 - 1),
                                perf_mode=DR,
                            )
                        t_sb = p4.tile([128, 512], F32, tag="tsb", bufs=2,
                                       name="t_sb")
                        nc.scalar.activation(
                            t_sb[:, :], pm2[:, :], AF.Identity,
                            bias=bproj_pp[:, mo:mo + 1], scale=1.0 / (PS * WS),
                        )
                        nc.vector.scalar_tensor_tensor(
                            xT[:, mo, nsl], t_sb[:, :],
                            ada_pp[:, 2, mo:mo + 1], xT[:, mo, nsl],
                            ALU.mult, ALU.add,
                        )
                    _ln_stats(tc, nc, xT, ones_col, pst4, pln4, ps_st2,
                              halves=(n,), st=st2)
                    _ln_apply(tc, nc, xT, mod12T, st2, ada_pp, 3, 4, pln4,
                              halves=(n,))

        # ================= phase E: FFN =====================================
        es_e = ExitStack()
        ph = es_e.enter_context(tc.tile_pool(name="ph", bufs=1))
        hT = ph.tile([128, MH, NT], FP8, name="hT")
        po = es_e.enter_context(tc.tile_pool(name="po", bufs=1))

        with tc.tile_pool(name="ps_f1", bufs=3, space="PSUM") as ps_f1, \
             tc.tile_pool(name="ps_f2", bufs=3, space="PSUM") as ps_f2, \
             tc.tile_pool(name="ps_tro", bufs=2, space="PSUM") as ps_tro:
            # fc1 in 18 chunks of 256 columns (2 m-tiles each), weights
            # already converted to fp8 in DRAM during the attention window
            with tc.tile_pool(name="p5a", bufs=1) as p5a:
                for ch in range(18):
                    f18 = p5a.tile([128, KTP, 256], FP8, tag="f18", bufs=3,
                                   name="f18")
                    nc.sync.dma_start(f18[:, 0:KT, :], w1f8_dr[ch, :, :, :])
                    nc.gpsimd.memset(f18[:, KT, :], 0.0)
                    if 2 <= ch < 12:
                        emit_fc2_chunk(6 + ch,
                                       nc.gpsimd if ch % 2 else nc.vector,
                                       p5a)
                    for m in range(2):
                        mo = ch * 2 + m
                        for n in range(2):
                            nsl = slice(n * 512, (n + 1) * 512)
                            pf1 = ps_f1.tile([128, 512], F32, tag="f1",
                                             name="pf1")
                            for i in range(KTP // 2):
                                nc.tensor.matmul(
                                    pf1[:, :],
                                    f18[:, 2 * i:2 * i + 2,
                                        m * 128:(m + 1) * 128],
                                    mod12T[:, 2 * i:2 * i + 2, nsl],
                                    start=(i == 0), stop=(i == KTP // 2 - 1),
                                    perf_mode=DR,
                                )
                            nc.scalar.activation(
                                hT[:, mo, nsl], pf1[:, :], AF.Gelu_apprx_tanh,
                                bias=bfc1_pp[:, mo:mo + 1],
                                scale=1.0 / (AS * WS),
                            )
            # fc2: weights already fp8-resident in SBUF (w2f8)
            with tc.tile_pool(name="p5b", bufs=1) as p5b:
                for mo in range(KT):
                    for n in range(2):
                        nsl = slice(n * 512, (n + 1) * 512)
                        pf2 = ps_f2.tile([128, 512], F32, tag="f2", name="pf2")
                        for i in range(MH // 2):
                            nc.tensor.matmul(
                                pf2[:, :], w2f8[:, mo, 2 * i:2 * i + 2, :],
                                hT[:, 2 * i:2 * i + 2, nsl],
                                start=(i == 0), stop=(i == MH // 2 - 1),
                                perf_mode=DR,
                            )
                        t2 = p5b.tile([128, 512], F32, tag="t2", bufs=3,
                                      name="t2")
                        nc.scalar.activation(
                            t2[:, :], pf2[:, :], AF.Identity,
                            bias=bfc2_pp[:, mo:mo + 1], scale=1.0 / WS,
                        )
                        nc.vector.scalar_tensor_tensor(
                            xT[:, mo, nsl], t2[:, :], ada_pp[:, 5, mo:mo + 1],
                            xT[:, mo, nsl], ALU.mult, ALU.add,
                        )
                    o_slab = po.tile([128, NT // 128, 128], F32, tag="osl",
                                     bufs=2, name="o_slab")
                    for tt in range(NT // 128):
                        pt = ps_tro.tile([128, 128], F32, tag="tro",
                                         name="pt6")
                        nc.tensor.transpose(
                            pt[:, :], xT[:, mo, tt * 128:(tt + 1) * 128],
                            ident[:, :],
                        )
                        dst = o_slab[:, tt, :]
                        nc.vector.tensor_copy(dst, pt[:, :])
                    nc.scalar.dma_start(
                        out_dram[:, mo * 128:(mo + 1) * 128]
                        .rearrange("(t p) m -> p t m", p=128),
                        o_slab[:, :, :])
        es_w2.close()
        es_wp.close()
        es_att.close()
        es_e.close()


_LOCK = threading.Lock()
_PROG = None


def _get_program():
    global _PROG
    with _LOCK:
        if _PROG is None:
            _PROG = _build_program()
    return _PROG


def _make_in_maps(inputs):
    arrs = {k: np.ascontiguousarray(np.asarray(v, dtype=np.float32))
            for k, v in inputs.items()}
    in_maps = []
    for c in range(NCORES):
        m = {k: v for k, v in arrs.items() if k not in ("x", "t_emb")}
        m["x"] = np.ascontiguousarray(arrs["x"][c])
        m["t_emb"] = np.ascontiguousarray(arrs["t_emb"][c])
        in_maps.append(m)
    return in_maps


def kernel(**inputs):
    nc = _get_program()
    res = run_bass_kernel_spmd(nc, _make_in_maps(inputs),
                               core_ids=list(range(NCORES)))
    return np.stack([r["out"] for r in res.results], axis=0)


def kernel_traced(inputs, **kw):
    """test-harness helper: returns full BassKernelResults with trace."""
    nc = _get_program()
    return run_bass_kernel_spmd(
        nc, _make_in_maps(inputs), core_ids=list(range(NCORES)), trace=True,
        **kw
    )
